# revision 1
# baseline (speedup 1.0000x reference)
"""Trainium2 Bass kernel for nn_EntityResolutionProcessor.

Strategy: data-parallel over mentions (M=1024 -> 128/core on 8 cores).
On-device per core:
  phase0: blocked cumsum of text -> csum scratch in DRAM (f32); indirect-DMA
          gather of 4 csum rows per mention; mention/context means (f32);
          weights + candidates converted to bf16 DRAM scratch.
  per-mention: feature-major projections (relik-W1a, q/k/v, uni-U1a), s_aa.
  8 macro-tiles of 512 pairs: candidate DMA-transpose, q/k/v projections,
          relik/unirel heads, 2-token attention via sigmoid softmax,
          wo + LN1, FFN, LN2+cosine fused via sufficient statistics.
Activations feature-major [feat->6x128 partitions, pairs]. Matmul operands
bf16 (fp32 psum accumulate); cumsum/means/layernorm lane math in fp32.
"""

from contextlib import ExitStack

import ml_dtypes
import numpy as np

import concourse.bass as bass
import concourse.mybir as mybir
import concourse.tile as tile
from concourse import bacc, bass_isa, bass_utils
from concourse.bass import IndirectOffsetOnAxis, ds, ts

S, D, M, K, H = 4096, 768, 1024, 32, 8
DH = D // H
CTX = 10
NCORES = 8
P = 128
FC = D // P                     # 6 feature chunks
HFC = 4 * D // P                # 24 ffn hidden chunks
M_LOC = M // NCORES             # 128 mentions per core
PAIRS = M_LOC * K               # 4096 pairs per core
NP = 512                        # pairs per macro tile
G = NP // K                     # 16 mentions per macro tile
NMACRO = PAIRS // NP            # 8
NCH = S // P                    # 32 text chunks
ISQ = 1.0 / float(np.sqrt(np.float32(DH)))
EPS_LN = 1e-5
EPS_COS = 1e-8

F32 = mybir.dt.float32
BF16 = mybir.dt.bfloat16
I32 = mybir.dt.int32
AF = mybir.ActivationFunctionType
ALU = mybir.AluOpType

_NC_CACHE = {}


def _gk(ap):
    """view a [128, NP] AP as [128, G, K]"""
    return ap.rearrange("p (g k) -> p g k", g=G)


def _feat_major(w_ap):
    """[in, out] dram AP -> [128, in//128, out] (partition = in % 128)"""
    return w_ap.rearrange("(i p) o -> p i o", p=P)


def _vec6(v_ap, n=FC):
    """[D] dram AP -> [128, n] per-feature layout"""
    return v_ap.rearrange("(i p) -> p i", p=P)


def _build_nc():
    nc = bacc.Bacc(
        "TRN2", target_bir_lowering=False, debug=False, num_devices=NCORES
    )

    def inp(name, shape, dtype=F32):
        return nc.dram_tensor(name, list(shape), dtype, kind="ExternalInput").ap()

    t = {}
    t["txt"] = inp("txt", [S, D])
    t["cand"] = inp("cand", [PAIRS, D])
    t["idx"] = inp("idx", [P, 4], I32)
    t["invl"] = inp("invl", [P, 2])
    t["seltab"] = inp("seltab", [NCH, 2, P])
    t["tri"] = inp("tri", [P, P])          # upper-tri incl (lhsT of L)
    t["tri32"] = inp("tri32", [NCH, NCH])  # strict upper (lhsT of strict L)
    t["ident"] = inp("ident", [P, P])
    t["identb"] = inp("identb", [P, P], BF16)
    t["zrow"] = inp("zrow", [1, D])
    t["hmat"] = inp("hmat", [D, H], BF16)  # head indicator
    t["i8neg"] = inp("i8neg", [H, H], BF16)

    for n, shp in [("relik_w1", [2 * D, D]), ("relik_b1", [D]),
                   ("relik_w2", [D, 1]), ("relik_b2", [1, 1]),
                   ("wq", [D, D]), ("bq", [D]), ("wk", [D, D]), ("bk", [D]),
                   ("wv", [D, D]), ("bv", [D]), ("wo", [D, D]), ("bo", [D]),
                   ("ln1_g", [D]), ("ln1_b", [D]),
                   ("ffn_w1", [D, 4 * D]), ("ffn_b1", [4 * D]),
                   ("ffn_w2", [4 * D, D]), ("ffn_b2", [D]),
                   ("ln2_g", [D]), ("ln2_b", [D]),
                   ("uni_w1", [2 * D, D]), ("uni_b1", [D]),
                   ("uni_w2", [D, D]), ("uni_b2", [1, D])]:
        t[n] = inp(n, shp)

    t["out"] = nc.dram_tensor("out", [3, PAIRS], F32, kind="ExternalOutput").ap()
    t["csum"] = nc.dram_tensor("csum_scratch", [S + 1, D], F32).ap()
    # bf16 scratch: candidates + streamed weights (strip-major layouts)
    t["cand_bf"] = nc.dram_tensor("cand_bf", [PAIRS, D], BF16).ap()
    for n, noc, nic in [("wq_bf", FC, FC), ("wk_bf", FC, FC),
                        ("wv_bf", FC, FC), ("wo_bf", FC, FC),
                        ("w1b_bf", FC, FC), ("u1b_bf", FC, FC),
                        ("fw1_bf", HFC, FC), ("fw2_bf", FC, HFC)]:
        t[n] = nc.dram_tensor(n, [noc, P, nic * P], BF16).ap()

    with tile.TileContext(nc) as tc:
        _body(nc, tc, t)
    nc.compile()
    return nc


def _body(nc, tc, t):
    with ExitStack() as _ctx:
        _body_inner(nc, tc, t, _ctx)


def _body_inner(nc, tc, t, _ctx):
    mm = lambda *a, **k: nc.tensor.matmul(*a, **k)

    # ---------------- pools ----------------
    psum = _ctx.enter_context(tc.tile_pool(name="psum", bufs=1, space="PSUM"))
    res = _ctx.enter_context(tc.tile_pool(name="res", bufs=1))

    def ps_mm(shape=(P, NP), dtype=F32):
        return psum.tile(list(shape), dtype, tag="mm", bufs=3,
                         padded_shape=[P, NP], name="ps_mm")

    def ps_score():
        return psum.tile([8, NP], F32, tag="score", bufs=1, name="ps_score")

    def ps_stat():
        # stats tile: MM groups land at base partitions 0 and 32
        return psum.tile([P, NP], F32, tag="stat", bufs=2, name="ps_stat")

    def ps_head():
        return psum.tile([1, NP], F32, tag="head", bufs=2, name="ps_head")

    # ---------------- resident constants ----------------
    def load_res(name, ap_src, shape, dtype=F32, conv=False):
        tl = res.tile(list(shape), dtype, name=name)
        nc.gpsimd.dma_start(tl[:], ap_src)
        return tl

    tri_sb = load_res("tri_sb", t["tri"][:], [P, P])
    tri32_sb = load_res("tri32_sb", t["tri32"][:], [NCH, NCH])
    ident_sb = load_res("ident_sb", t["ident"][:], [P, P])
    identb_sb = load_res("identb_sb", t["identb"][:], [P, P], BF16)
    i8neg_sb = load_res("i8neg_sb", t["i8neg"][:], [H, H], BF16)
    # H in two layouts: lhsT for head-reduce [128,6,8]; lhsT for bcast [8,6,128]
    h_sb = load_res("h_sb", t["hmat"].rearrange("(c p) h -> p c h", p=P),
                    [P, FC, H], BF16)
    ht_sb = load_res("ht_sb", t["hmat"].rearrange("(c p) h -> h c p", p=P),
                     [H, FC, P], BF16)
    negh_sb = res.tile([P, FC, H], BF16, name="negh_sb")
    nc.vector.tensor_scalar_mul(negh_sb[:], h_sb[:], -1.0)

    idx_sb = load_res("idx_sb", t["idx"][:], [P, 4], I32)
    invl_sb = load_res("invl_sb", t["invl"][:], [P, 2])
    sel_sb = load_res("sel_sb", t["seltab"][:], [NCH, 2, P])

    bq_sb = load_res("bq_sb", _vec6(t["bq"]), [P, FC])
    bk_sb = load_res("bk_sb", _vec6(t["bk"]), [P, FC])
    bv_sb = load_res("bv_sb", _vec6(t["bv"]), [P, FC])
    bo_sb = load_res("bo_sb", _vec6(t["bo"]), [P, FC])
    rb1_sb = load_res("rb1_sb", _vec6(t["relik_b1"]), [P, FC])
    ub1_sb = load_res("ub1_sb", _vec6(t["uni_b1"]), [P, FC])
    fb1_sb = load_res("fb1_sb", _vec6(t["ffn_b1"], HFC), [P, HFC])
    fb2_sb = load_res("fb2_sb", _vec6(t["ffn_b2"]), [P, FC])
    l1g_sb = load_res("l1g_sb", _vec6(t["ln1_g"]), [P, FC])
    l1b_sb = load_res("l1b_sb", _vec6(t["ln1_b"]), [P, FC])
    l2g_sb = load_res("l2g_sb", _vec6(t["ln2_g"]), [P, FC])
    l2b_sb = load_res("l2b_sb", _vec6(t["ln2_b"]), [P, FC])
    rw2_sb = load_res("rw2_sb",
                      t["relik_w2"].rearrange("(c p) o -> p c o", p=P),
                      [P, FC, 1], BF16, conv=True)
    rb2_sb = load_res("rb2_sb", t["relik_b2"][:], [1, 1])

    ones_sb = res.tile([P, 1], BF16, name="ones_sb")
    nc.vector.memset(ones_sb[:], 1.0)
    onesf_sb = res.tile([P, 1], F32, name="onesf_sb")
    nc.vector.memset(onesf_sb[:], 1.0)
    ones_row = res.tile([1, P], BF16, name="ones_row")
    nc.vector.memset(ones_row[:], 1.0)

    # stats lhsT [128, 6, 3]: cols = [1, g2^2, g2*b2] per feature chunk
    sl3_sb = res.tile([P, FC, 3], BF16, name="sl3_sb")
    g2sq_sb = res.tile([P, FC], F32, name="g2sq_sb")
    g2b2_sb = res.tile([P, FC], F32, name="g2b2_sb")
    nc.vector.tensor_mul(g2sq_sb[:], l2g_sb[:], l2g_sb[:])
    nc.vector.tensor_mul(g2b2_sb[:], l2g_sb[:], l2b_sb[:])
    for c in range(FC):
        nc.vector.tensor_copy(sl3_sb[:, c, 0:1], ones_sb[:])
        nc.vector.tensor_copy(sl3_sb[:, c, 1:2], g2sq_sb[:, c:c + 1])
        nc.vector.tensor_copy(sl3_sb[:, c, 2:3], g2b2_sb[:, c:c + 1])

    # scalar reductions of bias/gain vectors (each -> [1,1] on partition 0)
    def vec_sum(name, vecs):
        tmp = res.tile([P, FC], F32, name=name + "_t")
        if len(vecs) == 1:
            nc.vector.tensor_copy(tmp[:], vecs[0][:])
        else:
            nc.vector.tensor_mul(tmp[:], vecs[0][:], vecs[1][:])
            for v in vecs[2:]:
                nc.vector.tensor_mul(tmp[:], tmp[:], v[:])
        red = res.tile([P, 1], F32, name=name + "_r")
        nc.vector.tensor_reduce(red[:], tmp[:], axis=mybir.AxisListType.X,
                                op=ALU.add)
        pR = ps_head()
        mm(pR[:, 0:1], red[:], onesf_sb[:], start=True, stop=True)
        arr = res.tile([1, 1], F32, name=name)
        nc.vector.tensor_copy(arr[:], pR[:, 0:1])
        return arr[0:1, 0:1]

    s_bo = vec_sum("s_bo", [bo_sb])
    s_fb2 = vec_sum("s_fb2", [fb2_sb])
    s_g2 = vec_sum("s_g2", [l2g_sb, l2g_sb])
    s_gb = vec_sum("s_gb", [l2g_sb, l2b_sb])
    s_bb = vec_sum("s_bb", [l2b_sb, l2b_sb])
    s_g2f = vec_sum("s_g2f", [l2g_sb, l2g_sb, fb2_sb])
    s_gbf = vec_sum("s_gbf", [l2g_sb, l2b_sb, fb2_sb])

    u2rs_sb = res.tile([P, FC], BF16, name="u2rs_sb")
    b2m_sb = res.tile([1, 1], F32, name="b2m_sb")

    # per-mention outputs (feature-major [128, 6, 128])
    m_T = res.tile([P, FC, P], F32, name="m_T")     # f32: residual source
    m_Tb = res.tile([P, FC, P], BF16, name="m_Tb")  # bf16: matmul rhs
    c_Tb = res.tile([P, FC, P], BF16, name="c_Tb")
    m_q = res.tile([P, FC, P], BF16, name="m_q")
    m_k = res.tile([P, FC, P], BF16, name="m_k")
    m_v = res.tile([P, FC, P], BF16, name="m_v")
    m_relik = res.tile([P, FC, P], BF16, name="m_relik")
    c_uni = res.tile([P, FC, P], BF16, name="c_uni")
    s_aa_sb = res.tile([H, P], BF16, name="s_aa_sb")

    # ================= phase 0: csum + gather + bf16 conversion ==========
    with tc.tile_pool(name="p0", bufs=1) as p0:
        # uni_w2 row-sums (once)
        u2_sb = p0.tile([P, FC, D], F32, name="u2_sb")
        nc.gpsimd.dma_start(u2_sb[:], _feat_major(t["uni_w2"]))
        u2r_f = p0.tile([P, FC], F32, name="u2r_f")
        nc.vector.tensor_reduce(u2r_f[:], u2_sb[:],
                                axis=mybir.AxisListType.X, op=ALU.add)
        nc.vector.tensor_copy(u2rs_sb[:], u2r_f[:])
        ub2_sb = p0.tile([1, D], F32, name="ub2_sb")
        nc.gpsimd.dma_start(ub2_sb[:], t["uni_b2"][:])
        b2r = p0.tile([1, 1], F32, name="b2r")
        nc.vector.tensor_reduce(b2r[:], ub2_sb[:], axis=mybir.AxisListType.X,
                                op=ALU.add)
        nc.scalar.activation(b2m_sb[:], b2r[:], AF.Copy, scale=1.0 / D)

        # ---- bf16 weight conversion into strip-major scratch ----
        for src_ap, dst, noc, nic in [
            (_feat_major(t["wq"]), t["wq_bf"], FC, FC),
            (_feat_major(t["wk"]), t["wk_bf"], FC, FC),
            (_feat_major(t["wv"]), t["wv_bf"], FC, FC),
            (_feat_major(t["wo"]), t["wo_bf"], FC, FC),
            (_feat_major(t["relik_w1"][D:]), t["w1b_bf"], FC, FC),
            (_feat_major(t["uni_w1"][D:]), t["u1b_bf"], FC, FC),
            (_feat_major(t["ffn_w1"]), t["fw1_bf"], HFC, FC),
            (_feat_major(t["ffn_w2"]), t["fw2_bf"], FC, HFC),
        ]:
            for oc in range(noc):
                nc.gpsimd.dma_start(
                    dst[oc].rearrange("p (i q) -> p i q", q=P),
                    src_ap[:, :, ts(oc, P)])

        # ---- candidates to bf16 (converting DRAM->DRAM DMA) ----
        for c in range(4):
            q = PAIRS // 4
            nc.gpsimd.dma_start(t["cand_bf"][c * q:(c + 1) * q, :],
                                t["cand"][c * q:(c + 1) * q, :])

        # ---- cumsum ----
        totals_sb = p0.tile([NCH, D], F32, name="totals_sb")
        nc.gpsimd.dma_start(t["csum"][0:1, :], t["zrow"][:])

        for c in range(NCH):
            txt_c = p0.tile([P, D], F32, tag="txtc", bufs=3, name="txt_c")
            nc.gpsimd.dma_start(txt_c[:], t["txt"][c * P:(c + 1) * P, :])
            pre_sb = p0.tile([P, D], F32, tag="pre", bufs=3, name="pre_sb")
            for half in range(2):
                sl = ds(half * 384, 384)
                pA = ps_mm((P, 384))
                mm(pA[:], tri_sb[:], txt_c[:, sl], start=True, stop=True)
                nc.any.tensor_copy(pre_sb[:, sl], pA[:])
            nc.gpsimd.dma_start(t["csum"][1 + c * P: 1 + (c + 1) * P, :],
                                pre_sb[:])
            nc.gpsimd.dma_start(totals_sb[c:c + 1, :], pre_sb[P - 1:P, :])

        offs_sb = p0.tile([NCH, D], F32, name="offs_sb")
        for half in range(2):
            sl = ds(half * 384, 384)
            pA = ps_mm((NCH, 384))
            mm(pA[:], tri32_sb[:], totals_sb[:, sl], start=True, stop=True)
            nc.any.tensor_copy(offs_sb[:, sl], pA[:])

        # ---- gathers + means ----
        gath = []
        for j in range(4):
            g_t = p0.tile([P, D], F32, tag=f"g{j}", name=f"g_{j}")
            nc.gpsimd.indirect_dma_start(
                out=g_t[:], out_offset=None, in_=t["csum"][:],
                in_offset=IndirectOffsetOnAxis(ap=idx_sb[:, j:j + 1], axis=0),
            )
            gath.append(g_t)

        def mean_tile(out_name, gp, gm, selcol, inv_col):
            o_t = p0.tile([P, D], F32, name=out_name)
            dif = p0.tile([P, D], F32, tag="dif", bufs=2, name="dif")
            nc.vector.tensor_tensor(dif[:], gath[gp][:], gath[gm][:],
                                    op=ALU.subtract)
            for half in range(2):
                sl = ds(half * 384, 384)
                pA = ps_mm((P, 384))
                mm(pA[:], sel_sb[:, selcol, :], offs_sb[:, sl],
                   start=True, stop=True)
                nc.vector.tensor_tensor(o_t[:, sl], pA[:], dif[:, sl],
                                        op=ALU.add)
            nc.vector.tensor_scalar_mul(o_t[:], o_t[:],
                                        invl_sb[:, inv_col:inv_col + 1])
            return o_t

        mention_rm = mean_tile("mention_rm", 0, 1, 0, 0)
        ctx_rm = mean_tile("ctx_rm", 2, 3, 1, 1)

        for src, dstf, dstb in ((mention_rm, m_T, m_Tb),
                                (ctx_rm, None, c_Tb)):
            for fc in range(FC):
                pT = ps_mm((P, P))
                nc.tensor.transpose(pT[:], src[:, ts(fc, P)], ident_sb[:])
                if dstf is not None:
                    nc.vector.tensor_copy(dstf[:, fc, :], pT[:])
                nc.any.tensor_copy(dstb[:, fc, :], pT[:])

    # ================= pools for the main phase =================
    wts = _ctx.enter_context(tc.tile_pool(name="wts", bufs=1))
    act = _ctx.enter_context(tc.tile_pool(name="act", bufs=1))
    lane = _ctx.enter_context(tc.tile_pool(name="lane", bufs=1))

    def load_strip(bf_dram, oc):
        """stream bf16 weight strip [128, 6, 128] for out-chunk oc"""
        st = wts.tile([P, FC, P], BF16, tag="wstrip", bufs=6, name="w_strip")
        nc.gpsimd.dma_start(st[:],
                          bf_dram[oc].rearrange("p (i q) -> p i q", q=P))
        return st

    def load_strip_conv(w_fm_ap, oc):
        """one-shot converting load (per-mention phase)"""
        st = wts.tile([P, FC, P], BF16, tag="wstrip", bufs=6, name="w_strip")
        nc.gpsimd.dma_start(st[:], w_fm_ap[:, :, ts(oc, P)])
        return st

    def unit(tag, name, bufs=1):
        return act.tile([P, FC, NP], BF16, tag=tag, bufs=bufs, name=name)

    def chunk_t(name):
        return act.tile([P, NP], BF16, tag="tt", bufs=3, name=name)

    # ---------- per-mention projections (bf16, N=128) ----------
    for w_ap, b_sb, out_t, src in (
        (_feat_major(t["wq"]), bq_sb, m_q, m_Tb),
        (_feat_major(t["wk"]), bk_sb, m_k, m_Tb),
        (_feat_major(t["wv"]), bv_sb, m_v, m_Tb),
        (_feat_major(t["relik_w1"][:D]), rb1_sb, m_relik, m_Tb),
        (_feat_major(t["uni_w1"][:D]), ub1_sb, c_uni, c_Tb),
    ):
        for oc in range(FC):
            st = load_strip_conv(w_ap, oc)
            pA = ps_mm((P, P))
            for ic in range(FC):
                mm(pA[:], st[:, ic, :], src[:, ic, :],
                   start=(ic == 0), stop=(ic == FC - 1))
            nc.scalar.activation(out_t[:, oc, :], pA[:], AF.Identity,
                                 bias=b_sb[:, oc:oc + 1])

    # s_aa [8, 128]
    mprod = wts.tile([P, FC, P], BF16, tag="wstrip", bufs=6, name="mprod")
    for c in range(FC):
        nc.vector.tensor_mul(mprod[:, c, :], m_q[:, c, :], m_k[:, c, :])
    pS = ps_score()
    for c in range(FC):
        mm(pS[:, :P], h_sb[:, c, :], mprod[:, c, :],
           start=(c == 0), stop=(c == FC - 1))
    nc.any.tensor_copy(s_aa_sb[:], pS[:, :P])

    # ================= macro-tile loop =================
    for mt in range(NMACRO):
        g0 = mt * G
        gsl = ds(g0, G)

        lane_seq = [0]

        def lane_t(name, parts=1):
            lane_seq[0] += 1
            return lane.tile([parts, NP], F32, tag=name, bufs=1,
                             name=f"{name}_{lane_seq[0]}")

        def mview(mt_tile, c):
            """mention-side bcast view [128, G, K]"""
            return mt_tile[:, c, gsl, None].to_broadcast([P, G, K])

        # ---- candidate load + PE transpose (bf16) ----
        cand_rm = act.tile([P, 4, D], BF16, tag="cand_rm", bufs=1,
                           name="cand_rm")
        nc.gpsimd.dma_start(
            cand_rm[:],
            t["cand_bf"].rearrange("(q p) d -> p q d", p=P)[:, ds(4 * mt, 4), :])
        candT = unit("candT", "candT")
        for fc in range(FC):
            pT = ps_mm(dtype=BF16)
            for pc in range(4):
                nc.tensor.transpose(pT[:, ts(pc, P)],
                                    cand_rm[:, pc, ts(fc, P)], identb_sb[:])
            nc.vector.tensor_copy(candT[:, fc, :], pT[:])

        # ---- k/v projections ----
        k_b = unit("B", "k_b")
        v_b = unit("C", "v_b")
        for wbf, b_sb, out_t in ((t["wk_bf"], bk_sb, k_b),
                                 (t["wv_bf"], bv_sb, v_b)):
            for oc in range(FC):
                st = load_strip(wbf, oc)
                pA = ps_mm()
                for ic in range(FC):
                    mm(pA[:], st[:, ic, :], candT[:, ic, :],
                       start=(ic == 0), stop=(ic == FC - 1))
                nc.scalar.activation(out_t[:, oc, :], pA[:], AF.Identity,
                                     bias=b_sb[:, oc:oc + 1])

        # ---- relik / unirel heads ----
        for wbf, madd, hname, wv2, bias_ap, outrow, fn, scale in (
            (t["w1b_bf"], m_relik, "h_r", rw2_sb, rb2_sb[:], 0,
             AF.Identity, 1.0),
            (t["u1b_bf"], c_uni, "h_u", u2rs_sb, b2m_sb[:], 2,
             AF.Sigmoid, 1.0 / D),
        ):
            h_head = unit("hh", hname, bufs=2)
            for oc in range(FC):
                st = load_strip(wbf, oc)
                pA = ps_mm()
                for ic in range(FC):
                    mm(pA[:], st[:, ic, :], candT[:, ic, :],
                       start=(ic == 0), stop=(ic == FC - 1))
                nc.vector.tensor_tensor(_gk(h_head[:, oc, :]), _gk(pA[:]),
                                        mview(madd, oc), op=ALU.add)
                nc.scalar.activation(h_head[:, oc, :], h_head[:, oc, :],
                                     AF.Relu)
            pH = ps_head()
            for c in range(FC):
                if wv2 is rw2_sb:
                    lhsT = wv2[:, c, :]
                else:
                    lhsT = wv2[:, c:c + 1]
                mm(pH[:], lhsT, h_head[:, c, :],
                   start=(c == 0), stop=(c == FC - 1))
            osl = lane_t("osl_" + hname)
            nc.scalar.activation(osl[:], pH[:], fn, bias=bias_ap, scale=scale)
            nc.gpsimd.dma_start(t["out"][outrow:outrow + 1, ts(mt, NP)], osl[:])

        # ---- attention scores ----
        pAB = ps_score()
        for c in range(FC):
            pr1 = chunk_t("pr1")
            nc.vector.tensor_tensor(_gk(pr1[:]), _gk(k_b[:, c, :]),
                                    mview(m_q, c), op=ALU.mult)
            mm(pAB[:], h_sb[:, c, :], pr1[:], start=(c == 0), stop=False)
        mm(pAB[:], i8neg_sb[:],
           s_aa_sb[:, gsl, None].to_broadcast([H, G, K]),
           start=False, stop=True)
        p_ab = act.tile([H, NP], BF16, tag="p_ab", bufs=2, name="p_ab")
        nc.scalar.activation(p_ab[:], pAB[:], AF.Sigmoid, scale=ISQ)

        pBA = ps_score()
        first = True
        for c in range(FC):
            stq = load_strip(t["wq_bf"], c)
            pQ = ps_mm()
            for ic in range(FC):
                mm(pQ[:], stq[:, ic, :], candT[:, ic, :],
                   start=(ic == 0), stop=(ic == FC - 1))
            q_c = chunk_t("q_c")
            nc.scalar.activation(q_c[:], pQ[:], AF.Identity,
                                 bias=bq_sb[:, c:c + 1])
            pr2 = chunk_t("pr2")
            nc.vector.tensor_tensor(_gk(pr2[:]), _gk(q_c[:]), mview(m_k, c),
                                    op=ALU.mult)
            mm(pBA[:], h_sb[:, c, :], pr2[:], start=first, stop=False)
            first = False
            pr3 = chunk_t("pr3")
            nc.vector.tensor_mul(pr3[:], q_c[:], k_b[:, c, :])
            mm(pBA[:], negh_sb[:, c, :], pr3[:],
               start=False, stop=(c == FC - 1))
        p_ba = act.tile([H, NP], BF16, tag="p_ba", bufs=2, name="p_ba")
        nc.scalar.activation(p_ba[:], pBA[:], AF.Sigmoid, scale=ISQ)

        # ---- attention outputs ----
        o_a = unit("F", "o_a")
        o_b = unit("G", "o_b")
        for c in range(FC):
            dv = chunk_t("dv")
            nc.vector.tensor_tensor(_gk(dv[:]), _gk(v_b[:, c, :]),
                                    mview(m_v, c), op=ALU.subtract)
            pBC = ps_mm()
            mm(pBC[:], ht_sb[:, c, :], p_ab[:], start=True, stop=True)
            nc.vector.tensor_mul(o_a[:, c, :], pBC[:], dv[:])
            nc.vector.tensor_tensor(_gk(o_a[:, c, :]), _gk(o_a[:, c, :]),
                                    mview(m_v, c), op=ALU.add)
            pBC2 = ps_mm()
            mm(pBC2[:], ht_sb[:, c, :], p_ba[:], start=True, stop=True)
            nc.vector.tensor_mul(o_b[:, c, :], pBC2[:], dv[:])
            nc.vector.tensor_tensor(o_b[:, c, :], v_b[:, c, :], o_b[:, c, :],
                                    op=ALU.subtract)

        # ---- wo + residual ----
        r_a = unit("hh", "r_a", bufs=2)
        r_b = unit("hh", "r_b", bufs=2)
        for oc in range(FC):
            st = load_strip(t["wo_bf"], oc)
            pA = ps_mm()
            for ic in range(FC):
                mm(pA[:], st[:, ic, :], o_a[:, ic, :],
                   start=(ic == 0), stop=(ic == FC - 1))
            nc.vector.tensor_tensor(_gk(r_a[:, oc, :]), _gk(pA[:]),
                                    mview(m_T, oc), op=ALU.add)
            pB = ps_mm()
            for ic in range(FC):
                mm(pB[:], st[:, ic, :], o_b[:, ic, :],
                   start=(ic == 0), stop=(ic == FC - 1))
            nc.vector.tensor_tensor(r_b[:, oc, :], pB[:], candT[:, oc, :],
                                    op=ALU.add)

        # ---- LN1 (general gains) -> x1 ----
        def layernorm1(r_t, x1_t, tok):
            pSt = ps_stat()
            for c in range(FC):
                sq = chunk_t("sq")
                nc.scalar.activation(sq[:], r_t[:, c, :], AF.Square,
                                     bias=bo_sb[:, c:c + 1])
                mm(pSt[0:1, :], ones_sb[:], r_t[:, c, :],
                   start=(c == 0), stop=(c == FC - 1))
                mm(pSt[32:33, :], ones_sb[:], sq[:],
                   start=(c == 0), stop=(c == FC - 1))
            mu = lane_t("mu" + tok)
            nc.vector.tensor_scalar(mu[:], pSt[0:1, :], s_bo, 1.0 / D,
                                    op0=ALU.add, op1=ALU.mult)
            var = lane_t("var" + tok)
            nc.vector.tensor_mul(var[:], mu[:], mu[:])
            nc.vector.scalar_tensor_tensor(var[:], pSt[32:33, :], 1.0 / D,
                                           var[:], op0=ALU.mult,
                                           op1=ALU.subtract)
            rstd = lane_t("rstd" + tok)
            nc.vector.tensor_scalar_add(var[:], var[:], EPS_LN)
            nc.scalar.activation(rstd[:], var[:], AF.Sqrt)
            nc.vector.reciprocal(rstd[:], rstd[:])
            mubf = act.tile([1, NP], BF16, tag="mubf", bufs=2, name="mubf")
            rstdbf = act.tile([1, NP], BF16, tag="rstdbf", bufs=2,
                              name="rstdbf")
            nc.vector.tensor_copy(mubf[:], mu[:])
            nc.vector.tensor_copy(rstdbf[:], rstd[:])
            mu_bc = ps_mm()
            rstd_bc = ps_mm()
            mm(mu_bc[:], ones_row[:], mubf[:], start=True, stop=True)
            mm(rstd_bc[:], ones_row[:], rstdbf[:], start=True, stop=True)
            for c in range(FC):
                nc.vector.tensor_tensor(x1_t[:, c, :], r_t[:, c, :],
                                        mu_bc[:], op=ALU.subtract)
                nc.vector.scalar_tensor_tensor(
                    x1_t[:, c, :], x1_t[:, c, :], bo_sb[:, c:c + 1],
                    rstd_bc[:], op0=ALU.add, op1=ALU.mult)
                nc.vector.tensor_scalar(
                    x1_t[:, c, :], x1_t[:, c, :], l1g_sb[:, c:c + 1],
                    l1b_sb[:, c:c + 1], op0=ALU.mult, op1=ALU.add)

        x1_a = unit("A", "x1_a")
        x1_b = unit("B", "x1_b")
        layernorm1(r_a, x1_a, "a")
        layernorm1(r_b, x1_b, "b")

        # ---- FFN (both tokens share each weight strip) ----
        h_a = act.tile([P, HFC, NP], BF16, tag="h", bufs=1, name="h_a")
        # token-b hidden aliases four unit tags that are dead by now
        hb = [unit("candT", "hb0"), unit("G", "hb1"),
              unit("F", "hb2"), unit("hh", "hb3", bufs=2)]

        def ha_c(hc):
            return h_a[:, hc, :]

        def hb_c(hc):
            return hb[hc // FC][:, hc % FC, :]

        for hc in range(HFC):
            st = load_strip(t["fw1_bf"], hc)
            for x1_t, hcs in ((x1_a, ha_c), (x1_b, hb_c)):
                pA = ps_mm()
                for ic in range(FC):
                    mm(pA[:], st[:, ic, :], x1_t[:, ic, :],
                       start=(ic == 0), stop=(ic == FC - 1))
                nc.scalar.activation(hcs(hc), pA[:],
                                     AF.Relu, bias=fb1_sb[:, hc:hc + 1])
        r2_a = unit("C2", "r2_a")
        r2_b = unit("D", "r2_b")
        for oc in range(FC):
            stw = wts.tile([P, HFC, P], BF16, tag="w2strip", bufs=2,
                           name="stw")
            nc.gpsimd.dma_start(
                stw[:],
                t["fw2_bf"][oc].rearrange("p (i q) -> p i q", q=P))
            for x1_t, hcs, r2_t in ((x1_a, ha_c, r2_a), (x1_b, hb_c, r2_b)):
                pA = ps_mm()
                for hc in range(HFC):
                    mm(pA[:], stw[:, hc, :], hcs(hc),
                       start=(hc == 0), stop=(hc == HFC - 1))
                nc.vector.tensor_tensor(r2_t[:, oc, :], pA[:],
                                        x1_t[:, oc, :], op=ALU.add)

        # ---- LN2 + cosine via sufficient statistics ----
        def ln2_stats(r2_t, tok):
            pSt = ps_stat()
            for c in range(FC):
                sq = chunk_t("sq")
                nc.scalar.activation(sq[:], r2_t[:, c, :], AF.Square,
                                     bias=fb2_sb[:, c:c + 1])
                mm(pSt[0:1, :], sl3_sb[:, c, 0:1], r2_t[:, c, :],
                   start=(c == 0), stop=(c == FC - 1))
                mm(pSt[32:33, :], sl3_sb[:, c, 1:2], r2_t[:, c, :],
                   start=(c == 0), stop=(c == FC - 1))
                mm(pSt[64:65, :], sl3_sb[:, c, 2:3], r2_t[:, c, :],
                   start=(c == 0), stop=(c == FC - 1))
                mm(pSt[96:97, :], sl3_sb[:, c, 0:1], sq[:],
                   start=(c == 0), stop=(c == FC - 1),
                   tile_position=(0, 96))
            pS2 = ps_stat()
            for c in range(FC):
                sq2 = chunk_t("sq2")
                nc.scalar.activation(sq2[:], r2_t[:, c, :], AF.Square,
                                     bias=fb2_sb[:, c:c + 1])
                mm(pS2[0:1, :], sl3_sb[:, c, 1:2], sq2[:],
                   start=(c == 0), stop=(c == FC - 1))
            # evict the five stats rows into base-0 lane tiles, folding the
            # constant fb2 corrections
            sz = lane_t("sz" + tok)
            nc.vector.tensor_scalar_add(sz[:], pSt[0:1, :], s_fb2)
            g2z = lane_t("g2z" + tok)
            nc.vector.tensor_scalar_add(g2z[:], pSt[32:33, :], s_g2f)
            gbz = lane_t("gbz" + tok)
            nc.vector.tensor_scalar_add(gbz[:], pSt[64:65, :], s_gbf)
            sq_s = lane_t("sq" + tok)
            nc.vector.tensor_copy(sq_s[:], pSt[96:97, :])
            g2q = lane_t("g2q" + tok)
            nc.vector.tensor_copy(g2q[:], pS2[0:1, :])
            return sz, g2z, gbz, sq_s, g2q

        stats_a = ln2_stats(r2_a, "a")
        stats_b = ln2_stats(r2_b, "b")
        pX = ps_head()
        for c in range(FC):
            rr = chunk_t("rr")
            nc.vector.tensor_scalar_add(rr[:], r2_b[:, c, :],
                                        fb2_sb[:, c:c + 1])
            nc.vector.scalar_tensor_tensor(rr[:], r2_a[:, c, :],
                                           fb2_sb[:, c:c + 1], rr[:],
                                           op0=ALU.add, op1=ALU.mult)
            mm(pX[:], sl3_sb[:, c, 1:2], rr[:],
               start=(c == 0), stop=(c == FC - 1))

        # lane algebra for cosine
        def ln2_lane(stats, tok):
            sz, g2z, gbz, sq_s, g2q = stats
            muz = lane_t("muz" + tok)
            nc.vector.tensor_scalar_mul(muz[:], sz[:], 1.0 / D)
            var = lane_t("var2" + tok)
            nc.vector.tensor_mul(var[:], muz[:], muz[:])
            nc.vector.scalar_tensor_tensor(var[:], sq_s[:], 1.0 / D,
                                           var[:], op0=ALU.mult,
                                           op1=ALU.subtract)
            rstd = lane_t("rstd2" + tok)
            nc.vector.tensor_scalar_add(var[:], var[:], EPS_LN)
            nc.scalar.activation(rstd[:], var[:], AF.Sqrt)
            nc.vector.reciprocal(rstd[:], rstd[:])
            return muz, rstd, g2z, gbz, g2q

        mua, rsta, g2za, gbza, g2qa = ln2_lane(stats_a, "a")
        mub2, rstb, g2zb, gbzb, g2qb = ln2_lane(stats_b, "b")

        def gbt(mu, rstd, gbz, name):
            o_t = lane_t(name)
            nc.vector.tensor_scalar_mul(o_t[:], mu[:], s_gb)
            nc.vector.tensor_tensor(o_t[:], gbz[:], o_t[:], op=ALU.subtract)
            nc.vector.tensor_mul(o_t[:], o_t[:], rstd[:])
            return o_t

        gbta = gbt(mua, rsta, gbza, "gbta")
        gbtb = gbt(mub2, rstb, gbzb, "gbtb")

        def normsq(mu, rstd, g2z, g2q, gbt_t, name):
            o_t = lane_t(name)
            nc.vector.tensor_scalar_mul(o_t[:], mu[:], s_g2)
            nc.vector.scalar_tensor_tensor(o_t[:], g2z[:], -2.0, o_t[:],
                                           op0=ALU.mult, op1=ALU.add)
            nc.vector.tensor_mul(o_t[:], o_t[:], mu[:])
            nc.vector.tensor_add(o_t[:], o_t[:], g2q[:])
            nc.vector.tensor_mul(o_t[:], o_t[:], rstd[:])
            nc.vector.tensor_mul(o_t[:], o_t[:], rstd[:])
            nc.vector.scalar_tensor_tensor(o_t[:], gbt_t[:], 2.0, o_t[:],
                                           op0=ALU.mult, op1=ALU.add)
            nc.vector.tensor_scalar_add(o_t[:], o_t[:], s_bb)
            return o_t

        n2a = normsq(mua, rsta, g2za, g2qa, gbta, "n2a")
        n2b = normsq(mub2, rstb, g2zb, g2qb, gbtb, "n2b")

        d01 = lane_t("d01")
        nc.vector.tensor_scalar_mul(d01[:], mub2[:], s_g2)
        nc.vector.tensor_tensor(d01[:], d01[:], g2zb[:], op=ALU.subtract)
        nc.vector.tensor_mul(d01[:], d01[:], mua[:])
        t2 = lane_t("t2")
        nc.vector.tensor_mul(t2[:], mub2[:], g2za[:])
        nc.vector.tensor_tensor(d01[:], d01[:], t2[:], op=ALU.subtract)
        nc.vector.tensor_tensor(d01[:], pX[:], d01[:], op=ALU.add)
        nc.vector.tensor_mul(d01[:], d01[:], rsta[:])
        nc.vector.tensor_mul(d01[:], d01[:], rstb[:])
        nc.vector.tensor_add(d01[:], d01[:], gbta[:])
        nc.vector.tensor_add(d01[:], d01[:], gbtb[:])
        nc.vector.tensor_scalar_add(d01[:], d01[:], s_bb)

        den = lane_t("den")
        nc.scalar.activation(n2a[:], n2a[:], AF.Sqrt)
        nc.vector.tensor_scalar_max(n2a[:], n2a[:], EPS_COS)
        nc.scalar.activation(n2b[:], n2b[:], AF.Sqrt)
        nc.vector.tensor_scalar_max(n2b[:], n2b[:], EPS_COS)
        nc.vector.tensor_mul(den[:], n2a[:], n2b[:])
        nc.vector.reciprocal(den[:], den[:])
        atg_sl = lane_t("atg_sl")
        nc.vector.tensor_mul(atg_sl[:], d01[:], den[:])
        nc.gpsimd.dma_start(t["out"][1:2, ts(mt, NP)], atg_sl[:])


# ===================== host side =====================

def kernel(**inputs):
    f32 = np.float32
    bf16 = ml_dtypes.bfloat16
    txt = np.ascontiguousarray(
        np.asarray(inputs["text_embeddings"], f32).reshape(S, D))
    cand_full = np.ascontiguousarray(
        np.asarray(inputs["candidate_embeddings"], f32).reshape(M * K, D))
    starts = np.asarray(inputs["mention_starts"], np.int64)
    spans = np.asarray(inputs["span_lengths"], np.int64)
    ends = starts + spans

    j = np.stack([ends + 1, starts,
                  np.minimum(S - 1, ends + CTX),
                  np.maximum(0, starts - CTX)], axis=1)       # [M, 4]
    chunk_of = (np.maximum(j - 1, 0) // P).astype(np.int64)   # [M, 4]
    inv = np.stack([1.0 / (spans + 1).astype(f32),
                    1.0 / (j[:, 2] - j[:, 3]).astype(f32)], axis=1)

    consts = {
        "tri": np.triu(np.ones((P, P), f32)),
        "tri32": np.triu(np.ones((NCH, NCH), f32), k=1),
        "ident": np.eye(P, dtype=f32),
        "identb": np.eye(P, dtype=f32).astype(bf16),
        "zrow": np.zeros((1, D), f32),
        "hmat": np.repeat(np.eye(H, dtype=f32), DH, axis=0).astype(bf16),
        "i8neg": (-np.eye(H, dtype=f32)).astype(bf16),
    }
    wnames = ["relik_w1", "relik_b1", "relik_w2",
              "wq", "bq", "wk", "bk", "wv", "bv", "wo", "bo",
              "ln1_g", "ln1_b", "ffn_w1", "ffn_b1", "ffn_w2", "ffn_b2",
              "ln2_g", "ln2_b", "uni_w1", "uni_b1", "uni_w2"]
    weights = {n: np.ascontiguousarray(np.asarray(inputs[n], f32))
               for n in wnames}
    weights["relik_b2"] = np.asarray(inputs["relik_b2"], f32).reshape(1, 1)
    weights["uni_b2"] = np.ascontiguousarray(
        np.asarray(inputs["uni_b2"], f32).reshape(1, D))

    in_maps = []
    for core in range(NCORES):
        sl = slice(core * M_LOC, (core + 1) * M_LOC)
        selt = np.zeros((NCH, 2, P), f32)
        jc = chunk_of[sl]                                     # [128, 4]
        ar = np.arange(P)
        for col, (tp, tm) in enumerate(((0, 1), (2, 3))):
            np.add.at(selt, (jc[:, tp], col, ar), 1.0)
            np.add.at(selt, (jc[:, tm], col, ar), -1.0)
        im = {
            "txt": txt,
            "cand": cand_full[core * PAIRS:(core + 1) * PAIRS],
            "idx": np.ascontiguousarray(j[sl].astype(np.int32)),
            "invl": np.ascontiguousarray(inv[sl].astype(f32)),
            "seltab": selt,
        }
        im.update(consts)
        im.update(weights)
        in_maps.append(im)

    if "nc" not in _NC_CACHE:
        _NC_CACHE["nc"] = _build_nc()
    nc = _NC_CACHE["nc"]

    results = bass_utils.run_bass_kernel_spmd(
        nc, in_maps, core_ids=list(range(NCORES))).results

    out = np.zeros((3, M, K), f32)
    for core in range(NCORES):
        sl = slice(core * M_LOC, (core + 1) * M_LOC)
        out[:, sl, :] = results[core]["out"].reshape(3, M_LOC, K)
    return out


if __name__ == "__main__":
    nc = _build_nc()
    print("built ok")



# revision 27
# speedup vs baseline: 1.6861x; 1.6861x over previous
"""Trainium2 Bass kernel for nn_EntityResolutionProcessor.

Data-parallel over mentions (M=1024 -> 128/core on 8 cores).
Host side: weights pre-converted to bf16/fp8 strip-major layouts,
candidates pre-converted to bf16, mention/context selector matrices
(index metadata with 1/len folded) built in numpy.
Device side per core:
  phase0: stream text chunks; mention/context means as feature-major
          selector matmuls accumulated in SBUF; per-mention projections.
  8 macro-tiles of 512 pairs: candidate DMA + PE transpose, k/v/q
  projections from SBUF-resident weights, relik/unirel heads, 2-token
  attention via sigmoid softmax, wo + LN1, FFN (fp8 DoubleRow), LN2 +
  cosine via sufficient statistics with the per-pair lane algebra
  transposed to pair-major so it runs 128 lanes wide.
"""

from contextlib import ExitStack

import ml_dtypes
import numpy as np

import concourse.bass as bass
import concourse.mybir as mybir
import concourse.tile as tile
from concourse import bacc, bass_isa, bass_utils
from concourse.bass import ds, ts

S, D, M, K, H = 4096, 768, 1024, 32, 8
DH = D // H
CTX = 10
NCORES = 8
P = 128
FC = D // P                     # 6 feature chunks
HFC = 4 * D // P                # 24 ffn hidden chunks
M_LOC = M // NCORES             # 128 mentions per core
PAIRS = M_LOC * K               # 4096 pairs per core
NP = 512                        # pairs per macro tile
G = NP // K                     # 16 mentions per macro tile
NMACRO = PAIRS // NP            # 8
NCH = S // P                    # 32 text chunks
ISQ = 1.0 / float(np.sqrt(np.float32(DH)))
EPS_LN = 1e-5
EPS_COS = 1e-8

FP8_FFN = True                  # fp8 DoubleRow FFN matmuls
W_SCALE = 64.0                  # fp8 weight scale (folded out at eviction)

F32 = mybir.dt.float32
BF16 = mybir.dt.bfloat16
FP8 = mybir.dt.float8e4
I32 = mybir.dt.int32
AF = mybir.ActivationFunctionType
ALU = mybir.AluOpType
DR = mybir.MatmulPerfMode.DoubleRow

_NC_CACHE = {}

FFN_DT = FP8 if FP8_FFN else BF16


def _gk(ap):
    """view a [128, NP] AP as [128, G, K]"""
    return ap.rearrange("p (g k) -> p g k", g=G)


def _build_nc():
    nc = bacc.Bacc(
        "TRN2", target_bir_lowering=False, debug=False, num_devices=NCORES
    )

    def inp(name, shape, dtype=F32):
        return nc.dram_tensor(name, list(shape), dtype, kind="ExternalInput").ap()

    t = {}
    t["txt"] = inp("txt", [S, D])
    t["sel"] = inp("sel", [NCH, P, 2 * P])
    t["cand"] = inp("cand", [PAIRS, D], BF16)
    t["ident"] = inp("ident", [P, P])
    t["identb"] = inp("identb", [P, P], BF16)
    t["hmat"] = inp("hmat", [D, H], BF16)  # head indicator
    t["i8neg"] = inp("i8neg", [H, H], BF16)

    # resident weights [p, oc, ic, q] bf16
    for n in ["wq_r", "wk_r", "wv_r", "wo_r", "w1b_r"]:
        t[n] = inp(n, [P, FC, FC, P], BF16)
    # streamed strips
    t["w1a_s"] = inp("w1a_s", [FC, P, FC * P], BF16)
    t["u1a_s"] = inp("u1a_s", [FC, P, FC * P], BF16)
    t["u1b_s"] = inp("u1b_s", [FC, P, FC * P], BF16)
    t["fw1_s"] = inp("fw1_s", [HFC, P, FC * P], FFN_DT)
    t["fw2_s"] = inp("fw2_s", [FC, P, HFC * P], FFN_DT)

    for n, shp in [("relik_b1", [D]), ("relik_w2", [D, 1]), ("relik_b2", [1, 1]),
                   ("bq", [D]), ("bk", [D]), ("bv", [D]), ("bo", [D]),
                   ("ln1_g", [D]), ("ln1_b", [D]),
                   ("ffn_b1", [4 * D]), ("ffn_b2", [D]),
                   ("ln2_g", [D]), ("ln2_b", [D]),
                   ("uni_b1", [D]), ("uni_w2", [D, D]), ("uni_b2", [1, D])]:
        t[n] = inp(n, shp)

    t["out"] = nc.dram_tensor("out", [3, PAIRS], F32, kind="ExternalOutput").ap()

    with tile.TileContext(nc) as tc:
        _body(nc, tc, t)
    nc.compile()
    return nc


def _vec6(v_ap, n=FC):
    """[D] dram AP -> [128, n] per-feature layout"""
    return v_ap.rearrange("(i p) -> p i", p=P)


def _body(nc, tc, t):
    with ExitStack() as _ctx:
        _body_inner(nc, tc, t, _ctx)


def _body_inner(nc, tc, t, _ctx):
    mm = lambda *a, **k: nc.tensor.matmul(*a, **k)

    # ---------------- pools ----------------
    psum = _ctx.enter_context(tc.tile_pool(name="psum", bufs=1, space="PSUM"))
    res = _ctx.enter_context(tc.tile_pool(name="res", bufs=1))

    def ps_mm(shape=(P, NP), dtype=F32):
        return psum.tile(list(shape), dtype, tag="mm", bufs=3,
                         padded_shape=[P, NP], name="ps_mm")

    def ps_score():
        return psum.tile([8, NP], F32, tag="score", bufs=1, name="ps_score")

    def ps_l1():
        return psum.tile([P, NP], F32, tag="l1", bufs=1, name="ps_l1")

    def ps_l2():
        return psum.tile([P, NP], F32, tag="l2", bufs=1, name="ps_l2")

    def ps_head():
        return psum.tile([1, NP], F32, tag="head", bufs=2, name="ps_head")

    # ---------------- resident constants ----------------
    def load_res(name, ap_src, shape, dtype=F32):
        tl = res.tile(list(shape), dtype, name=name)
        nc.gpsimd.dma_start(tl[:], ap_src)
        return tl

    # resident weights (issued first; load during phase0 on Pool queue)
    w_res = {}
    for n in ["wq_r", "wk_r", "wv_r", "wo_r", "w1b_r"]:
        w_res[n] = load_res(n, t[n][:], [P, FC, FC, P], BF16)

    ident_sb = load_res("ident_sb", t["ident"][:], [P, P])
    identb_sb = load_res("identb_sb", t["identb"][:], [P, P], BF16)
    i8neg_sb = load_res("i8neg_sb", t["i8neg"][:], [H, H], BF16)
    h_sb = load_res("h_sb", t["hmat"].rearrange("(c p) h -> p c h", p=P),
                    [P, FC, H], BF16)
    ht_sb = load_res("ht_sb", t["hmat"].rearrange("(c p) h -> h c p", p=P),
                     [H, FC, P], BF16)
    negh_sb = res.tile([P, FC, H], BF16, name="negh_sb")
    nc.vector.tensor_scalar_mul(negh_sb[:], h_sb[:], -1.0)

    bq_sb = load_res("bq_sb", _vec6(t["bq"]), [P, FC])
    bk_sb = load_res("bk_sb", _vec6(t["bk"]), [P, FC])
    bv_sb = load_res("bv_sb", _vec6(t["bv"]), [P, FC])
    bo_sb = load_res("bo_sb", _vec6(t["bo"]), [P, FC])
    rb1_sb = load_res("rb1_sb", _vec6(t["relik_b1"]), [P, FC])
    ub1_sb = load_res("ub1_sb", _vec6(t["uni_b1"]), [P, FC])
    fb1_sb = load_res("fb1_sb", _vec6(t["ffn_b1"], HFC), [P, HFC])
    fb2_sb = load_res("fb2_sb", _vec6(t["ffn_b2"]), [P, FC])
    l1g_sb = load_res("l1g_sb", _vec6(t["ln1_g"]), [P, FC])
    l1b_sb = load_res("l1b_sb", _vec6(t["ln1_b"]), [P, FC])
    l2g_sb = load_res("l2g_sb", _vec6(t["ln2_g"]), [P, FC])
    l2b_sb = load_res("l2b_sb", _vec6(t["ln2_b"]), [P, FC])
    rw2_sb = load_res("rw2_sb",
                      t["relik_w2"].rearrange("(c p) o -> p c o", p=P),
                      [P, FC, 1], BF16)
    rb2_sb = load_res("rb2_sb", t["relik_b2"][:], [1, 1])

    ones_sb = res.tile([P, 1], BF16, name="ones_sb")
    nc.vector.memset(ones_sb[:], 1.0)
    onesf_sb = res.tile([P, 1], F32, name="onesf_sb")
    nc.vector.memset(onesf_sb[:], 1.0)
    ones_row = res.tile([1, P], BF16, name="ones_row")
    nc.vector.memset(ones_row[:], 1.0)
    eps_col = res.tile([P, 1], F32, name="eps_col")
    nc.vector.memset(eps_col[:], EPS_LN)
    onesf_sq = res.tile([P, P], F32, name="onesf_sq")
    nc.vector.memset(onesf_sq[:], 1.0)

    # stats lhsT [128, 6, 3]: cols = [1, g2^2, g2*b2] per feature chunk
    sl3_sb = res.tile([P, FC, 3], BF16, name="sl3_sb")
    g2sq_sb = res.tile([P, FC], F32, name="g2sq_sb")
    g2b2_sb = res.tile([P, FC], F32, name="g2b2_sb")
    nc.vector.tensor_mul(g2sq_sb[:], l2g_sb[:], l2g_sb[:])
    nc.vector.tensor_mul(g2b2_sb[:], l2g_sb[:], l2b_sb[:])
    for c in range(FC):
        nc.vector.tensor_copy(sl3_sb[:, c, 0:1], ones_sb[:])
        nc.vector.tensor_copy(sl3_sb[:, c, 1:2], g2sq_sb[:, c:c + 1])
        nc.vector.tensor_copy(sl3_sb[:, c, 2:3], g2b2_sb[:, c:c + 1])

    # scalar reductions of bias/gain vectors -> [128,1] columns (value
    # replicated on every partition; [0:1] slice gives the row-space form)
    def vec_sum(name, vecs):
        tmp = res.tile([P, FC], F32, name=name + "_t")
        if len(vecs) == 1:
            nc.vector.tensor_copy(tmp[:], vecs[0][:])
        else:
            nc.vector.tensor_mul(tmp[:], vecs[0][:], vecs[1][:])
            for v in vecs[2:]:
                nc.vector.tensor_mul(tmp[:], tmp[:], v[:])
        red = res.tile([P, 1], F32, name=name + "_r")
        nc.vector.tensor_reduce(red[:], tmp[:], axis=mybir.AxisListType.X,
                                op=ALU.add)
        pR = ps_mm((P, 1))
        mm(pR[:, 0:1], onesf_sq[:], red[:], start=True, stop=True)
        arr = res.tile([P, 1], F32, name=name)
        nc.vector.tensor_copy(arr[:], pR[:, 0:1])
        return arr

    s_bo_c = vec_sum("s_bo", [bo_sb])
    s_fb2_c = vec_sum("s_fb2", [fb2_sb])
    s_g2_c = vec_sum("s_g2", [l2g_sb, l2g_sb])
    s_gb_c = vec_sum("s_gb", [l2g_sb, l2b_sb])
    s_bb_c = vec_sum("s_bb", [l2b_sb, l2b_sb])
    s_g2f_c = vec_sum("s_g2f", [l2g_sb, l2g_sb, fb2_sb])
    s_gbf_c = vec_sum("s_gbf", [l2g_sb, l2b_sb, fb2_sb])
    s_bo = s_bo_c[0:1, 0:1]

    u2rs_sb = res.tile([P, FC], BF16, name="u2rs_sb")
    b2m_sb = res.tile([1, 1], F32, name="b2m_sb")

    # per-mention outputs (feature-major): mcT cols 0:128 mention, 128:256 ctx
    mc_T = res.tile([P, FC, 2 * P], F32, name="mc_T")
    m_T = mc_T[:, :, 0:P]
    m_Tb = res.tile([P, FC, P], BF16, name="m_Tb")
    c_Tb = res.tile([P, FC, P], BF16, name="c_Tb")
    m_q = res.tile([P, FC, P], BF16, name="m_q")
    m_k = res.tile([P, FC, P], BF16, name="m_k")
    m_v = res.tile([P, FC, P], BF16, name="m_v")
    m_relik = res.tile([P, FC, P], BF16, name="m_relik")
    c_uni = res.tile([P, FC, P], BF16, name="c_uni")
    s_aa_sb = res.tile([H, P], BF16, name="s_aa_sb")
    mprod_sb = res.tile([P, FC, P], BF16, name="mprod_sb")

    # ================= phase 0: uni_w2 reduce + selector means ==========
    with tc.tile_pool(name="p0", bufs=1) as p0:
        # uni_w2 row-sums (once)
        u2_sb = p0.tile([P, FC, D], F32, name="u2_sb")
        nc.gpsimd.dma_start(u2_sb[:], t["uni_w2"].rearrange("(i p) o -> p i o", p=P))
        u2r_f = p0.tile([P, FC], F32, name="u2r_f")
        nc.vector.tensor_reduce(u2r_f[:], u2_sb[:],
                                axis=mybir.AxisListType.X, op=ALU.add)
        nc.vector.tensor_copy(u2rs_sb[:], u2r_f[:])
        ub2_sb = p0.tile([1, D], F32, name="ub2_sb")
        nc.gpsimd.dma_start(ub2_sb[:], t["uni_b2"][:])
        b2r = p0.tile([1, 1], F32, name="b2r")
        nc.vector.tensor_reduce(b2r[:], ub2_sb[:], axis=mybir.AxisListType.X,
                                op=ALU.add)
        nc.scalar.activation(b2m_sb[:], b2r[:], AF.Copy, scale=1.0 / D)

        # ---- mention/context means: feature-major selector matmuls ----
        GRP = 4
        for g in range(NCH // GRP):
            txts = []
            sels = []
            for cc in range(GRP):
                c = g * GRP + cc
                txt_c = p0.tile([P, D], F32, tag="txtc", bufs=2 * GRP + 2,
                                name="txt_c")
                nc.sync.dma_start(txt_c[:], t["txt"][c * P:(c + 1) * P, :])
                sel_c = p0.tile([P, 2 * P], F32, tag="selc", bufs=2 * GRP + 2,
                                name="sel_c")
                nc.sync.dma_start(sel_c[:], t["sel"][c])
                txts.append(txt_c)
                sels.append(sel_c)
            for fc in range(FC):
                pA = ps_mm((P, 2 * P))
                for cc in range(GRP):
                    mm(pA[:], txts[cc][:, ts(fc, P)], sels[cc][:],
                       start=(cc == 0), stop=(cc == GRP - 1))
                if g == 0:
                    nc.vector.tensor_copy(mc_T[:, fc, :], pA[:])
                else:
                    nc.vector.tensor_tensor(mc_T[:, fc, :], mc_T[:, fc, :],
                                            pA[:], op=ALU.add)

        nc.vector.tensor_copy(m_Tb[:], mc_T[:, :, 0:P])
        nc.vector.tensor_copy(c_Tb[:], mc_T[:, :, P:2 * P])

    wts = _ctx.enter_context(tc.tile_pool(name="wts", bufs=1))
    act = _ctx.enter_context(tc.tile_pool(name="act", bufs=1))
    lane = _ctx.enter_context(tc.tile_pool(name="lane", bufs=1))

    # ---------- per-mention projections (bf16, N=128) ----------
    def load_strip(bf_dram, oc, tag="wstrip", bufs=6):
        st = wts.tile([P, FC, P], BF16, tag=tag, bufs=bufs, name="w_strip")
        nc.sync.dma_start(st[:],
                          bf_dram[oc].rearrange("p (i q) -> p i q", q=P))
        return st

    for w_r, b_sb, out_t, src in (
        ("wq_r", bq_sb, m_q, m_Tb),
        ("wk_r", bk_sb, m_k, m_Tb),
        ("wv_r", bv_sb, m_v, m_Tb),
        (None, rb1_sb, m_relik, m_Tb),
        (None, ub1_sb, c_uni, c_Tb),
    ):
        for oc in range(FC):
            if w_r is None:
                strip_src = t["w1a_s"] if out_t is m_relik else t["u1a_s"]
                st_ = load_strip(strip_src, oc)
                sl = lambda ic: st_[:, ic, :]
            else:
                sl = lambda ic: w_res[w_r][:, oc, ic, :]
            pA = ps_mm((P, P))
            for ic in range(FC):
                mm(pA[:], sl(ic), src[:, ic, :],
                   start=(ic == 0), stop=(ic == FC - 1))
            nc.scalar.activation(out_t[:, oc, :], pA[:], AF.Identity,
                                 bias=b_sb[:, oc:oc + 1])

    # s_aa [8, 128]
    for c in range(FC):
        nc.vector.tensor_mul(mprod_sb[:, c, :], m_q[:, c, :], m_k[:, c, :])
    pS = ps_score()
    for c in range(FC):
        mm(pS[:, :P], h_sb[:, c, :], mprod_sb[:, c, :],
           start=(c == 0), stop=(c == FC - 1))
    nc.any.tensor_copy(s_aa_sb[:], pS[:, :P])

    def unit(tag, name, bufs=1):
        return act.tile([P, FC, NP], BF16, tag=tag, bufs=bufs, name=name)

    def chunk_t(name):
        return act.tile([P, NP], BF16, tag="tt", bufs=3, name=name)

    # ================= macro-tile loop =================
    for mt in range(NMACRO):
        g0 = mt * G
        gsl = ds(g0, G)

        lane_seq = [0]

        def lane_t(name, parts=1, width=NP):
            lane_seq[0] += 1
            return lane.tile([parts, width], F32, tag="lnrow", bufs=3,
                             name=f"{name}_{lane_seq[0]}")

        def mview(mt_tile, c):
            """mention-side bcast view [128, G, K]"""
            return mt_tile[:, c, gsl, None].to_broadcast([P, G, K])

        # ---- candidate load + PE transpose (bf16) ----
        cand_rm = act.tile([P, 4, D], BF16, tag="cand_rm", bufs=1,
                           name="cand_rm")
        nc.sync.dma_start(
            cand_rm[:],
            t["cand"].rearrange("(q p) d -> p q d", p=P)[:, ds(4 * mt, 4), :])
        candT = unit("candT", "candT")
        for fc in range(FC):
            pT = ps_mm(dtype=BF16)
            for pc in range(4):
                nc.tensor.transpose(pT[:, ts(pc, P)],
                                    cand_rm[:, pc, ts(fc, P)], identb_sb[:])
            nc.vector.tensor_copy(candT[:, fc, :], pT[:])

        # ---- k/v projections ----
        k_b = unit("B", "k_b")
        v_b = unit("C", "v_b")
        for w_r, b_sb, out_t in (("wk_r", bk_sb, k_b), ("wv_r", bv_sb, v_b)):
            for oc in range(FC):
                pA = ps_mm()
                for ic in range(FC):
                    mm(pA[:], w_res[w_r][:, oc, ic, :], candT[:, ic, :],
                       start=(ic == 0), stop=(ic == FC - 1))
                nc.scalar.activation(out_t[:, oc, :], pA[:], AF.Identity,
                                     bias=b_sb[:, oc:oc + 1])

        # ---- attention scores ----
        pAB = ps_score()
        for c in range(FC):
            pr1 = chunk_t("pr1")
            nc.vector.tensor_tensor(_gk(pr1[:]), _gk(k_b[:, c, :]),
                                    mview(m_q, c), op=ALU.mult)
            mm(pAB[:], h_sb[:, c, :], pr1[:], start=(c == 0), stop=False)
        mm(pAB[:], i8neg_sb[:],
           s_aa_sb[:, gsl, None].to_broadcast([H, G, K]),
           start=False, stop=True)
        p_ab = act.tile([H, NP], BF16, tag="p_ab", bufs=2, name="p_ab")
        nc.scalar.activation(p_ab[:], pAB[:], AF.Sigmoid, scale=ISQ)

        pBA = ps_score()
        first = True
        for c in range(FC):
            pQ = ps_mm()
            for ic in range(FC):
                mm(pQ[:], w_res["wq_r"][:, c, ic, :], candT[:, ic, :],
                   start=(ic == 0), stop=(ic == FC - 1))
            q_c = chunk_t("q_c")
            nc.scalar.activation(q_c[:], pQ[:], AF.Identity,
                                 bias=bq_sb[:, c:c + 1])
            pr2 = chunk_t("pr2")
            nc.vector.tensor_tensor(_gk(pr2[:]), _gk(q_c[:]), mview(m_k, c),
                                    op=ALU.mult)
            mm(pBA[:], h_sb[:, c, :], pr2[:], start=first, stop=False)
            first = False
            pr3 = chunk_t("pr3")
            nc.vector.tensor_mul(pr3[:], q_c[:], k_b[:, c, :])
            mm(pBA[:], negh_sb[:, c, :], pr3[:],
               start=False, stop=(c == FC - 1))
        p_ba = act.tile([H, NP], BF16, tag="p_ba", bufs=2, name="p_ba")
        nc.scalar.activation(p_ba[:], pBA[:], AF.Sigmoid, scale=ISQ)

        # ---- attention outputs ----
        o_a = unit("F", "o_a")
        o_b = unit("G", "o_b")
        for c in range(FC):
            dv = chunk_t("dv")
            nc.vector.tensor_tensor(_gk(dv[:]), _gk(v_b[:, c, :]),
                                    mview(m_v, c), op=ALU.subtract)
            pBC = ps_mm()
            mm(pBC[:], ht_sb[:, c, :], p_ab[:], start=True, stop=True)
            nc.vector.tensor_mul(o_a[:, c, :], pBC[:], dv[:])
            nc.vector.tensor_tensor(_gk(o_a[:, c, :]), _gk(o_a[:, c, :]),
                                    mview(m_v, c), op=ALU.add)
            pBC2 = ps_mm()
            mm(pBC2[:], ht_sb[:, c, :], p_ba[:], start=True, stop=True)
            nc.vector.tensor_mul(o_b[:, c, :], pBC2[:], dv[:])
            nc.vector.tensor_tensor(o_b[:, c, :], v_b[:, c, :], o_b[:, c, :],
                                    op=ALU.subtract)

        # ---- wo + residual ----
        r_a = unit("hh", "r_a", bufs=2)
        r_b = unit("hh", "r_b", bufs=2)
        for oc in range(FC):
            pA = ps_mm()
            for ic in range(FC):
                mm(pA[:], w_res["wo_r"][:, oc, ic, :], o_a[:, ic, :],
                   start=(ic == 0), stop=(ic == FC - 1))
            nc.vector.tensor_tensor(_gk(r_a[:, oc, :]), _gk(pA[:]),
                                    mview(m_T, oc), op=ALU.add)
            pB = ps_mm()
            for ic in range(FC):
                mm(pB[:], w_res["wo_r"][:, oc, ic, :], o_b[:, ic, :],
                   start=(ic == 0), stop=(ic == FC - 1))
            nc.vector.tensor_tensor(r_b[:, oc, :], pB[:], candT[:, oc, :],
                                    op=ALU.add)

        # ---- LN1: merged stat bank, rows a:(0,32) b:(64,96) ----
        pL1 = ps_l1()
        for r_t, base in ((r_a, 0), (r_b, 64)):
            for c in range(FC):
                sq = chunk_t("sq")
                nc.scalar.activation(sq[:], r_t[:, c, :], AF.Square,
                                     bias=bo_sb[:, c:c + 1])
                mm(pL1[base:base + 1, :], ones_sb[:], r_t[:, c, :],
                   start=(c == 0), stop=(c == FC - 1),
                   tile_position=(0, base))
                mm(pL1[base + 32:base + 33, :], ones_sb[:], sq[:],
                   start=(c == 0), stop=(c == FC - 1),
                   tile_position=(0, base + 32))

        # ---- relik / unirel heads (PE filler while LN1 lane math runs) ----
        for w_r, madd, htag, wv2, bias_ap, outrow, fn, scale in (
            ("w1b_r", m_relik, "C2", rw2_sb, rb2_sb[:], 0,
             AF.Identity, 1.0),
            (None, c_uni, "D", u2rs_sb, b2m_sb[:], 2,
             AF.Sigmoid, 1.0 / D),
        ):
            h_head = unit(htag, "hh_" + htag)
            for oc in range(FC):
                if w_r is None:
                    st_u = load_strip(t["u1b_s"], oc)
                    wsl = lambda ic: st_u[:, ic, :]
                else:
                    wsl = lambda ic: w_res[w_r][:, oc, ic, :]
                pA = ps_mm()
                for ic in range(FC):
                    mm(pA[:], wsl(ic), candT[:, ic, :],
                       start=(ic == 0), stop=(ic == FC - 1))
                nc.vector.tensor_tensor(_gk(h_head[:, oc, :]), _gk(pA[:]),
                                        mview(madd, oc), op=ALU.add)
                nc.scalar.activation(h_head[:, oc, :], h_head[:, oc, :],
                                     AF.Relu)
            pH = ps_head()
            for c in range(FC):
                if wv2 is rw2_sb:
                    lhsT = wv2[:, c, :]
                else:
                    lhsT = wv2[:, c:c + 1]
                mm(pH[:], lhsT, h_head[:, c, :],
                   start=(c == 0), stop=(c == FC - 1))
            osl = lane_t("osl_" + htag)
            nc.scalar.activation(osl[:], pH[:], fn, bias=bias_ap, scale=scale)
            nc.gpsimd.dma_start(t["out"][outrow:outrow + 1, ts(mt, NP)], osl[:])

        def lnrow(name):
            lane_seq[0] += 1
            return lane.tile([1, NP], F32, tag="lnrow", bufs=3,
                             name=f"{name}_{lane_seq[0]}")

        def layernorm1(r_t, x1_t, base, tok):
            mu = lnrow("mu" + tok)
            nc.vector.tensor_scalar(mu[:], pL1[base:base + 1, :], s_bo,
                                    1.0 / D, op0=ALU.add, op1=ALU.mult)
            var = lnrow("var" + tok)
            nc.vector.tensor_mul(var[:], mu[:], mu[:])
            nc.vector.scalar_tensor_tensor(var[:], pL1[base + 32:base + 33, :],
                                           1.0 / D, var[:], op0=ALU.mult,
                                           op1=ALU.subtract)
            rstd = lnrow("rstd" + tok)
            nc.scalar.activation(rstd[:], var[:], AF.Sqrt,
                                 bias=eps_col[0:1, 0:1])
            nc.vector.reciprocal(rstd[:], rstd[:])
            mubf = act.tile([1, NP], BF16, tag="mubf", bufs=1, name="mubf")
            rstdbf = act.tile([1, NP], BF16, tag="rstdbf", bufs=1,
                              name="rstdbf")
            nc.vector.tensor_copy(mubf[:], mu[:])
            nc.vector.tensor_copy(rstdbf[:], rstd[:])
            mu_bc = ps_mm()
            rstd_bc = ps_mm()
            mm(mu_bc[:], ones_row[:], mubf[:], start=True, stop=True)
            mm(rstd_bc[:], ones_row[:], rstdbf[:], start=True, stop=True)
            for c in range(FC):
                nc.vector.tensor_tensor(x1_t[:, c, :], r_t[:, c, :],
                                        mu_bc[:], op=ALU.subtract)
                nc.vector.scalar_tensor_tensor(
                    x1_t[:, c, :], x1_t[:, c, :], bo_sb[:, c:c + 1],
                    rstd_bc[:], op0=ALU.add, op1=ALU.mult)
                nc.vector.tensor_scalar(
                    x1_t[:, c, :], x1_t[:, c, :], l1g_sb[:, c:c + 1],
                    l1b_sb[:, c:c + 1], op0=ALU.mult, op1=ALU.add)

        x1_a = unit("A", "x1_a")
        x1_b = unit("B", "x1_b")
        layernorm1(r_a, x1_a, 0, "a")
        layernorm1(r_b, x1_b, 64, "b")

        # ---- FFN (both tokens share each weight strip) ----
        if FP8_FFN:
            x1a_8 = act.tile([P, FC, NP], FP8, tag="x1a8", bufs=1, name="x1a8")
            x1b_8 = act.tile([P, FC, NP], FP8, tag="x1b8", bufs=1, name="x1b8")
            for c in range(FC):
                nc.scalar.activation(x1a_8[:, c, :], x1_a[:, c, :], AF.Copy)
                nc.scalar.activation(x1b_8[:, c, :], x1_b[:, c, :], AF.Copy)
            h_a = act.tile([P, HFC, NP], FP8, tag="h8a", bufs=1, name="h_a")
            h_b = act.tile([P, HFC, NP], FP8, tag="h8b", bufs=1, name="h_b")

            def ha_c(hc):
                return h_a[:, hc, :]

            def hb_c(hc):
                return h_b[:, hc, :]

            for hc in range(HFC):
                st = wts.tile([P, FC, P], FP8, tag="w1strip", bufs=4,
                              name="w1_strip")
                nc.sync.dma_start(
                    st[:], t["fw1_s"][hc].rearrange("p (i q) -> p i q", q=P))
                for x8_t, hcs in ((x1a_8, ha_c), (x1b_8, hb_c)):
                    pA = ps_mm()
                    for i in range(FC // 2):
                        mm(pA[:], st[:, 2 * i:2 * i + 2, :],
                           x8_t[:, 2 * i:2 * i + 2, :],
                           start=(i == 0), stop=(i == FC // 2 - 1),
                           perf_mode=DR)
                    nc.scalar.activation(hcs(hc), pA[:], AF.Relu,
                                         bias=fb1_sb[:, hc:hc + 1],
                                         scale=1.0 / W_SCALE)
            r2_a = unit("C2", "r2_a")
            r2_b = unit("D", "r2_b")
            for oc in range(FC):
                stw = wts.tile([P, HFC, P], FP8, tag="w2strip", bufs=2,
                               name="stw")
                nc.sync.dma_start(
                    stw[:],
                    t["fw2_s"][oc].rearrange("p (i q) -> p i q", q=P))
                for x1_t, h_t, r2_t in ((x1_a, h_a, r2_a), (x1_b, h_b, r2_b)):
                    pA = ps_mm()
                    for i in range(HFC // 2):
                        mm(pA[:], stw[:, 2 * i:2 * i + 2, :],
                           h_t[:, 2 * i:2 * i + 2, :],
                           start=(i == 0), stop=(i == HFC // 2 - 1),
                           perf_mode=DR)
                    nc.vector.scalar_tensor_tensor(
                        r2_t[:, oc, :], pA[:], 1.0 / W_SCALE, x1_t[:, oc, :],
                        op0=ALU.mult, op1=ALU.add)
        else:
            h_a = act.tile([P, HFC, NP], BF16, tag="h", bufs=1, name="h_a")
            hb = [unit("candT", "hb0"), unit("G", "hb1"),
                  unit("F", "hb2"), unit("hh", "hb3", bufs=2)]

            def ha_c(hc):
                return h_a[:, hc, :]

            def hb_c(hc):
                return hb[hc // FC][:, hc % FC, :]

            for hc in range(HFC):
                st = wts.tile([P, FC, P], BF16, tag="w1strip", bufs=4,
                              name="w1_strip")
                nc.sync.dma_start(
                    st[:], t["fw1_s"][hc].rearrange("p (i q) -> p i q", q=P))
                for x1_t, hcs in ((x1_a, ha_c), (x1_b, hb_c)):
                    pA = ps_mm()
                    for ic in range(FC):
                        mm(pA[:], st[:, ic, :], x1_t[:, ic, :],
                           start=(ic == 0), stop=(ic == FC - 1))
                    nc.scalar.activation(hcs(hc), pA[:],
                                         AF.Relu, bias=fb1_sb[:, hc:hc + 1])
            r2_a = unit("C2", "r2_a")
            r2_b = unit("D", "r2_b")
            for oc in range(FC):
                stw = wts.tile([P, HFC, P], BF16, tag="w2strip", bufs=2,
                               name="stw")
                nc.sync.dma_start(
                    stw[:],
                    t["fw2_s"][oc].rearrange("p (i q) -> p i q", q=P))
                for x1_t, hcs, r2_t in ((x1_a, ha_c, r2_a), (x1_b, hb_c, r2_b)):
                    pA = ps_mm()
                    for hc in range(HFC):
                        mm(pA[:], stw[:, hc, :], hcs(hc),
                           start=(hc == 0), stop=(hc == HFC - 1))
                    nc.vector.tensor_tensor(r2_t[:, oc, :], pA[:],
                                            x1_t[:, oc, :], op=ALU.add)

        # ---- LN2 + cosine via sufficient statistics ----
        # merged stat bank rows: a:(0..2, 32..33)  b:(64..66, 96..97)
        #   base+0: [sum, g2^2, g2*b2] . y      (y = r2 + fb2, via bias)
        #   base+32: [sum, g2^2] . y^2
        pL2 = ps_l2()
        pX = ps_head()
        for c in range(FC):
            sqa = chunk_t("sq")
            nc.scalar.activation(sqa[:], r2_a[:, c, :], AF.Square,
                                 bias=fb2_sb[:, c:c + 1])
            sqb = chunk_t("sq")
            nc.scalar.activation(sqb[:], r2_b[:, c, :], AF.Square,
                                 bias=fb2_sb[:, c:c + 1])
            rr = chunk_t("rr")
            nc.vector.tensor_scalar_add(rr[:], r2_b[:, c, :],
                                        fb2_sb[:, c:c + 1])
            nc.vector.scalar_tensor_tensor(rr[:], r2_a[:, c, :],
                                           fb2_sb[:, c:c + 1], rr[:],
                                           op0=ALU.add, op1=ALU.mult)
            mm(pL2[0:3, :], sl3_sb[:, c, 0:3], r2_a[:, c, :],
               start=(c == 0), stop=(c == FC - 1), tile_position=(0, 0))
            mm(pL2[32:34, :], sl3_sb[:, c, 0:2], sqa[:],
               start=(c == 0), stop=(c == FC - 1), tile_position=(0, 32))
            mm(pL2[64:67, :], sl3_sb[:, c, 0:3], r2_b[:, c, :],
               start=(c == 0), stop=(c == FC - 1), tile_position=(0, 64))
            mm(pL2[96:98, :], sl3_sb[:, c, 0:2], sqb[:],
               start=(c == 0), stop=(c == FC - 1), tile_position=(0, 96))
            mm(pX[:], sl3_sb[:, c, 1:2], rr[:],
               start=(c == 0), stop=(c == FC - 1))

        # evict stats + pX to SBUF, transpose to pair-major [128, 4, 128]
        # (pX lands in spare transposed column 3 via [1,128]^T matmuls)
        stat_sb = act.tile([P, NP], F32, tag="stat_sb", bufs=1, name="stat_sb")
        nc.vector.tensor_copy(stat_sb[:], pL2[:])
        pX_sb = act.tile([1, NP], F32, tag="pX_sb", bufs=1, name="pX_sb")
        nc.vector.tensor_copy(pX_sb[:], pX[:])
        pT = ps_mm()
        for b in range(4):
            nc.tensor.transpose(pT[:, ts(b, P)], stat_sb[:, ts(b, P)],
                                ident_sb[:])
        for b in range(4):
            mm(pT[:, b * P + 3:b * P + 4], pX_sb[0:1, ts(b, P)],
               onesf_sb[0:1, 0:1], start=True, stop=True)
        sT = act.tile([P, 4, P], F32, tag="sT", bufs=1, name="sT")
        nc.vector.tensor_copy(sT[:], pT[:])

        # pair-major lane algebra on [128, 4] slices
        def col(j):
            return sT[:, :, j]

        def lane4(name):
            lane_seq[0] += 1
            return lane.tile([P, 4], F32, tag=name + "4", bufs=1,
                             name=f"{name}4_{lane_seq[0]}")

        def ln2_lane(base, tok):
            muz = lane4("muz" + tok)
            nc.vector.tensor_scalar(muz[:], col(base + 0), s_fb2_c[:],
                                    1.0 / D, op0=ALU.add, op1=ALU.mult)
            g2z = lane4("g2z" + tok)
            nc.vector.tensor_scalar_add(g2z[:], col(base + 1), s_g2f_c[:])
            gbz = lane4("gbz" + tok)
            nc.vector.tensor_scalar_add(gbz[:], col(base + 2), s_gbf_c[:])
            var = lane4("var2" + tok)
            nc.vector.tensor_mul(var[:], muz[:], muz[:])
            nc.vector.scalar_tensor_tensor(var[:], col(base + 32), 1.0 / D,
                                           var[:], op0=ALU.mult,
                                           op1=ALU.subtract)
            rstd = lane4("rstd2" + tok)
            nc.scalar.activation(rstd[:], var[:], AF.Sqrt, bias=eps_col[:])
            nc.vector.reciprocal(rstd[:], rstd[:])
            g2q = col(base + 33)
            return muz, rstd, g2z, gbz, g2q

        mua, rsta, g2za, gbza, g2qa = ln2_lane(0, "a")
        mub2, rstb, g2zb, gbzb, g2qb = ln2_lane(64, "b")

        def gbt(mu, rstd, gbz, name):
            o_t = lane4(name)
            nc.vector.tensor_scalar_mul(o_t[:], mu[:], s_gb_c[:])
            nc.vector.tensor_tensor(o_t[:], gbz[:], o_t[:], op=ALU.subtract)
            nc.vector.tensor_mul(o_t[:], o_t[:], rstd[:])
            return o_t

        gbta = gbt(mua, rsta, gbza, "gbta")
        gbtb = gbt(mub2, rstb, gbzb, "gbtb")

        def normsq(mu, rstd, g2z, g2q, gbt_t, name):
            o_t = lane4(name)
            nc.vector.tensor_scalar_mul(o_t[:], mu[:], s_g2_c[:])
            nc.vector.scalar_tensor_tensor(o_t[:], g2z[:], -2.0, o_t[:],
                                           op0=ALU.mult, op1=ALU.add)
            nc.vector.tensor_mul(o_t[:], o_t[:], mu[:])
            nc.vector.tensor_tensor(o_t[:], o_t[:], g2q, op=ALU.add)
            nc.vector.tensor_mul(o_t[:], o_t[:], rstd[:])
            nc.vector.tensor_mul(o_t[:], o_t[:], rstd[:])
            nc.vector.scalar_tensor_tensor(o_t[:], gbt_t[:], 2.0, o_t[:],
                                           op0=ALU.mult, op1=ALU.add)
            nc.vector.tensor_scalar_add(o_t[:], o_t[:], s_bb_c[:])
            return o_t

        n2a = normsq(mua, rsta, g2za, g2qa, gbta, "n2a")
        n2b = normsq(mub2, rstb, g2zb, g2qb, gbtb, "n2b")

        d01 = lane4("d01")
        nc.vector.tensor_scalar_mul(d01[:], mub2[:], s_g2_c[:])
        nc.vector.tensor_tensor(d01[:], d01[:], g2zb[:], op=ALU.subtract)
        nc.vector.tensor_mul(d01[:], d01[:], mua[:])
        t2 = lane4("t2")
        nc.vector.tensor_mul(t2[:], mub2[:], g2za[:])
        nc.vector.tensor_tensor(d01[:], d01[:], t2[:], op=ALU.subtract)
        nc.vector.tensor_tensor(d01[:], col(3), d01[:], op=ALU.add)
        nc.vector.tensor_mul(d01[:], d01[:], rsta[:])
        nc.vector.tensor_mul(d01[:], d01[:], rstb[:])
        nc.vector.tensor_add(d01[:], d01[:], gbta[:])
        nc.vector.tensor_add(d01[:], d01[:], gbtb[:])
        nc.vector.tensor_scalar_add(d01[:], d01[:], s_bb_c[:])

        den = lane4("den")
        nc.scalar.activation(n2a[:], n2a[:], AF.Sqrt)
        nc.vector.tensor_scalar_max(n2a[:], n2a[:], EPS_COS)
        nc.scalar.activation(n2b[:], n2b[:], AF.Sqrt)
        nc.vector.tensor_scalar_max(n2b[:], n2b[:], EPS_COS)
        nc.vector.tensor_mul(den[:], n2a[:], n2b[:])
        nc.vector.reciprocal(den[:], den[:])
        atg_T = lane4("atg_T")
        nc.vector.tensor_mul(atg_T[:], d01[:], den[:])

        # transpose back [128,4] -> [4,128] and write out
        pback = ps_mm()
        nc.tensor.transpose(pback[0:4, 0:P], atg_T[:], ident_sb[:])
        atg_row = act.tile([4, P], F32, tag="atg_row", bufs=2, name="atg_row")
        nc.vector.tensor_copy(atg_row[:], pback[0:4, 0:P])
        nc.gpsimd.dma_start(
            t["out"][1:2, ts(mt, NP)].rearrange("o (b q) -> (o b) q", q=P),
            atg_row[:])


# ===================== host side =====================

def kernel(**inputs):
    f32 = np.float32
    bf16 = ml_dtypes.bfloat16
    fp8 = ml_dtypes.float8_e4m3
    txt = np.ascontiguousarray(
        np.asarray(inputs["text_embeddings"], f32).reshape(S, D))
    cand_full = np.asarray(inputs["candidate_embeddings"], f32).reshape(M * K, D)
    cand_bf = np.ascontiguousarray(cand_full.astype(bf16))
    starts = np.asarray(inputs["mention_starts"], np.int64)
    spans = np.asarray(inputs["span_lengths"], np.int64)
    ends = starts + spans
    c_start = np.maximum(0, starts - CTX)
    c_end = np.minimum(S - 1, ends + CTX)

    def w(name):
        return np.asarray(inputs[name], f32)

    def strips_oc(wmat, n_in, n_out):
        # [in, out] -> [n_out, P, n_in*P]  (strip oc: [p, i, q])
        a = wmat.reshape(n_in, P, n_out, P)
        return np.ascontiguousarray(a.transpose(2, 1, 0, 3).reshape(
            n_out, P, n_in * P))

    def resident(wmat):
        # [in, out] -> [P, FC(oc), FC(ic), P]
        a = wmat.reshape(FC, P, FC, P)
        return np.ascontiguousarray(a.transpose(1, 2, 0, 3))

    ffn_dt = fp8 if FP8_FFN else bf16
    fscale = W_SCALE if FP8_FFN else 1.0
    consts = {
        "ident": np.eye(P, dtype=f32),
        "identb": np.eye(P, dtype=f32).astype(bf16),
        "hmat": np.repeat(np.eye(H, dtype=f32), DH, axis=0).astype(bf16),
        "i8neg": (-np.eye(H, dtype=f32)).astype(bf16),
        "wq_r": resident(w("wq")).astype(bf16),
        "wk_r": resident(w("wk")).astype(bf16),
        "wv_r": resident(w("wv")).astype(bf16),
        "wo_r": resident(w("wo")).astype(bf16),
        "w1b_r": resident(w("relik_w1")[D:]).astype(bf16),
        "w1a_s": strips_oc(w("relik_w1")[:D], FC, FC).astype(bf16),
        "u1a_s": strips_oc(w("uni_w1")[:D], FC, FC).astype(bf16),
        "u1b_s": strips_oc(w("uni_w1")[D:], FC, FC).astype(bf16),
        "fw1_s": (strips_oc(w("ffn_w1"), FC, HFC) * fscale).astype(ffn_dt),
        "fw2_s": (strips_oc(w("ffn_w2"), HFC, FC) * fscale).astype(ffn_dt),
    }
    vnames = ["relik_b1", "relik_w2", "bq", "bk", "bv", "bo",
              "ln1_g", "ln1_b", "ffn_b1", "ffn_b2",
              "ln2_g", "ln2_b", "uni_b1", "uni_w2"]
    weights = {n: np.ascontiguousarray(np.asarray(inputs[n], f32))
               for n in vnames}
    weights["relik_b2"] = np.asarray(inputs["relik_b2"], f32).reshape(1, 1)
    weights["uni_b2"] = np.ascontiguousarray(
        np.asarray(inputs["uni_b2"], f32).reshape(1, D))

    in_maps = []
    for core in range(NCORES):
        sl = slice(core * M_LOC, (core + 1) * M_LOC)
        # selector matrices with 1/len folded (pure index metadata)
        mark = np.zeros((S + 1, 2, M_LOC), f32)
        ar = np.arange(M_LOC)
        vm = 1.0 / (spans[sl] + 1).astype(f32)
        np.add.at(mark, (starts[sl], 0, ar), vm)
        np.add.at(mark, (ends[sl] + 1, 0, ar), -vm)
        vc = 1.0 / (c_end[sl] - c_start[sl]).astype(f32)
        np.add.at(mark, (c_start[sl], 1, ar), vc)
        np.add.at(mark, (c_end[sl], 1, ar), -vc)
        selm = np.cumsum(mark[:S], axis=0).reshape(NCH, P, 2 * M_LOC)
        im = {
            "txt": txt,
            "sel": np.ascontiguousarray(selm),
            "cand": cand_bf[core * PAIRS:(core + 1) * PAIRS],
        }
        im.update(consts)
        im.update(weights)
        in_maps.append(im)

    if "nc" not in _NC_CACHE:
        _NC_CACHE["nc"] = _build_nc()
    nc = _NC_CACHE["nc"]

    results = bass_utils.run_bass_kernel_spmd(
        nc, in_maps, core_ids=list(range(NCORES))).results

    out = np.zeros((3, M, K), f32)
    for core in range(NCORES):
        sl = slice(core * M_LOC, (core + 1) * M_LOC)
        out[:, sl, :] = results[core]["out"].reshape(3, M_LOC, K)
    return out


if __name__ == "__main__":
    nc = _build_nc()
    print("built ok")


# revision 41
# speedup vs baseline: 1.8482x; 1.0962x over previous
"""Trainium2 Bass kernel for nn_EntityResolutionProcessor.

Data-parallel over mentions (M=1024 -> 128/core on 8 cores).
Host side: weights pre-converted to bf16/fp8 strip-major layouts,
candidates pre-converted to bf16, mention/context selector matrices
(index metadata with 1/len folded) built in numpy.
Device side per core:
  phase0: stream text chunks; mention/context means as feature-major
          selector matmuls accumulated in SBUF; per-mention projections.
  8 macro-tiles of 512 pairs: candidate DMA + PE transpose, k/v/q
  projections from SBUF-resident weights, relik/unirel heads, 2-token
  attention via sigmoid softmax, wo + LN1, FFN (fp8 DoubleRow), LN2 +
  cosine via sufficient statistics with the per-pair lane algebra
  transposed to pair-major so it runs 128 lanes wide.
"""

from contextlib import ExitStack

import ml_dtypes
import numpy as np

import concourse.bass as bass
import concourse.mybir as mybir
import concourse.tile as tile
from concourse import bacc, bass_isa, bass_utils
from concourse.bass import ds, ts

S, D, M, K, H = 4096, 768, 1024, 32, 8
DH = D // H
CTX = 10
NCORES = 8
P = 128
FC = D // P                     # 6 feature chunks
HFC = 4 * D // P                # 24 ffn hidden chunks
M_LOC = M // NCORES             # 128 mentions per core
PAIRS = M_LOC * K               # 4096 pairs per core
NP = 512                        # pairs per macro tile
G = NP // K                     # 16 mentions per macro tile
NMACRO = PAIRS // NP            # 8
NCH = S // P                    # 32 text chunks
ISQ = 1.0 / float(np.sqrt(np.float32(DH)))
EPS_LN = 1e-5
EPS_COS = 1e-8

FP8_FFN = True                  # fp8 DoubleRow FFN matmuls
W_SCALE = 64.0                  # fp8 weight scale (folded out at eviction)

F32 = mybir.dt.float32
BF16 = mybir.dt.bfloat16
FP16 = mybir.dt.float16
FP8 = mybir.dt.float8e4
I32 = mybir.dt.int32
AF = mybir.ActivationFunctionType
ALU = mybir.AluOpType
DR = mybir.MatmulPerfMode.DoubleRow

_NC_CACHE = {}

FFN_DT = FP8 if FP8_FFN else BF16


def _gk(ap):
    """view a [128, NP] AP as [128, G, K]"""
    return ap.rearrange("p (g k) -> p g k", g=G)


def _build_nc():
    nc = bacc.Bacc(
        "TRN2", target_bir_lowering=False, debug=False, num_devices=NCORES
    )

    def inp(name, shape, dtype=F32):
        return nc.dram_tensor(name, list(shape), dtype, kind="ExternalInput").ap()

    t = {}
    t["txt"] = inp("txt", [S, D], FP16)
    t["sel"] = inp("sel", [NCH, P, 2 * P], FP16)
    t["cand"] = inp("cand", [PAIRS, D], BF16)
    t["ident"] = inp("ident", [P, P])
    t["identb"] = inp("identb", [P, P], BF16)
    t["identb64"] = inp("identb64", [P, P], BF16)
    t["hmat"] = inp("hmat", [D, H], BF16)  # head indicator
    t["i8neg"] = inp("i8neg", [H, H], BF16)

    # resident weights [p, oc, ic, q]: attention fp8 (x W_SCALE), relik bf16
    for n in ["wq_r", "wk_r", "wv_r", "wo_r"]:
        t[n] = inp(n, [P, FC, FC, P], FP8)
    t["w1b_r"] = inp("w1b_r", [P, FC, FC, P], BF16)
    # streamed strips
    t["w1a_s"] = inp("w1a_s", [FC, P, FC * P], BF16)
    t["u1a_s"] = inp("u1a_s", [FC, P, FC * P], BF16)
    t["u1b_s"] = inp("u1b_s", [FC, P, FC * P], BF16)
    t["fw1_s"] = inp("fw1_s", [HFC, P, FC * P], FFN_DT)
    t["fw2_s"] = inp("fw2_s", [FC, P, HFC * P], FFN_DT)

    for n, shp in [("relik_b1", [D]), ("relik_w2", [D, 1]), ("relik_b2", [1, 1]),
                   ("bq", [D]), ("bk", [D]), ("bv", [D]), ("bo", [D]),
                   ("ln1_g", [D]), ("ln1_b", [D]),
                   ("ffn_b1", [4 * D]), ("ffn_b2", [D]),
                   ("ln2_g", [D]), ("ln2_b", [D]),
                   ("uni_b1", [D]), ("uni_w2", [D, D]), ("uni_b2", [1, D])]:
        t[n] = inp(n, shp)

    t["out"] = nc.dram_tensor("out", [3, PAIRS], F32, kind="ExternalOutput").ap()

    with tile.TileContext(nc) as tc:
        _body(nc, tc, t)
    nc.compile()
    return nc


def _vec6(v_ap, n=FC):
    """[D] dram AP -> [128, n] per-feature layout"""
    return v_ap.rearrange("(i p) -> p i", p=P)


def _body(nc, tc, t):
    with ExitStack() as _ctx:
        _body_inner(nc, tc, t, _ctx)


def _body_inner(nc, tc, t, _ctx):
    mm = lambda *a, **k: nc.tensor.matmul(*a, **k)

    # ---------------- pools ----------------
    psum = _ctx.enter_context(tc.tile_pool(name="psum", bufs=1, space="PSUM"))
    res = _ctx.enter_context(tc.tile_pool(name="res", bufs=1))

    def ps_mm(shape=(P, NP), dtype=F32):
        return psum.tile(list(shape), dtype, tag="mm", bufs=3,
                         padded_shape=[P, NP], name="ps_mm")

    def ps_score():
        return psum.tile([8, NP], F32, tag="score", bufs=1, name="ps_score")

    def ps_l1():
        return psum.tile([P, NP], F32, tag="l1", bufs=1, name="ps_l1")

    def ps_l2():
        return psum.tile([P, NP], F32, tag="l2", bufs=1, name="ps_l2")

    def ps_head():
        return psum.tile([1, NP], F32, tag="head", bufs=2, name="ps_head")

    # ---------------- resident constants ----------------
    def load_res(name, ap_src, shape, dtype=F32):
        tl = res.tile(list(shape), dtype, name=name)
        nc.gpsimd.dma_start(tl[:], ap_src)
        return tl

    # resident weights (issued first; load during phase0 on Pool queue)
    w_res = {}
    for n in ["wq_r", "wk_r", "wv_r", "wo_r"]:
        w_res[n] = load_res(n, t[n][:], [P, FC, FC, P], FP8)
    w_res["w1b_r"] = load_res("w1b_r", t["w1b_r"][:], [P, FC, FC, P], BF16)

    ident_sb = load_res("ident_sb", t["ident"][:], [P, P])
    identb_sb = load_res("identb_sb", t["identb"][:], [P, P], BF16)
    identb64_sb = load_res("identb64_sb", t["identb64"][:], [P, P], BF16)
    i8neg_sb = load_res("i8neg_sb", t["i8neg"][:], [H, H], BF16)
    h_sb = load_res("h_sb", t["hmat"].rearrange("(c p) h -> p c h", p=P),
                    [P, FC, H], BF16)
    ht_sb = load_res("ht_sb", t["hmat"].rearrange("(c p) h -> h c p", p=P),
                     [H, FC, P], BF16)
    negh_sb = res.tile([P, FC, H], BF16, name="negh_sb")
    nc.vector.tensor_scalar_mul(negh_sb[:], h_sb[:], -1.0)

    bq_sb = load_res("bq_sb", _vec6(t["bq"]), [P, FC])
    bk_sb = load_res("bk_sb", _vec6(t["bk"]), [P, FC])
    bv_sb = load_res("bv_sb", _vec6(t["bv"]), [P, FC])
    bo_sb = load_res("bo_sb", _vec6(t["bo"]), [P, FC])
    rb1_sb = load_res("rb1_sb", _vec6(t["relik_b1"]), [P, FC])
    ub1_sb = load_res("ub1_sb", _vec6(t["uni_b1"]), [P, FC])
    fb1_sb = load_res("fb1_sb", _vec6(t["ffn_b1"], HFC), [P, HFC])
    fb2_sb = load_res("fb2_sb", _vec6(t["ffn_b2"]), [P, FC])
    l1g_sb = load_res("l1g_sb", _vec6(t["ln1_g"]), [P, FC])
    l1b_sb = load_res("l1b_sb", _vec6(t["ln1_b"]), [P, FC])
    l2g_sb = load_res("l2g_sb", _vec6(t["ln2_g"]), [P, FC])
    l2b_sb = load_res("l2b_sb", _vec6(t["ln2_b"]), [P, FC])
    rw2_sb = load_res("rw2_sb",
                      t["relik_w2"].rearrange("(c p) o -> p c o", p=P),
                      [P, FC, 1], BF16)
    rb2_sb = load_res("rb2_sb", t["relik_b2"][:], [1, 1])

    ones_sb = res.tile([P, 1], BF16, name="ones_sb")
    nc.vector.memset(ones_sb[:], 1.0)
    onesf_sb = res.tile([P, 1], F32, name="onesf_sb")
    nc.vector.memset(onesf_sb[:], 1.0)
    ones_row = res.tile([1, P], BF16, name="ones_row")
    nc.vector.memset(ones_row[:], 1.0)
    eps_col = res.tile([P, 1], F32, name="eps_col")
    nc.vector.memset(eps_col[:], EPS_LN)
    onesf_sq = res.tile([P, P], F32, name="onesf_sq")
    nc.vector.memset(onesf_sq[:], 1.0)

    # stats lhsT [128, 6, 3]: cols = [1, g2^2, g2*b2] per feature chunk
    sl3_sb = res.tile([P, FC, 3], BF16, name="sl3_sb")
    g2sq_sb = res.tile([P, FC], F32, name="g2sq_sb")
    g2b2_sb = res.tile([P, FC], F32, name="g2b2_sb")
    nc.vector.tensor_mul(g2sq_sb[:], l2g_sb[:], l2g_sb[:])
    nc.vector.tensor_mul(g2b2_sb[:], l2g_sb[:], l2b_sb[:])
    for c in range(FC):
        nc.vector.tensor_copy(sl3_sb[:, c, 0:1], ones_sb[:])
        nc.vector.tensor_copy(sl3_sb[:, c, 1:2], g2sq_sb[:, c:c + 1])
        nc.vector.tensor_copy(sl3_sb[:, c, 2:3], g2b2_sb[:, c:c + 1])

    # scalar reductions of bias/gain vectors -> [128,1] columns (value
    # replicated on every partition; [0:1] slice gives the row-space form)
    def vec_sum(name, vecs):
        tmp = res.tile([P, FC], F32, name=name + "_t")
        if len(vecs) == 1:
            nc.vector.tensor_copy(tmp[:], vecs[0][:])
        else:
            nc.vector.tensor_mul(tmp[:], vecs[0][:], vecs[1][:])
            for v in vecs[2:]:
                nc.vector.tensor_mul(tmp[:], tmp[:], v[:])
        red = res.tile([P, 1], F32, name=name + "_r")
        nc.vector.tensor_reduce(red[:], tmp[:], axis=mybir.AxisListType.X,
                                op=ALU.add)
        pR = ps_mm((P, 1))
        mm(pR[:, 0:1], onesf_sq[:], red[:], start=True, stop=True)
        arr = res.tile([P, 1], F32, name=name)
        nc.vector.tensor_copy(arr[:], pR[:, 0:1])
        return arr

    s_bo_c = vec_sum("s_bo", [bo_sb])
    s_fb2_c = vec_sum("s_fb2", [fb2_sb])
    s_g2_c = vec_sum("s_g2", [l2g_sb, l2g_sb])
    s_gb_c = vec_sum("s_gb", [l2g_sb, l2b_sb])
    s_bb_c = vec_sum("s_bb", [l2b_sb, l2b_sb])
    s_g2f_c = vec_sum("s_g2f", [l2g_sb, l2g_sb, fb2_sb])
    s_gbf_c = vec_sum("s_gbf", [l2g_sb, l2b_sb, fb2_sb])
    s_bo = s_bo_c[0:1, 0:1]

    u2rs_sb = res.tile([P, FC], BF16, name="u2rs_sb")
    b2m_sb = res.tile([1, 1], F32, name="b2m_sb")

    # per-mention outputs (feature-major): mcT cols 0:128 mention, 128:256 ctx
    mc_T = res.tile([P, FC, 2 * P], F32, name="mc_T")
    m_T = mc_T[:, :, 0:P]
    m_Tb = res.tile([P, FC, P], BF16, name="m_Tb")
    c_Tb = res.tile([P, FC, P], BF16, name="c_Tb")
    m_q = res.tile([P, FC, P], BF16, name="m_q")
    m_k = res.tile([P, FC, P], BF16, name="m_k")
    m_v = res.tile([P, FC, P], BF16, name="m_v")
    m_relik = res.tile([P, FC, P], BF16, name="m_relik")
    c_uni = res.tile([P, FC, P], BF16, name="c_uni")
    s_aa_sb = res.tile([H, P], BF16, name="s_aa_sb")
    mprod_sb = res.tile([P, FC, P], BF16, name="mprod_sb")

    # ================= phase 0: uni_w2 reduce + selector means ==========
    with tc.tile_pool(name="p0", bufs=1) as p0:
        # uni_w2 row-sums (once)
        u2_sb = p0.tile([P, FC, D], F32, name="u2_sb")
        nc.gpsimd.dma_start(u2_sb[:], t["uni_w2"].rearrange("(i p) o -> p i o", p=P))
        u2r_f = p0.tile([P, FC], F32, name="u2r_f")
        nc.vector.tensor_reduce(u2r_f[:], u2_sb[:],
                                axis=mybir.AxisListType.X, op=ALU.add)
        nc.vector.tensor_copy(u2rs_sb[:], u2r_f[:])
        ub2_sb = p0.tile([1, D], F32, name="ub2_sb")
        nc.gpsimd.dma_start(ub2_sb[:], t["uni_b2"][:])
        b2r = p0.tile([1, 1], F32, name="b2r")
        nc.vector.tensor_reduce(b2r[:], ub2_sb[:], axis=mybir.AxisListType.X,
                                op=ALU.add)
        nc.scalar.activation(b2m_sb[:], b2r[:], AF.Copy, scale=1.0 / D)

        # ---- mention/context means: feature-major selector matmuls ----
        GRP = 4
        for g in range(NCH // GRP):
            txts = []
            sels = []
            for cc in range(GRP):
                c = g * GRP + cc
                txt_c = p0.tile([P, D], FP16, tag="txtc", bufs=2 * GRP + 2,
                                name="txt_c")
                nc.sync.dma_start(txt_c[:], t["txt"][c * P:(c + 1) * P, :])
                sel_c = p0.tile([P, 2 * P], FP16, tag="selc", bufs=2 * GRP + 2,
                                name="sel_c")
                nc.sync.dma_start(sel_c[:], t["sel"][c])
                txts.append(txt_c)
                sels.append(sel_c)
            for fc in range(FC):
                pA = ps_mm((P, 2 * P))
                for cc in range(GRP):
                    mm(pA[:], txts[cc][:, ts(fc, P)], sels[cc][:],
                       start=(cc == 0), stop=(cc == GRP - 1))
                if g == 0:
                    nc.vector.tensor_copy(mc_T[:, fc, :], pA[:])
                else:
                    nc.vector.tensor_tensor(mc_T[:, fc, :], mc_T[:, fc, :],
                                            pA[:], op=ALU.add)

        nc.vector.tensor_copy(m_Tb[:], mc_T[:, :, 0:P])
        nc.vector.tensor_copy(c_Tb[:], mc_T[:, :, P:2 * P])

    wts = _ctx.enter_context(tc.tile_pool(name="wts", bufs=1))
    act = _ctx.enter_context(tc.tile_pool(name="act", bufs=1))
    lane = _ctx.enter_context(tc.tile_pool(name="lane", bufs=1))

    # ---------- per-mention projections (bf16, N=128) ----------
    def load_strip(bf_dram, oc, tag="wstrip", bufs=6):
        st = wts.tile([P, FC, P], BF16, tag=tag, bufs=bufs, name="w_strip")
        nc.sync.dma_start(st[:],
                          bf_dram[oc].rearrange("p (i q) -> p i q", q=P))
        return st

    m_T8 = res.tile([P, FC, P], FP8, name="m_T8")
    nc.scalar.activation(m_T8[:], mc_T[:, :, 0:P], AF.Copy)
    for w_r, b_sb, out_t, src in (
        ("wq_r", bq_sb, m_q, m_T8),
        ("wk_r", bk_sb, m_k, m_T8),
        ("wv_r", bv_sb, m_v, m_T8),
        (None, rb1_sb, m_relik, m_Tb),
        (None, ub1_sb, c_uni, c_Tb),
    ):
        for oc in range(FC):
            pA = ps_mm((P, P))
            if w_r is None:
                strip_src = t["w1a_s"] if out_t is m_relik else t["u1a_s"]
                st_ = load_strip(strip_src, oc)
                for ic in range(FC):
                    mm(pA[:], st_[:, ic, :], src[:, ic, :],
                       start=(ic == 0), stop=(ic == FC - 1))
                sc = 1.0
            else:
                for i in range(FC // 2):
                    mm(pA[:], w_res[w_r][:, oc, 2 * i:2 * i + 2, :],
                       src[:, 2 * i:2 * i + 2, :],
                       start=(i == 0), stop=(i == FC // 2 - 1), perf_mode=DR)
                sc = 1.0 / W_SCALE
            nc.scalar.activation(out_t[:, oc, :], pA[:], AF.Identity,
                                 bias=b_sb[:, oc:oc + 1], scale=sc)

    # s_aa [8, 128]
    for c in range(FC):
        nc.vector.tensor_mul(mprod_sb[:, c, :], m_q[:, c, :], m_k[:, c, :])
    pS = ps_score()
    for c in range(FC):
        mm(pS[:, :P], h_sb[:, c, :], mprod_sb[:, c, :],
           start=(c == 0), stop=(c == FC - 1))
    nc.any.tensor_copy(s_aa_sb[:], pS[:, :P])

    def unit(tag, name, bufs=1):
        return act.tile([P, FC, NP], BF16, tag=tag, bufs=bufs, name=name)

    def chunk_t(name):
        return act.tile([P, NP], BF16, tag="tt", bufs=3, name=name)

    # ================= macro-tile loop =================
    for mt in range(NMACRO):
        g0 = mt * G
        gsl = ds(g0, G)

        lane_seq = [0]

        def lane_t(name, parts=1, width=NP):
            lane_seq[0] += 1
            return lane.tile([parts, width], F32, tag="lnrow", bufs=3,
                             name=f"{name}_{lane_seq[0]}")

        def mview(mt_tile, c):
            """mention-side bcast view [128, G, K]"""
            return mt_tile[:, c, gsl, None].to_broadcast([P, G, K])

        # ---- candidate load + PE transpose (bf16) ----
        cand_rm = act.tile([P, 4, D], BF16, tag="cand_rm", bufs=1,
                           name="cand_rm")
        nc.sync.dma_start(
            cand_rm[:],
            t["cand"].rearrange("(q p) d -> p q d", p=P)[:, ds(4 * mt, 4), :])
        candT = unit("candT", "candT")
        candT8 = act.tile([P, FC, NP], FP8, tag="candT8", bufs=1,
                          name="candT8")
        for fc in range(FC):
            pT = ps_mm(dtype=BF16)
            for pc in range(4):
                nc.tensor.transpose(pT[:, ts(pc, P)],
                                    cand_rm[:, pc, ts(fc, P)], identb_sb[:])
            nc.vector.tensor_copy(candT[:, fc, :], pT[:])
            nc.scalar.activation(candT8[:, fc, :], pT[:], AF.Copy)

        # ---- k/v projections (fp8 DoubleRow) ----
        k_b = unit("B", "k_b")
        v_b = unit("C", "v_b")
        for w_r, b_sb, out_t in (("wk_r", bk_sb, k_b), ("wv_r", bv_sb, v_b)):
            for oc in range(FC):
                pA = ps_mm()
                for i in range(FC // 2):
                    mm(pA[:], w_res[w_r][:, oc, 2 * i:2 * i + 2, :],
                       candT8[:, 2 * i:2 * i + 2, :],
                       start=(i == 0), stop=(i == FC // 2 - 1), perf_mode=DR)
                nc.scalar.activation(out_t[:, oc, :], pA[:], AF.Identity,
                                     bias=b_sb[:, oc:oc + 1],
                                     scale=1.0 / W_SCALE)

        # ---- attention scores ----
        pAB = ps_score()
        for c in range(FC):
            pr1 = chunk_t("pr1")
            nc.vector.tensor_tensor(_gk(pr1[:]), _gk(k_b[:, c, :]),
                                    mview(m_q, c), op=ALU.mult)
            mm(pAB[:], h_sb[:, c, :], pr1[:], start=(c == 0), stop=False)
        mm(pAB[:], i8neg_sb[:],
           s_aa_sb[:, gsl, None].to_broadcast([H, G, K]),
           start=False, stop=True)
        p_ab = act.tile([H, NP], BF16, tag="p_ab", bufs=2, name="p_ab")
        nc.scalar.activation(p_ab[:], pAB[:], AF.Sigmoid, scale=ISQ)

        pBA = ps_score()
        first = True
        for c in range(FC):
            pQ = ps_mm()
            for i in range(FC // 2):
                mm(pQ[:], w_res["wq_r"][:, c, 2 * i:2 * i + 2, :],
                   candT8[:, 2 * i:2 * i + 2, :],
                   start=(i == 0), stop=(i == FC // 2 - 1), perf_mode=DR)
            q_c = chunk_t("q_c")
            nc.scalar.activation(q_c[:], pQ[:], AF.Identity,
                                 bias=bq_sb[:, c:c + 1], scale=1.0 / W_SCALE)
            pr2 = chunk_t("pr2")
            nc.vector.tensor_tensor(_gk(pr2[:]), _gk(q_c[:]), mview(m_k, c),
                                    op=ALU.mult)
            mm(pBA[:], h_sb[:, c, :], pr2[:], start=first, stop=False)
            first = False
            pr3 = chunk_t("pr3")
            nc.vector.tensor_mul(pr3[:], q_c[:], k_b[:, c, :])
            mm(pBA[:], negh_sb[:, c, :], pr3[:],
               start=False, stop=(c == FC - 1))
        p_ba = act.tile([H, NP], BF16, tag="p_ba", bufs=2, name="p_ba")
        nc.scalar.activation(p_ba[:], pBA[:], AF.Sigmoid, scale=ISQ)

        # ---- attention outputs (fp8 for the wo matmul) ----
        o_a = act.tile([P, FC, NP], FP8, tag="o8a", bufs=1, name="o_a")
        o_b = act.tile([P, FC, NP], FP8, tag="o8b", bufs=1, name="o_b")
        for c in range(FC):
            dv = chunk_t("dv")
            nc.vector.tensor_tensor(_gk(dv[:]), _gk(v_b[:, c, :]),
                                    mview(m_v, c), op=ALU.subtract)
            pBC = ps_mm()
            mm(pBC[:], ht_sb[:, c, :], p_ab[:], start=True, stop=True)
            nc.vector.tensor_mul(o_a[:, c, :], pBC[:], dv[:])
            nc.vector.tensor_tensor(_gk(o_a[:, c, :]), _gk(o_a[:, c, :]),
                                    mview(m_v, c), op=ALU.add)
            pBC2 = ps_mm()
            mm(pBC2[:], ht_sb[:, c, :], p_ba[:], start=True, stop=True)
            nc.vector.tensor_mul(o_b[:, c, :], pBC2[:], dv[:])
            nc.vector.tensor_tensor(o_b[:, c, :], v_b[:, c, :], o_b[:, c, :],
                                    op=ALU.subtract)

        # ---- wo + residual (residual folded into psum via identity mm) ----
        r_a = unit("hh", "r_a", bufs=2)
        r_b = unit("hh", "r_b", bufs=2)
        for oc in range(FC):
            pA = ps_mm()
            for i in range(FC // 2):
                mm(pA[:], w_res["wo_r"][:, oc, 2 * i:2 * i + 2, :],
                   o_a[:, 2 * i:2 * i + 2, :],
                   start=(i == 0), stop=False, perf_mode=DR)
            mm(_gk(pA[:]), identb64_sb[:],
               m_Tb[:, oc, gsl, None].to_broadcast([P, G, K]),
               start=False, stop=True)
            nc.vector.tensor_scalar_mul(r_a[:, oc, :], pA[:], 1.0 / W_SCALE)
            pB = ps_mm()
            for i in range(FC // 2):
                mm(pB[:], w_res["wo_r"][:, oc, 2 * i:2 * i + 2, :],
                   o_b[:, 2 * i:2 * i + 2, :],
                   start=(i == 0), stop=False, perf_mode=DR)
            mm(pB[:], identb64_sb[:], candT[:, oc, :],
               start=False, stop=True)
            nc.vector.tensor_scalar_mul(r_b[:, oc, :], pB[:], 1.0 / W_SCALE)

        # ---- LN1: merged stat bank, rows a:(0,32) b:(64,96) ----
        pL1 = ps_l1()
        for r_t, base in ((r_a, 0), (r_b, 64)):
            for c in range(FC):
                sq = chunk_t("sq")
                nc.scalar.activation(sq[:], r_t[:, c, :], AF.Square,
                                     bias=bo_sb[:, c:c + 1])
                mm(pL1[base:base + 1, :], ones_sb[:], r_t[:, c, :],
                   start=(c == 0), stop=(c == FC - 1),
                   tile_position=(0, base))
                mm(pL1[base + 32:base + 33, :], ones_sb[:], sq[:],
                   start=(c == 0), stop=(c == FC - 1),
                   tile_position=(0, base + 32))

        # ---- relik / unirel heads (PE filler while LN1 lane math runs) ----
        for w_r, madd, htag, wv2, bias_ap, outrow, fn, scale in (
            ("w1b_r", m_relik, "C2", rw2_sb, rb2_sb[:], 0,
             AF.Identity, 1.0),
            (None, c_uni, "D", u2rs_sb, b2m_sb[:], 2,
             AF.Sigmoid, 1.0 / D),
        ):
            h_head = unit(htag, "hh_" + htag)
            for oc in range(FC):
                if w_r is None:
                    st_u = load_strip(t["u1b_s"], oc)
                    wsl = lambda ic: st_u[:, ic, :]
                else:
                    wsl = lambda ic: w_res[w_r][:, oc, ic, :]
                pA = ps_mm()
                for ic in range(FC):
                    mm(pA[:], wsl(ic), candT[:, ic, :],
                       start=(ic == 0), stop=False)
                mm(_gk(pA[:]), identb_sb[:], mview(madd, oc),
                   start=False, stop=True)
                nc.scalar.activation(h_head[:, oc, :], pA[:], AF.Relu)
            pH = ps_head()
            for c in range(FC):
                if wv2 is rw2_sb:
                    lhsT = wv2[:, c, :]
                else:
                    lhsT = wv2[:, c:c + 1]
                mm(pH[:], lhsT, h_head[:, c, :],
                   start=(c == 0), stop=(c == FC - 1))
            osl = lane_t("osl_" + htag)
            nc.scalar.activation(osl[:], pH[:], fn, bias=bias_ap, scale=scale)
            nc.gpsimd.dma_start(t["out"][outrow:outrow + 1, ts(mt, NP)], osl[:])

        def lnrow(name):
            lane_seq[0] += 1
            return lane.tile([1, NP], F32, tag="lnrow", bufs=3,
                             name=f"{name}_{lane_seq[0]}")

        def layernorm1(r_t, x1_t, base, tok):
            mu = lnrow("mu" + tok)
            nc.vector.tensor_scalar(mu[:], pL1[base:base + 1, :], s_bo,
                                    1.0 / D, op0=ALU.add, op1=ALU.mult)
            var = lnrow("var" + tok)
            nc.vector.tensor_mul(var[:], mu[:], mu[:])
            nc.vector.scalar_tensor_tensor(var[:], pL1[base + 32:base + 33, :],
                                           1.0 / D, var[:], op0=ALU.mult,
                                           op1=ALU.subtract)
            rstd = lnrow("rstd" + tok)
            nc.scalar.activation(rstd[:], var[:], AF.Sqrt,
                                 bias=eps_col[0:1, 0:1])
            nc.vector.reciprocal(rstd[:], rstd[:])
            mubf = act.tile([1, NP], BF16, tag="mubf", bufs=1, name="mubf")
            rstdbf = act.tile([1, NP], BF16, tag="rstdbf", bufs=1,
                              name="rstdbf")
            nc.vector.tensor_copy(mubf[:], mu[:])
            nc.vector.tensor_copy(rstdbf[:], rstd[:])
            mu_bc = ps_mm()
            rstd_bc = ps_mm()
            mm(mu_bc[:], ones_row[:], mubf[:], start=True, stop=True)
            mm(rstd_bc[:], ones_row[:], rstdbf[:], start=True, stop=True)
            for c in range(FC):
                nc.vector.tensor_tensor(x1_t[:, c, :], r_t[:, c, :],
                                        mu_bc[:], op=ALU.subtract)
            for c in range(FC):
                nc.vector.scalar_tensor_tensor(
                    x1_t[:, c, :], x1_t[:, c, :], bo_sb[:, c:c + 1],
                    rstd_bc[:], op0=ALU.add, op1=ALU.mult)
            for c in range(FC):
                nc.vector.tensor_scalar(
                    x1_t[:, c, :], x1_t[:, c, :], l1g_sb[:, c:c + 1],
                    l1b_sb[:, c:c + 1], op0=ALU.mult, op1=ALU.add)

        x1_a = unit("A", "x1_a")
        x1_b = unit("B", "x1_b")
        layernorm1(r_a, x1_a, 0, "a")
        layernorm1(r_b, x1_b, 64, "b")

        # ---- FFN (both tokens share each weight strip) ----
        if FP8_FFN:
            x1a_8 = act.tile([P, FC, NP], FP8, tag="x1a8", bufs=1, name="x1a8")
            x1b_8 = act.tile([P, FC, NP], FP8, tag="x1b8", bufs=1, name="x1b8")
            for c in range(FC):
                nc.scalar.activation(x1a_8[:, c, :], x1_a[:, c, :], AF.Copy)
                nc.scalar.activation(x1b_8[:, c, :], x1_b[:, c, :], AF.Copy)
            h_a = act.tile([P, HFC, NP], FP8, tag="h8a", bufs=1, name="h_a")
            h_b = act.tile([P, HFC, NP], FP8, tag="h8b", bufs=1, name="h_b")

            def ha_c(hc):
                return h_a[:, hc, :]

            def hb_c(hc):
                return h_b[:, hc, :]

            for hc in range(HFC):
                st = wts.tile([P, FC, P], FP8, tag="w1strip", bufs=4,
                              name="w1_strip")
                nc.sync.dma_start(
                    st[:], t["fw1_s"][hc].rearrange("p (i q) -> p i q", q=P))
                for x8_t, hcs in ((x1a_8, ha_c), (x1b_8, hb_c)):
                    pA = ps_mm()
                    for i in range(FC // 2):
                        mm(pA[:], st[:, 2 * i:2 * i + 2, :],
                           x8_t[:, 2 * i:2 * i + 2, :],
                           start=(i == 0), stop=(i == FC // 2 - 1),
                           perf_mode=DR)
                    nc.scalar.activation(hcs(hc), pA[:], AF.Relu,
                                         bias=fb1_sb[:, hc:hc + 1],
                                         scale=1.0 / W_SCALE)
            r2_a = unit("C2", "r2_a")
            r2_b = unit("D", "r2_b")
            for oc in range(FC):
                stw = wts.tile([P, HFC, P], FP8, tag="w2strip", bufs=2,
                               name="stw")
                nc.sync.dma_start(
                    stw[:],
                    t["fw2_s"][oc].rearrange("p (i q) -> p i q", q=P))
                for x1_t, h_t, r2_t in ((x1_a, h_a, r2_a), (x1_b, h_b, r2_b)):
                    pA = ps_mm()
                    for i in range(HFC // 2):
                        mm(pA[:], stw[:, 2 * i:2 * i + 2, :],
                           h_t[:, 2 * i:2 * i + 2, :],
                           start=(i == 0), stop=(i == HFC // 2 - 1),
                           perf_mode=DR)
                    nc.vector.scalar_tensor_tensor(
                        r2_t[:, oc, :], pA[:], 1.0 / W_SCALE, x1_t[:, oc, :],
                        op0=ALU.mult, op1=ALU.add)
        else:
            h_a = act.tile([P, HFC, NP], BF16, tag="h", bufs=1, name="h_a")
            hb = [unit("candT", "hb0"), unit("G", "hb1"),
                  unit("F", "hb2"), unit("hh", "hb3", bufs=2)]

            def ha_c(hc):
                return h_a[:, hc, :]

            def hb_c(hc):
                return hb[hc // FC][:, hc % FC, :]

            for hc in range(HFC):
                st = wts.tile([P, FC, P], BF16, tag="w1strip", bufs=4,
                              name="w1_strip")
                nc.sync.dma_start(
                    st[:], t["fw1_s"][hc].rearrange("p (i q) -> p i q", q=P))
                for x1_t, hcs in ((x1_a, ha_c), (x1_b, hb_c)):
                    pA = ps_mm()
                    for ic in range(FC):
                        mm(pA[:], st[:, ic, :], x1_t[:, ic, :],
                           start=(ic == 0), stop=(ic == FC - 1))
                    nc.scalar.activation(hcs(hc), pA[:],
                                         AF.Relu, bias=fb1_sb[:, hc:hc + 1])
            r2_a = unit("C2", "r2_a")
            r2_b = unit("D", "r2_b")
            for oc in range(FC):
                stw = wts.tile([P, HFC, P], BF16, tag="w2strip", bufs=2,
                               name="stw")
                nc.sync.dma_start(
                    stw[:],
                    t["fw2_s"][oc].rearrange("p (i q) -> p i q", q=P))
                for x1_t, hcs, r2_t in ((x1_a, ha_c, r2_a), (x1_b, hb_c, r2_b)):
                    pA = ps_mm()
                    for hc in range(HFC):
                        mm(pA[:], stw[:, hc, :], hcs(hc),
                           start=(hc == 0), stop=(hc == HFC - 1))
                    nc.vector.tensor_tensor(r2_t[:, oc, :], pA[:],
                                            x1_t[:, oc, :], op=ALU.add)

        # ---- LN2 + cosine via sufficient statistics ----
        # merged stat bank rows: a:(0..2, 32..33)  b:(64..66, 96..97)
        #   base+0: [sum, g2^2, g2*b2] . y      (y = r2 + fb2, via bias)
        #   base+32: [sum, g2^2] . y^2
        pL2 = ps_l2()
        pX = ps_head()
        for c in range(FC):
            sqa = chunk_t("sq")
            nc.scalar.activation(sqa[:], r2_a[:, c, :], AF.Square,
                                 bias=fb2_sb[:, c:c + 1])
            sqb = chunk_t("sq")
            nc.scalar.activation(sqb[:], r2_b[:, c, :], AF.Square,
                                 bias=fb2_sb[:, c:c + 1])
            rr = chunk_t("rr")
            nc.vector.tensor_scalar_add(rr[:], r2_b[:, c, :],
                                        fb2_sb[:, c:c + 1])
            nc.vector.scalar_tensor_tensor(rr[:], r2_a[:, c, :],
                                           fb2_sb[:, c:c + 1], rr[:],
                                           op0=ALU.add, op1=ALU.mult)
            mm(pL2[0:3, :], sl3_sb[:, c, 0:3], r2_a[:, c, :],
               start=(c == 0), stop=(c == FC - 1), tile_position=(0, 0))
            mm(pL2[32:34, :], sl3_sb[:, c, 0:2], sqa[:],
               start=(c == 0), stop=(c == FC - 1), tile_position=(0, 32))
            mm(pL2[64:67, :], sl3_sb[:, c, 0:3], r2_b[:, c, :],
               start=(c == 0), stop=(c == FC - 1), tile_position=(0, 64))
            mm(pL2[96:98, :], sl3_sb[:, c, 0:2], sqb[:],
               start=(c == 0), stop=(c == FC - 1), tile_position=(0, 96))
            mm(pX[:], sl3_sb[:, c, 1:2], rr[:],
               start=(c == 0), stop=(c == FC - 1))

        # evict stats + pX to SBUF, transpose to pair-major [128, 4, 128]
        # (pX lands in spare transposed column 3 via [1,128]^T matmuls)
        stat_sb = act.tile([P, NP], F32, tag="stat_sb", bufs=1, name="stat_sb")
        nc.vector.tensor_copy(stat_sb[:], pL2[:])
        pX_sb = act.tile([1, NP], F32, tag="pX_sb", bufs=1, name="pX_sb")
        nc.vector.tensor_copy(pX_sb[:], pX[:])
        pT = ps_mm()
        for b in range(4):
            nc.tensor.transpose(pT[:, ts(b, P)], stat_sb[:, ts(b, P)],
                                ident_sb[:])
        for b in range(4):
            mm(pT[:, b * P + 3:b * P + 4], pX_sb[0:1, ts(b, P)],
               onesf_sb[0:1, 0:1], start=True, stop=True)
        sT = act.tile([P, 4, P], F32, tag="sT", bufs=1, name="sT")
        nc.vector.tensor_copy(sT[:], pT[:])

        # pair-major lane algebra on [128, 4] slices
        def col(j):
            return sT[:, :, j]

        def lane4(name):
            lane_seq[0] += 1
            return lane.tile([P, 4], F32, tag=name + "4", bufs=1,
                             name=f"{name}4_{lane_seq[0]}")

        def ln2_lane(base, tok):
            muz = lane4("muz" + tok)
            nc.vector.tensor_scalar(muz[:], col(base + 0), s_fb2_c[:],
                                    1.0 / D, op0=ALU.add, op1=ALU.mult)
            g2z = lane4("g2z" + tok)
            nc.vector.tensor_scalar_add(g2z[:], col(base + 1), s_g2f_c[:])
            gbz = lane4("gbz" + tok)
            nc.vector.tensor_scalar_add(gbz[:], col(base + 2), s_gbf_c[:])
            var = lane4("var2" + tok)
            nc.vector.tensor_mul(var[:], muz[:], muz[:])
            nc.vector.scalar_tensor_tensor(var[:], col(base + 32), 1.0 / D,
                                           var[:], op0=ALU.mult,
                                           op1=ALU.subtract)
            rstd = lane4("rstd2" + tok)
            nc.scalar.activation(rstd[:], var[:], AF.Sqrt, bias=eps_col[:])
            nc.vector.reciprocal(rstd[:], rstd[:])
            g2q = col(base + 33)
            return muz, rstd, g2z, gbz, g2q

        mua, rsta, g2za, gbza, g2qa = ln2_lane(0, "a")
        mub2, rstb, g2zb, gbzb, g2qb = ln2_lane(64, "b")

        def gbt(mu, rstd, gbz, name):
            o_t = lane4(name)
            nc.vector.tensor_scalar_mul(o_t[:], mu[:], s_gb_c[:])
            nc.vector.tensor_tensor(o_t[:], gbz[:], o_t[:], op=ALU.subtract)
            nc.vector.tensor_mul(o_t[:], o_t[:], rstd[:])
            return o_t

        gbta = gbt(mua, rsta, gbza, "gbta")
        gbtb = gbt(mub2, rstb, gbzb, "gbtb")

        def normsq(mu, rstd, g2z, g2q, gbt_t, name):
            o_t = lane4(name)
            nc.vector.tensor_scalar_mul(o_t[:], mu[:], s_g2_c[:])
            nc.vector.scalar_tensor_tensor(o_t[:], g2z[:], -2.0, o_t[:],
                                           op0=ALU.mult, op1=ALU.add)
            nc.vector.tensor_mul(o_t[:], o_t[:], mu[:])
            nc.vector.tensor_tensor(o_t[:], o_t[:], g2q, op=ALU.add)
            nc.vector.tensor_mul(o_t[:], o_t[:], rstd[:])
            nc.vector.tensor_mul(o_t[:], o_t[:], rstd[:])
            nc.vector.scalar_tensor_tensor(o_t[:], gbt_t[:], 2.0, o_t[:],
                                           op0=ALU.mult, op1=ALU.add)
            nc.vector.tensor_scalar_add(o_t[:], o_t[:], s_bb_c[:])
            return o_t

        n2a = normsq(mua, rsta, g2za, g2qa, gbta, "n2a")
        n2b = normsq(mub2, rstb, g2zb, g2qb, gbtb, "n2b")

        d01 = lane4("d01")
        nc.vector.tensor_scalar_mul(d01[:], mub2[:], s_g2_c[:])
        nc.vector.tensor_tensor(d01[:], d01[:], g2zb[:], op=ALU.subtract)
        nc.vector.tensor_mul(d01[:], d01[:], mua[:])
        t2 = lane4("t2")
        nc.vector.tensor_mul(t2[:], mub2[:], g2za[:])
        nc.vector.tensor_tensor(d01[:], d01[:], t2[:], op=ALU.subtract)
        nc.vector.tensor_tensor(d01[:], col(3), d01[:], op=ALU.add)
        nc.vector.tensor_mul(d01[:], d01[:], rsta[:])
        nc.vector.tensor_mul(d01[:], d01[:], rstb[:])
        nc.vector.tensor_add(d01[:], d01[:], gbta[:])
        nc.vector.tensor_add(d01[:], d01[:], gbtb[:])
        nc.vector.tensor_scalar_add(d01[:], d01[:], s_bb_c[:])

        den = lane4("den")
        nc.scalar.activation(n2a[:], n2a[:], AF.Sqrt)
        nc.vector.tensor_scalar_max(n2a[:], n2a[:], EPS_COS)
        nc.scalar.activation(n2b[:], n2b[:], AF.Sqrt)
        nc.vector.tensor_scalar_max(n2b[:], n2b[:], EPS_COS)
        nc.vector.tensor_mul(den[:], n2a[:], n2b[:])
        nc.vector.reciprocal(den[:], den[:])
        atg_T = lane4("atg_T")
        nc.vector.tensor_mul(atg_T[:], d01[:], den[:])

        # transpose back [128,4] -> [4,128] and write out
        pback = ps_mm()
        nc.tensor.transpose(pback[0:4, 0:P], atg_T[:], ident_sb[:])
        atg_row = act.tile([4, P], F32, tag="atg_row", bufs=2, name="atg_row")
        nc.vector.tensor_copy(atg_row[:], pback[0:4, 0:P])
        nc.gpsimd.dma_start(
            t["out"][1:2, ts(mt, NP)].rearrange("o (b q) -> (o b) q", q=P),
            atg_row[:])


# ===================== host side =====================

def kernel(**inputs):
    f32 = np.float32
    bf16 = ml_dtypes.bfloat16
    fp8 = ml_dtypes.float8_e4m3
    txt = np.ascontiguousarray(
        np.asarray(inputs["text_embeddings"], f32).reshape(S, D))
    cand_full = np.asarray(inputs["candidate_embeddings"], f32).reshape(M * K, D)
    cand_bf = np.ascontiguousarray(cand_full.astype(bf16))
    starts = np.asarray(inputs["mention_starts"], np.int64)
    spans = np.asarray(inputs["span_lengths"], np.int64)
    ends = starts + spans
    c_start = np.maximum(0, starts - CTX)
    c_end = np.minimum(S - 1, ends + CTX)

    def w(name):
        return np.asarray(inputs[name], f32)

    def strips_oc(wmat, n_in, n_out):
        # [in, out] -> [n_out, P, n_in*P]  (strip oc: [p, i, q])
        a = wmat.reshape(n_in, P, n_out, P)
        return np.ascontiguousarray(a.transpose(2, 1, 0, 3).reshape(
            n_out, P, n_in * P))

    def resident(wmat):
        # [in, out] -> [P, FC(oc), FC(ic), P]
        a = wmat.reshape(FC, P, FC, P)
        return np.ascontiguousarray(a.transpose(1, 2, 0, 3))

    ffn_dt = fp8 if FP8_FFN else bf16
    fscale = W_SCALE if FP8_FFN else 1.0
    consts = {
        "ident": np.eye(P, dtype=f32),
        "identb": np.eye(P, dtype=f32).astype(bf16),
        "identb64": (np.eye(P, dtype=f32) * W_SCALE).astype(bf16),
        "hmat": np.repeat(np.eye(H, dtype=f32), DH, axis=0).astype(bf16),
        "i8neg": (-np.eye(H, dtype=f32)).astype(bf16),
        "wq_r": (resident(w("wq")) * W_SCALE).astype(fp8),
        "wk_r": (resident(w("wk")) * W_SCALE).astype(fp8),
        "wv_r": (resident(w("wv")) * W_SCALE).astype(fp8),
        "wo_r": (resident(w("wo")) * W_SCALE).astype(fp8),
        "w1b_r": resident(w("relik_w1")[D:]).astype(bf16),
        "w1a_s": strips_oc(w("relik_w1")[:D], FC, FC).astype(bf16),
        "u1a_s": strips_oc(w("uni_w1")[:D], FC, FC).astype(bf16),
        "u1b_s": strips_oc(w("uni_w1")[D:], FC, FC).astype(bf16),
        "fw1_s": (strips_oc(w("ffn_w1"), FC, HFC) * fscale).astype(ffn_dt),
        "fw2_s": (strips_oc(w("ffn_w2"), HFC, FC) * fscale).astype(ffn_dt),
    }
    vnames = ["relik_b1", "relik_w2", "bq", "bk", "bv", "bo",
              "ln1_g", "ln1_b", "ffn_b1", "ffn_b2",
              "ln2_g", "ln2_b", "uni_b1", "uni_w2"]
    weights = {n: np.ascontiguousarray(np.asarray(inputs[n], f32))
               for n in vnames}
    weights["relik_b2"] = np.asarray(inputs["relik_b2"], f32).reshape(1, 1)
    weights["uni_b2"] = np.ascontiguousarray(
        np.asarray(inputs["uni_b2"], f32).reshape(1, D))

    in_maps = []
    for core in range(NCORES):
        sl = slice(core * M_LOC, (core + 1) * M_LOC)
        # selector matrices with 1/len folded (pure index metadata)
        mark = np.zeros((S + 1, 2, M_LOC), f32)
        ar = np.arange(M_LOC)
        vm = 1.0 / (spans[sl] + 1).astype(f32)
        np.add.at(mark, (starts[sl], 0, ar), vm)
        np.add.at(mark, (ends[sl] + 1, 0, ar), -vm)
        vc = 1.0 / (c_end[sl] - c_start[sl]).astype(f32)
        np.add.at(mark, (c_start[sl], 1, ar), vc)
        np.add.at(mark, (c_end[sl], 1, ar), -vc)
        selm = np.cumsum(mark[:S], axis=0).reshape(NCH, P, 2 * M_LOC)
        im = {
            "txt": txt.astype(np.float16),
            "sel": np.ascontiguousarray(selm.astype(np.float16)),
            "cand": cand_bf[core * PAIRS:(core + 1) * PAIRS],
        }
        im.update(consts)
        im.update(weights)
        in_maps.append(im)

    if "nc" not in _NC_CACHE:
        _NC_CACHE["nc"] = _build_nc()
    nc = _NC_CACHE["nc"]

    results = bass_utils.run_bass_kernel_spmd(
        nc, in_maps, core_ids=list(range(NCORES))).results

    out = np.zeros((3, M, K), f32)
    for core in range(NCORES):
        sl = slice(core * M_LOC, (core + 1) * M_LOC)
        out[:, sl, :] = results[core]["out"].reshape(3, M_LOC, K)
    return out


if __name__ == "__main__":
    nc = _build_nc()
    print("built ok")


# revision 46
# speedup vs baseline: 1.8755x; 1.0148x over previous
"""Trainium2 Bass kernel for nn_EntityResolutionProcessor.

Data-parallel over mentions (M=1024 -> 128/core on 8 cores).
Host side: weights pre-converted to bf16/fp8 strip-major layouts,
candidates pre-converted to bf16, mention/context selector matrices
(index metadata with 1/len folded) built in numpy.
Device side per core:
  phase0: stream text chunks; mention/context means as feature-major
          selector matmuls accumulated in SBUF; per-mention projections.
  8 macro-tiles of 512 pairs: candidate DMA + PE transpose, k/v/q
  projections from SBUF-resident weights, relik/unirel heads, 2-token
  attention via sigmoid softmax, wo + LN1, FFN (fp8 DoubleRow), LN2 +
  cosine via sufficient statistics with the per-pair lane algebra
  transposed to pair-major so it runs 128 lanes wide.
"""

from contextlib import ExitStack

import ml_dtypes
import numpy as np

import concourse.bass as bass
import concourse.mybir as mybir
import concourse.tile as tile
from concourse import bacc, bass_isa, bass_utils
from concourse.bass import ds, ts

S, D, M, K, H = 4096, 768, 1024, 32, 8
DH = D // H
CTX = 10
NCORES = 8
P = 128
FC = D // P                     # 6 feature chunks
HFC = 4 * D // P                # 24 ffn hidden chunks
M_LOC = M // NCORES             # 128 mentions per core
PAIRS = M_LOC * K               # 4096 pairs per core
NP = 512                        # pairs per macro tile
G = NP // K                     # 16 mentions per macro tile
NMACRO = PAIRS // NP            # 8
NCH = S // P                    # 32 text chunks
ISQ = 1.0 / float(np.sqrt(np.float32(DH)))
EPS_LN = 1e-5
EPS_COS = 1e-8

FP8_FFN = True                  # fp8 DoubleRow FFN matmuls
W_SCALE = 64.0                  # fp8 weight scale (folded out at eviction)

F32 = mybir.dt.float32
BF16 = mybir.dt.bfloat16
FP16 = mybir.dt.float16
FP8 = mybir.dt.float8e4
I32 = mybir.dt.int32
AF = mybir.ActivationFunctionType
ALU = mybir.AluOpType
DR = mybir.MatmulPerfMode.DoubleRow

_NC_CACHE = {}

FFN_DT = FP8 if FP8_FFN else BF16


def _gk(ap):
    """view a [128, NP] AP as [128, G, K]"""
    return ap.rearrange("p (g k) -> p g k", g=G)


def _build_nc():
    nc = bacc.Bacc(
        "TRN2", target_bir_lowering=False, debug=False, num_devices=NCORES
    )

    def inp(name, shape, dtype=F32):
        return nc.dram_tensor(name, list(shape), dtype, kind="ExternalInput").ap()

    t = {}
    t["txt"] = inp("txt", [S, D], FP16)
    t["sel"] = inp("sel", [NCH, P, 2 * P], FP16)
    t["cand"] = inp("cand", [PAIRS, D], BF16)
    t["ident"] = inp("ident", [P, P])
    t["identb"] = inp("identb", [P, P], BF16)
    t["identb64"] = inp("identb64", [P, P], BF16)
    t["hmat"] = inp("hmat", [D, H], BF16)  # head indicator
    t["i8neg"] = inp("i8neg", [H, H], BF16)

    # resident weights [p, oc, ic, q]: attention fp8 (x W_SCALE), relik bf16
    for n in ["wq_r", "wk_r", "wv_r", "wo_r"]:
        t[n] = inp(n, [P, FC, FC, P], FP8)
    t["w1b_r"] = inp("w1b_r", [P, FC, FC, P], BF16)
    # streamed strips
    t["w1a_s"] = inp("w1a_s", [FC, P, FC * P], BF16)
    t["u1a_s"] = inp("u1a_s", [FC, P, FC * P], BF16)
    t["u1b_s"] = inp("u1b_s", [FC, P, FC * P], BF16)
    t["fw1_s"] = inp("fw1_s", [HFC, P, FC * P], FFN_DT)
    t["fw2_s"] = inp("fw2_s", [FC, P, HFC * P], FFN_DT)

    for n, shp in [("relik_b1", [D]), ("relik_w2", [D, 1]), ("relik_b2", [1, 1]),
                   ("bq", [D]), ("bk", [D]), ("bv", [D]), ("bo", [D]),
                   ("ln1_g", [D]), ("ln1_b", [D]),
                   ("ffn_b1", [4 * D]), ("ffn_b2", [D]),
                   ("ln2_g", [D]), ("ln2_b", [D]),
                   ("uni_b1", [D]), ("uni_w2", [D, D]), ("uni_b2", [1, D])]:
        t[n] = inp(n, shp)

    t["out"] = nc.dram_tensor("out", [3, PAIRS], F32, kind="ExternalOutput").ap()

    with tile.TileContext(nc) as tc:
        _body(nc, tc, t)
    nc.compile()
    return nc


def _vec6(v_ap, n=FC):
    """[D] dram AP -> [128, n] per-feature layout"""
    return v_ap.rearrange("(i p) -> p i", p=P)


def _body(nc, tc, t):
    with ExitStack() as _ctx:
        _body_inner(nc, tc, t, _ctx)


def _body_inner(nc, tc, t, _ctx):
    mm = lambda *a, **k: nc.tensor.matmul(*a, **k)

    # ---------------- pools ----------------
    psum = _ctx.enter_context(tc.tile_pool(name="psum", bufs=1, space="PSUM"))
    res = _ctx.enter_context(tc.tile_pool(name="res", bufs=1))

    def ps_mm(shape=(P, NP), dtype=F32):
        return psum.tile(list(shape), dtype, tag="mm", bufs=3,
                         padded_shape=[P, NP], name="ps_mm")

    def ps_score():
        return psum.tile([8, NP], F32, tag="score", bufs=1, name="ps_score")

    def ps_l1():
        return psum.tile([P, NP], F32, tag="l1", bufs=1, name="ps_l1")

    def ps_l2():
        return psum.tile([P, NP], F32, tag="l2", bufs=1, name="ps_l2")

    def ps_head():
        return psum.tile([1, NP], F32, tag="head", bufs=2, name="ps_head")

    # ---------------- resident constants ----------------
    def load_res(name, ap_src, shape, dtype=F32):
        tl = res.tile(list(shape), dtype, name=name)
        nc.gpsimd.dma_start(tl[:], ap_src)
        return tl

    # resident weights (issued first; load during phase0 on Pool queue)
    w_res = {}
    for n in ["wq_r", "wk_r", "wv_r", "wo_r"]:
        w_res[n] = load_res(n, t[n][:], [P, FC, FC, P], FP8)
    w_res["w1b_r"] = load_res("w1b_r", t["w1b_r"][:], [P, FC, FC, P], BF16)

    ident_sb = load_res("ident_sb", t["ident"][:], [P, P])
    identb_sb = load_res("identb_sb", t["identb"][:], [P, P], BF16)
    identb64_sb = load_res("identb64_sb", t["identb64"][:], [P, P], BF16)
    i8neg_sb = load_res("i8neg_sb", t["i8neg"][:], [H, H], BF16)
    h_sb = load_res("h_sb", t["hmat"].rearrange("(c p) h -> p c h", p=P),
                    [P, FC, H], BF16)
    ht_sb = load_res("ht_sb", t["hmat"].rearrange("(c p) h -> h c p", p=P),
                     [H, FC, P], BF16)
    negh_sb = res.tile([P, FC, H], BF16, name="negh_sb")
    nc.vector.tensor_scalar_mul(negh_sb[:], h_sb[:], -1.0)

    bq_sb = load_res("bq_sb", _vec6(t["bq"]), [P, FC])
    bk_sb = load_res("bk_sb", _vec6(t["bk"]), [P, FC])
    bv_sb = load_res("bv_sb", _vec6(t["bv"]), [P, FC])
    bo_sb = load_res("bo_sb", _vec6(t["bo"]), [P, FC])
    rb1_sb = load_res("rb1_sb", _vec6(t["relik_b1"]), [P, FC])
    ub1_sb = load_res("ub1_sb", _vec6(t["uni_b1"]), [P, FC])
    fb1_sb = load_res("fb1_sb", _vec6(t["ffn_b1"], HFC), [P, HFC])
    fb2_sb = load_res("fb2_sb", _vec6(t["ffn_b2"]), [P, FC])
    l1g_sb = load_res("l1g_sb", _vec6(t["ln1_g"]), [P, FC])
    l1b_sb = load_res("l1b_sb", _vec6(t["ln1_b"]), [P, FC])
    l2g_sb = load_res("l2g_sb", _vec6(t["ln2_g"]), [P, FC])
    l2b_sb = load_res("l2b_sb", _vec6(t["ln2_b"]), [P, FC])
    rw2_sb = load_res("rw2_sb",
                      t["relik_w2"].rearrange("(c p) o -> p c o", p=P),
                      [P, FC, 1], BF16)
    rb2_sb = load_res("rb2_sb", t["relik_b2"][:], [1, 1])

    ones_sb = res.tile([P, 1], BF16, name="ones_sb")
    nc.vector.memset(ones_sb[:], 1.0)
    onesf_sb = res.tile([P, 1], F32, name="onesf_sb")
    nc.vector.memset(onesf_sb[:], 1.0)
    ones_row = res.tile([1, P], BF16, name="ones_row")
    nc.vector.memset(ones_row[:], 1.0)
    eps_col = res.tile([P, 1], F32, name="eps_col")
    nc.vector.memset(eps_col[:], EPS_LN)
    onesf_sq = res.tile([P, P], F32, name="onesf_sq")
    nc.vector.memset(onesf_sq[:], 1.0)

    # stats lhsT [128, 6, 3]: cols = [1, g2^2, g2*b2] per feature chunk
    sl3_sb = res.tile([P, FC, 3], BF16, name="sl3_sb")
    g2sq_sb = res.tile([P, FC], F32, name="g2sq_sb")
    g2b2_sb = res.tile([P, FC], F32, name="g2b2_sb")
    nc.vector.tensor_mul(g2sq_sb[:], l2g_sb[:], l2g_sb[:])
    nc.vector.tensor_mul(g2b2_sb[:], l2g_sb[:], l2b_sb[:])
    for c in range(FC):
        nc.vector.tensor_copy(sl3_sb[:, c, 0:1], ones_sb[:])
        nc.vector.tensor_copy(sl3_sb[:, c, 1:2], g2sq_sb[:, c:c + 1])
        nc.vector.tensor_copy(sl3_sb[:, c, 2:3], g2b2_sb[:, c:c + 1])

    # scalar reductions of bias/gain vectors -> [128,1] columns (value
    # replicated on every partition; [0:1] slice gives the row-space form)
    def vec_sum(name, vecs):
        tmp = res.tile([P, FC], F32, name=name + "_t")
        if len(vecs) == 1:
            nc.vector.tensor_copy(tmp[:], vecs[0][:])
        else:
            nc.vector.tensor_mul(tmp[:], vecs[0][:], vecs[1][:])
            for v in vecs[2:]:
                nc.vector.tensor_mul(tmp[:], tmp[:], v[:])
        red = res.tile([P, 1], F32, name=name + "_r")
        nc.vector.tensor_reduce(red[:], tmp[:], axis=mybir.AxisListType.X,
                                op=ALU.add)
        pR = ps_mm((P, 1))
        mm(pR[:, 0:1], onesf_sq[:], red[:], start=True, stop=True)
        arr = res.tile([P, 1], F32, name=name)
        nc.vector.tensor_copy(arr[:], pR[:, 0:1])
        return arr

    s_bo_c = vec_sum("s_bo", [bo_sb])
    s_fb2_c = vec_sum("s_fb2", [fb2_sb])
    s_g2_c = vec_sum("s_g2", [l2g_sb, l2g_sb])
    s_gb_c = vec_sum("s_gb", [l2g_sb, l2b_sb])
    s_bb_c = vec_sum("s_bb", [l2b_sb, l2b_sb])
    s_g2f_c = vec_sum("s_g2f", [l2g_sb, l2g_sb, fb2_sb])
    s_gbf_c = vec_sum("s_gbf", [l2g_sb, l2b_sb, fb2_sb])
    s_bo = s_bo_c[0:1, 0:1]

    u2rs_sb = res.tile([P, FC], BF16, name="u2rs_sb")
    b2m_sb = res.tile([1, 1], F32, name="b2m_sb")

    # per-mention outputs (feature-major): mcT cols 0:128 mention, 128:256 ctx
    mc_T = res.tile([P, FC, 2 * P], F32, name="mc_T")
    m_T = mc_T[:, :, 0:P]
    m_Tb = res.tile([P, FC, P], BF16, name="m_Tb")
    c_Tb = res.tile([P, FC, P], BF16, name="c_Tb")
    m_q = res.tile([P, FC, P], BF16, name="m_q")
    m_k = res.tile([P, FC, P], BF16, name="m_k")
    m_v = res.tile([P, FC, P], BF16, name="m_v")
    m_relik = res.tile([P, FC, P], BF16, name="m_relik")
    c_uni = res.tile([P, FC, P], BF16, name="c_uni")
    s_aa_sb = res.tile([H, P], BF16, name="s_aa_sb")
    mprod_sb = res.tile([P, FC, P], BF16, name="mprod_sb")

    # ================= phase 0: uni_w2 reduce + selector means ==========
    with tc.tile_pool(name="p0", bufs=1) as p0:
        # uni_w2 row-sums (once)
        u2_sb = p0.tile([P, FC, D], F32, name="u2_sb")
        nc.gpsimd.dma_start(u2_sb[:], t["uni_w2"].rearrange("(i p) o -> p i o", p=P))
        u2r_f = p0.tile([P, FC], F32, name="u2r_f")
        nc.vector.tensor_reduce(u2r_f[:], u2_sb[:],
                                axis=mybir.AxisListType.X, op=ALU.add)
        nc.vector.tensor_copy(u2rs_sb[:], u2r_f[:])
        ub2_sb = p0.tile([1, D], F32, name="ub2_sb")
        nc.gpsimd.dma_start(ub2_sb[:], t["uni_b2"][:])
        b2r = p0.tile([1, 1], F32, name="b2r")
        nc.vector.tensor_reduce(b2r[:], ub2_sb[:], axis=mybir.AxisListType.X,
                                op=ALU.add)
        nc.scalar.activation(b2m_sb[:], b2r[:], AF.Copy, scale=1.0 / D)

        # ---- mention/context means: feature-major selector matmuls ----
        GRP = 4
        for g in range(NCH // GRP):
            txts = []
            sels = []
            for cc in range(GRP):
                c = g * GRP + cc
                txt_c = p0.tile([P, D], FP16, tag="txtc", bufs=2 * GRP + 2,
                                name="txt_c")
                nc.sync.dma_start(txt_c[:], t["txt"][c * P:(c + 1) * P, :])
                sel_c = p0.tile([P, 2 * P], FP16, tag="selc", bufs=2 * GRP + 2,
                                name="sel_c")
                nc.sync.dma_start(sel_c[:], t["sel"][c])
                txts.append(txt_c)
                sels.append(sel_c)
            for fc in range(FC):
                pA = ps_mm((P, 2 * P))
                for cc in range(GRP):
                    mm(pA[:], txts[cc][:, ts(fc, P)], sels[cc][:],
                       start=(cc == 0), stop=(cc == GRP - 1))
                if g == 0:
                    nc.vector.tensor_copy(mc_T[:, fc, :], pA[:])
                else:
                    nc.vector.tensor_tensor(mc_T[:, fc, :], mc_T[:, fc, :],
                                            pA[:], op=ALU.add)

        nc.vector.tensor_copy(m_Tb[:], mc_T[:, :, 0:P])
        nc.vector.tensor_copy(c_Tb[:], mc_T[:, :, P:2 * P])

    wts = _ctx.enter_context(tc.tile_pool(name="wts", bufs=1))
    act = _ctx.enter_context(tc.tile_pool(name="act", bufs=1))
    lane = _ctx.enter_context(tc.tile_pool(name="lane", bufs=1))

    # ---------- per-mention projections (bf16, N=128) ----------
    def load_strip(bf_dram, oc, tag="wstrip", bufs=6):
        st = wts.tile([P, FC, P], BF16, tag=tag, bufs=bufs, name="w_strip")
        nc.sync.dma_start(st[:],
                          bf_dram[oc].rearrange("p (i q) -> p i q", q=P))
        return st

    m_T8 = res.tile([P, FC, P], FP8, name="m_T8")
    nc.scalar.activation(m_T8[:], mc_T[:, :, 0:P], AF.Copy)
    for w_r, b_sb, out_t, src in (
        ("wq_r", bq_sb, m_q, m_T8),
        ("wk_r", bk_sb, m_k, m_T8),
        ("wv_r", bv_sb, m_v, m_T8),
        (None, rb1_sb, m_relik, m_Tb),
        (None, ub1_sb, c_uni, c_Tb),
    ):
        for oc in range(FC):
            pA = ps_mm((P, P))
            if w_r is None:
                strip_src = t["w1a_s"] if out_t is m_relik else t["u1a_s"]
                st_ = load_strip(strip_src, oc)
                for ic in range(FC):
                    mm(pA[:], st_[:, ic, :], src[:, ic, :],
                       start=(ic == 0), stop=(ic == FC - 1))
                sc = 1.0
            else:
                for i in range(FC // 2):
                    mm(pA[:], w_res[w_r][:, oc, 2 * i:2 * i + 2, :],
                       src[:, 2 * i:2 * i + 2, :],
                       start=(i == 0), stop=(i == FC // 2 - 1), perf_mode=DR)
                sc = 1.0 / W_SCALE
            nc.scalar.activation(out_t[:, oc, :], pA[:], AF.Identity,
                                 bias=b_sb[:, oc:oc + 1], scale=sc)

    # s_aa [8, 128]
    for c in range(FC):
        nc.vector.tensor_mul(mprod_sb[:, c, :], m_q[:, c, :], m_k[:, c, :])
    pS = ps_score()
    for c in range(FC):
        mm(pS[:, :P], h_sb[:, c, :], mprod_sb[:, c, :],
           start=(c == 0), stop=(c == FC - 1))
    nc.any.tensor_copy(s_aa_sb[:], pS[:, :P])

    def unit(tag, name, bufs=1):
        return act.tile([P, FC, NP], BF16, tag=tag, bufs=bufs, name=name)

    def chunk_t(name):
        return act.tile([P, NP], BF16, tag="tt", bufs=3, name=name)

    # ================= macro-tile loop =================
    # cosine finish of tile t is deferred into tile t+1 so the tiny
    # transpose-back matmul doesn't head-of-line block the PE queue while
    # the pair-major lane chain drains
    pending_fin = [None]
    for mt in range(NMACRO):
        g0 = mt * G
        gsl = ds(g0, G)

        lane_seq = [0]

        def lane_t(name, parts=1, width=NP):
            lane_seq[0] += 1
            return lane.tile([parts, width], F32, tag="lnrow", bufs=3,
                             name=f"{name}_{lane_seq[0]}")

        def mview(mt_tile, c):
            """mention-side bcast view [128, G, K]"""
            return mt_tile[:, c, gsl, None].to_broadcast([P, G, K])

        # ---- candidate load + PE transpose (bf16) ----
        cand_rm = act.tile([P, 4, D], BF16, tag="cand_rm", bufs=1,
                           name="cand_rm")
        nc.sync.dma_start(
            cand_rm[:],
            t["cand"].rearrange("(q p) d -> p q d", p=P)[:, ds(4 * mt, 4), :])
        candT = unit("candT", "candT", bufs=2)
        candT8 = act.tile([P, FC, NP], FP8, tag="candT8", bufs=2,
                          name="candT8")
        for fc in range(FC):
            pT = ps_mm(dtype=BF16)
            for pc in range(4):
                nc.tensor.transpose(pT[:, ts(pc, P)],
                                    cand_rm[:, pc, ts(fc, P)], identb_sb[:])
            nc.vector.tensor_copy(candT[:, fc, :], pT[:])
            nc.scalar.activation(candT8[:, fc, :], pT[:], AF.Copy)

        # ---- k/v projections (fp8 DoubleRow) ----
        k_b = unit("B", "k_b")
        v_b = unit("C", "v_b")
        for w_r, b_sb, out_t in (("wk_r", bk_sb, k_b), ("wv_r", bv_sb, v_b)):
            for oc in range(FC):
                pA = ps_mm()
                for i in range(FC // 2):
                    mm(pA[:], w_res[w_r][:, oc, 2 * i:2 * i + 2, :],
                       candT8[:, 2 * i:2 * i + 2, :],
                       start=(i == 0), stop=(i == FC // 2 - 1), perf_mode=DR)
                nc.scalar.activation(out_t[:, oc, :], pA[:], AF.Identity,
                                     bias=b_sb[:, oc:oc + 1],
                                     scale=1.0 / W_SCALE)

        if pending_fin[0] is not None:
            pending_fin[0]()
            pending_fin[0] = None

        # ---- attention scores ----
        pAB = ps_score()
        for c in range(FC):
            pr1 = chunk_t("pr1")
            nc.vector.tensor_tensor(_gk(pr1[:]), _gk(k_b[:, c, :]),
                                    mview(m_q, c), op=ALU.mult)
            mm(pAB[:], h_sb[:, c, :], pr1[:], start=(c == 0), stop=False)
        mm(pAB[:], i8neg_sb[:],
           s_aa_sb[:, gsl, None].to_broadcast([H, G, K]),
           start=False, stop=True)
        p_ab = act.tile([H, NP], BF16, tag="p_ab", bufs=2, name="p_ab")
        nc.scalar.activation(p_ab[:], pAB[:], AF.Sigmoid, scale=ISQ)

        pBA = ps_score()
        first = True
        for c in range(FC):
            pQ = ps_mm()
            for i in range(FC // 2):
                mm(pQ[:], w_res["wq_r"][:, c, 2 * i:2 * i + 2, :],
                   candT8[:, 2 * i:2 * i + 2, :],
                   start=(i == 0), stop=(i == FC // 2 - 1), perf_mode=DR)
            q_c = chunk_t("q_c")
            nc.scalar.activation(q_c[:], pQ[:], AF.Identity,
                                 bias=bq_sb[:, c:c + 1], scale=1.0 / W_SCALE)
            pr2 = chunk_t("pr2")
            nc.vector.tensor_tensor(_gk(pr2[:]), _gk(q_c[:]), mview(m_k, c),
                                    op=ALU.mult)
            mm(pBA[:], h_sb[:, c, :], pr2[:], start=first, stop=False)
            first = False
            pr3 = chunk_t("pr3")
            nc.vector.tensor_mul(pr3[:], q_c[:], k_b[:, c, :])
            mm(pBA[:], negh_sb[:, c, :], pr3[:],
               start=False, stop=(c == FC - 1))
        p_ba = act.tile([H, NP], BF16, tag="p_ba", bufs=2, name="p_ba")
        nc.scalar.activation(p_ba[:], pBA[:], AF.Sigmoid, scale=ISQ)

        # ---- attention outputs (fp8 for the wo matmul) ----
        o_a = act.tile([P, FC, NP], FP8, tag="o8a", bufs=1, name="o_a")
        o_b = act.tile([P, FC, NP], FP8, tag="o8b", bufs=1, name="o_b")
        for c in range(FC):
            dv = chunk_t("dv")
            nc.vector.tensor_tensor(_gk(dv[:]), _gk(v_b[:, c, :]),
                                    mview(m_v, c), op=ALU.subtract)
            pBC = ps_mm()
            mm(pBC[:], ht_sb[:, c, :], p_ab[:], start=True, stop=True)
            nc.vector.tensor_mul(o_a[:, c, :], pBC[:], dv[:])
            nc.vector.tensor_tensor(_gk(o_a[:, c, :]), _gk(o_a[:, c, :]),
                                    mview(m_v, c), op=ALU.add)
            pBC2 = ps_mm()
            mm(pBC2[:], ht_sb[:, c, :], p_ba[:], start=True, stop=True)
            nc.vector.tensor_mul(o_b[:, c, :], pBC2[:], dv[:])
            nc.vector.tensor_tensor(o_b[:, c, :], v_b[:, c, :], o_b[:, c, :],
                                    op=ALU.subtract)

        # ---- wo + residual (residual folded into psum via identity mm) ----
        r_a = unit("hh", "r_a", bufs=2)
        r_b = unit("hh", "r_b", bufs=2)
        for oc in range(FC):
            pA = ps_mm()
            for i in range(FC // 2):
                mm(pA[:], w_res["wo_r"][:, oc, 2 * i:2 * i + 2, :],
                   o_a[:, 2 * i:2 * i + 2, :],
                   start=(i == 0), stop=False, perf_mode=DR)
            mm(_gk(pA[:]), identb64_sb[:],
               m_Tb[:, oc, gsl, None].to_broadcast([P, G, K]),
               start=False, stop=True)
            nc.vector.tensor_scalar_mul(r_a[:, oc, :], pA[:], 1.0 / W_SCALE)
            pB = ps_mm()
            for i in range(FC // 2):
                mm(pB[:], w_res["wo_r"][:, oc, 2 * i:2 * i + 2, :],
                   o_b[:, 2 * i:2 * i + 2, :],
                   start=(i == 0), stop=False, perf_mode=DR)
            mm(pB[:], identb64_sb[:], candT[:, oc, :],
               start=False, stop=True)
            nc.vector.tensor_scalar_mul(r_b[:, oc, :], pB[:], 1.0 / W_SCALE)

        # ---- LN1: merged stat bank, rows a:(0,32) b:(64,96) ----
        pL1 = ps_l1()
        for r_t, base in ((r_a, 0), (r_b, 64)):
            for c in range(FC):
                sq = chunk_t("sq")
                nc.scalar.activation(sq[:], r_t[:, c, :], AF.Square,
                                     bias=bo_sb[:, c:c + 1])
                mm(pL1[base:base + 1, :], ones_sb[:], r_t[:, c, :],
                   start=(c == 0), stop=(c == FC - 1),
                   tile_position=(0, base))
                mm(pL1[base + 32:base + 33, :], ones_sb[:], sq[:],
                   start=(c == 0), stop=(c == FC - 1),
                   tile_position=(0, base + 32))

        # ---- relik / unirel heads (PE filler while LN1 lane math runs) ----
        for w_r, madd, htag, wv2, bias_ap, outrow, fn, scale in (
            ("w1b_r", m_relik, "C2", rw2_sb, rb2_sb[:], 0,
             AF.Identity, 1.0),
            (None, c_uni, "D", u2rs_sb, b2m_sb[:], 2,
             AF.Sigmoid, 1.0 / D),
        ):
            h_head = unit(htag, "hh_" + htag)
            for oc in range(FC):
                if w_r is None:
                    st_u = load_strip(t["u1b_s"], oc)
                    wsl = lambda ic: st_u[:, ic, :]
                else:
                    wsl = lambda ic: w_res[w_r][:, oc, ic, :]
                pA = ps_mm()
                for ic in range(FC):
                    mm(pA[:], wsl(ic), candT[:, ic, :],
                       start=(ic == 0), stop=False)
                mm(_gk(pA[:]), identb_sb[:], mview(madd, oc),
                   start=False, stop=True)
                nc.scalar.activation(h_head[:, oc, :], pA[:], AF.Relu)
            pH = ps_head()
            for c in range(FC):
                if wv2 is rw2_sb:
                    lhsT = wv2[:, c, :]
                else:
                    lhsT = wv2[:, c:c + 1]
                mm(pH[:], lhsT, h_head[:, c, :],
                   start=(c == 0), stop=(c == FC - 1))
            osl = lane_t("osl_" + htag)
            nc.scalar.activation(osl[:], pH[:], fn, bias=bias_ap, scale=scale)
            nc.gpsimd.dma_start(t["out"][outrow:outrow + 1, ts(mt, NP)], osl[:])

        def lnrow(name):
            lane_seq[0] += 1
            return lane.tile([1, NP], F32, tag="lnrow", bufs=3,
                             name=f"{name}_{lane_seq[0]}")

        def layernorm1(r_t, x1_t, base, tok):
            mu = lnrow("mu" + tok)
            nc.vector.tensor_scalar(mu[:], pL1[base:base + 1, :], s_bo,
                                    1.0 / D, op0=ALU.add, op1=ALU.mult)
            var = lnrow("var" + tok)
            nc.vector.tensor_mul(var[:], mu[:], mu[:])
            nc.vector.scalar_tensor_tensor(var[:], pL1[base + 32:base + 33, :],
                                           1.0 / D, var[:], op0=ALU.mult,
                                           op1=ALU.subtract)
            rstd = lnrow("rstd" + tok)
            nc.scalar.activation(rstd[:], var[:], AF.Sqrt,
                                 bias=eps_col[0:1, 0:1])
            nc.vector.reciprocal(rstd[:], rstd[:])
            mubf = act.tile([1, NP], BF16, tag="mubf", bufs=1, name="mubf")
            rstdbf = act.tile([1, NP], BF16, tag="rstdbf", bufs=1,
                              name="rstdbf")
            nc.vector.tensor_copy(mubf[:], mu[:])
            nc.vector.tensor_copy(rstdbf[:], rstd[:])
            mu_bc = ps_mm()
            rstd_bc = ps_mm()
            mm(mu_bc[:], ones_row[:], mubf[:], start=True, stop=True)
            mm(rstd_bc[:], ones_row[:], rstdbf[:], start=True, stop=True)
            for c in range(FC):
                nc.vector.tensor_tensor(x1_t[:, c, :], r_t[:, c, :],
                                        mu_bc[:], op=ALU.subtract)
            for c in range(FC):
                nc.vector.scalar_tensor_tensor(
                    x1_t[:, c, :], x1_t[:, c, :], bo_sb[:, c:c + 1],
                    rstd_bc[:], op0=ALU.add, op1=ALU.mult)
            for c in range(FC):
                nc.vector.tensor_scalar(
                    x1_t[:, c, :], x1_t[:, c, :], l1g_sb[:, c:c + 1],
                    l1b_sb[:, c:c + 1], op0=ALU.mult, op1=ALU.add)

        x1_a = unit("A", "x1_a")
        x1_b = unit("Bx", "x1_b")
        layernorm1(r_a, x1_a, 0, "a")
        layernorm1(r_b, x1_b, 64, "b")

        # ---- FFN (both tokens share each weight strip) ----
        if FP8_FFN:
            x1a_8 = act.tile([P, FC, NP], FP8, tag="x1a8", bufs=1, name="x1a8")
            x1b_8 = act.tile([P, FC, NP], FP8, tag="x1b8", bufs=1, name="x1b8")
            for c in range(FC):
                nc.scalar.activation(x1a_8[:, c, :], x1_a[:, c, :], AF.Copy)
                nc.scalar.activation(x1b_8[:, c, :], x1_b[:, c, :], AF.Copy)
            h_a = act.tile([P, HFC, NP], FP8, tag="h8a", bufs=1, name="h_a")
            h_b = act.tile([P, HFC, NP], FP8, tag="h8b", bufs=1, name="h_b")

            def ha_c(hc):
                return h_a[:, hc, :]

            def hb_c(hc):
                return h_b[:, hc, :]

            for hc in range(HFC):
                st = wts.tile([P, FC, P], FP8, tag="w1strip", bufs=4,
                              name="w1_strip")
                nc.sync.dma_start(
                    st[:], t["fw1_s"][hc].rearrange("p (i q) -> p i q", q=P))
                for x8_t, hcs in ((x1a_8, ha_c), (x1b_8, hb_c)):
                    pA = ps_mm()
                    for i in range(FC // 2):
                        mm(pA[:], st[:, 2 * i:2 * i + 2, :],
                           x8_t[:, 2 * i:2 * i + 2, :],
                           start=(i == 0), stop=(i == FC // 2 - 1),
                           perf_mode=DR)
                    nc.scalar.activation(hcs(hc), pA[:], AF.Relu,
                                         bias=fb1_sb[:, hc:hc + 1],
                                         scale=1.0 / W_SCALE)
            r2_a = unit("C2", "r2_a")
            r2_b = unit("D", "r2_b")
            for oc in range(FC):
                stw = wts.tile([P, HFC, P], FP8, tag="w2strip", bufs=2,
                               name="stw")
                nc.sync.dma_start(
                    stw[:],
                    t["fw2_s"][oc].rearrange("p (i q) -> p i q", q=P))
                for x1_t, h_t, r2_t in ((x1_a, h_a, r2_a), (x1_b, h_b, r2_b)):
                    pA = ps_mm()
                    for i in range(HFC // 2):
                        mm(pA[:], stw[:, 2 * i:2 * i + 2, :],
                           h_t[:, 2 * i:2 * i + 2, :],
                           start=(i == 0), stop=(i == HFC // 2 - 1),
                           perf_mode=DR)
                    nc.vector.scalar_tensor_tensor(
                        r2_t[:, oc, :], pA[:], 1.0 / W_SCALE, x1_t[:, oc, :],
                        op0=ALU.mult, op1=ALU.add)
        else:
            h_a = act.tile([P, HFC, NP], BF16, tag="h", bufs=1, name="h_a")
            hb = [unit("candT", "hb0"), unit("G", "hb1"),
                  unit("F", "hb2"), unit("hh", "hb3", bufs=2)]

            def ha_c(hc):
                return h_a[:, hc, :]

            def hb_c(hc):
                return hb[hc // FC][:, hc % FC, :]

            for hc in range(HFC):
                st = wts.tile([P, FC, P], BF16, tag="w1strip", bufs=4,
                              name="w1_strip")
                nc.sync.dma_start(
                    st[:], t["fw1_s"][hc].rearrange("p (i q) -> p i q", q=P))
                for x1_t, hcs in ((x1_a, ha_c), (x1_b, hb_c)):
                    pA = ps_mm()
                    for ic in range(FC):
                        mm(pA[:], st[:, ic, :], x1_t[:, ic, :],
                           start=(ic == 0), stop=(ic == FC - 1))
                    nc.scalar.activation(hcs(hc), pA[:],
                                         AF.Relu, bias=fb1_sb[:, hc:hc + 1])
            r2_a = unit("C2", "r2_a")
            r2_b = unit("D", "r2_b")
            for oc in range(FC):
                stw = wts.tile([P, HFC, P], BF16, tag="w2strip", bufs=2,
                               name="stw")
                nc.sync.dma_start(
                    stw[:],
                    t["fw2_s"][oc].rearrange("p (i q) -> p i q", q=P))
                for x1_t, hcs, r2_t in ((x1_a, ha_c, r2_a), (x1_b, hb_c, r2_b)):
                    pA = ps_mm()
                    for hc in range(HFC):
                        mm(pA[:], stw[:, hc, :], hcs(hc),
                           start=(hc == 0), stop=(hc == HFC - 1))
                    nc.vector.tensor_tensor(r2_t[:, oc, :], pA[:],
                                            x1_t[:, oc, :], op=ALU.add)

        # ---- LN2 + cosine via sufficient statistics ----
        # merged stat bank rows: a:(0..2, 32..33)  b:(64..66, 96..97)
        #   base+0: [sum, g2^2, g2*b2] . y      (y = r2 + fb2, via bias)
        #   base+32: [sum, g2^2] . y^2
        pL2 = ps_l2()
        pX = ps_head()
        for c in range(FC):
            sqa = chunk_t("sq")
            nc.scalar.activation(sqa[:], r2_a[:, c, :], AF.Square,
                                 bias=fb2_sb[:, c:c + 1])
            sqb = chunk_t("sq")
            nc.scalar.activation(sqb[:], r2_b[:, c, :], AF.Square,
                                 bias=fb2_sb[:, c:c + 1])
            rr = chunk_t("rr")
            nc.vector.tensor_scalar_add(rr[:], r2_b[:, c, :],
                                        fb2_sb[:, c:c + 1])
            nc.vector.scalar_tensor_tensor(rr[:], r2_a[:, c, :],
                                           fb2_sb[:, c:c + 1], rr[:],
                                           op0=ALU.add, op1=ALU.mult)
            mm(pL2[0:3, :], sl3_sb[:, c, 0:3], r2_a[:, c, :],
               start=(c == 0), stop=(c == FC - 1), tile_position=(0, 0))
            mm(pL2[32:34, :], sl3_sb[:, c, 0:2], sqa[:],
               start=(c == 0), stop=(c == FC - 1), tile_position=(0, 32))
            mm(pL2[64:67, :], sl3_sb[:, c, 0:3], r2_b[:, c, :],
               start=(c == 0), stop=(c == FC - 1), tile_position=(0, 64))
            mm(pL2[96:98, :], sl3_sb[:, c, 0:2], sqb[:],
               start=(c == 0), stop=(c == FC - 1), tile_position=(0, 96))
            mm(pX[:], sl3_sb[:, c, 1:2], rr[:],
               start=(c == 0), stop=(c == FC - 1))

        # evict stats + pX to SBUF, transpose to pair-major [128, 4, 128]
        # (pX lands in spare transposed column 3 via [1,128]^T matmuls)
        stat_sb = act.tile([P, NP], F32, tag="stat_sb", bufs=1, name="stat_sb")
        nc.vector.tensor_copy(stat_sb[:], pL2[:])
        pX_sb = act.tile([1, NP], F32, tag="pX_sb", bufs=1, name="pX_sb")
        nc.vector.tensor_copy(pX_sb[:], pX[:])
        pT = ps_mm()
        for b in range(4):
            nc.tensor.transpose(pT[:, ts(b, P)], stat_sb[:, ts(b, P)],
                                ident_sb[:])
        for b in range(4):
            mm(pT[:, b * P + 3:b * P + 4], pX_sb[0:1, ts(b, P)],
               onesf_sb[0:1, 0:1], start=True, stop=True)
        sT = act.tile([P, 4, P], F32, tag="sT", bufs=1, name="sT")
        nc.vector.tensor_copy(sT[:], pT[:])

        # pair-major lane algebra on [128, 4] slices
        def col(j):
            return sT[:, :, j]

        def lane4(name):
            lane_seq[0] += 1
            return lane.tile([P, 4], F32, tag=name + "4", bufs=1,
                             name=f"{name}4_{lane_seq[0]}")

        def ln2_lane(base, tok):
            muz = lane4("muz" + tok)
            nc.vector.tensor_scalar(muz[:], col(base + 0), s_fb2_c[:],
                                    1.0 / D, op0=ALU.add, op1=ALU.mult)
            g2z = lane4("g2z" + tok)
            nc.vector.tensor_scalar_add(g2z[:], col(base + 1), s_g2f_c[:])
            gbz = lane4("gbz" + tok)
            nc.vector.tensor_scalar_add(gbz[:], col(base + 2), s_gbf_c[:])
            var = lane4("var2" + tok)
            nc.vector.tensor_mul(var[:], muz[:], muz[:])
            nc.vector.scalar_tensor_tensor(var[:], col(base + 32), 1.0 / D,
                                           var[:], op0=ALU.mult,
                                           op1=ALU.subtract)
            rstd = lane4("rstd2" + tok)
            nc.scalar.activation(rstd[:], var[:], AF.Sqrt, bias=eps_col[:])
            nc.vector.reciprocal(rstd[:], rstd[:])
            g2q = col(base + 33)
            return muz, rstd, g2z, gbz, g2q

        mua, rsta, g2za, gbza, g2qa = ln2_lane(0, "a")
        mub2, rstb, g2zb, gbzb, g2qb = ln2_lane(64, "b")

        def gbt(mu, rstd, gbz, name):
            o_t = lane4(name)
            nc.vector.tensor_scalar_mul(o_t[:], mu[:], s_gb_c[:])
            nc.vector.tensor_tensor(o_t[:], gbz[:], o_t[:], op=ALU.subtract)
            nc.vector.tensor_mul(o_t[:], o_t[:], rstd[:])
            return o_t

        gbta = gbt(mua, rsta, gbza, "gbta")
        gbtb = gbt(mub2, rstb, gbzb, "gbtb")

        def normsq(mu, rstd, g2z, g2q, gbt_t, name):
            o_t = lane4(name)
            nc.vector.tensor_scalar_mul(o_t[:], mu[:], s_g2_c[:])
            nc.vector.scalar_tensor_tensor(o_t[:], g2z[:], -2.0, o_t[:],
                                           op0=ALU.mult, op1=ALU.add)
            nc.vector.tensor_mul(o_t[:], o_t[:], mu[:])
            nc.vector.tensor_tensor(o_t[:], o_t[:], g2q, op=ALU.add)
            nc.vector.tensor_mul(o_t[:], o_t[:], rstd[:])
            nc.vector.tensor_mul(o_t[:], o_t[:], rstd[:])
            nc.vector.scalar_tensor_tensor(o_t[:], gbt_t[:], 2.0, o_t[:],
                                           op0=ALU.mult, op1=ALU.add)
            nc.vector.tensor_scalar_add(o_t[:], o_t[:], s_bb_c[:])
            return o_t

        n2a = normsq(mua, rsta, g2za, g2qa, gbta, "n2a")
        n2b = normsq(mub2, rstb, g2zb, g2qb, gbtb, "n2b")

        d01 = lane4("d01")
        nc.vector.tensor_scalar_mul(d01[:], mub2[:], s_g2_c[:])
        nc.vector.tensor_tensor(d01[:], d01[:], g2zb[:], op=ALU.subtract)
        nc.vector.tensor_mul(d01[:], d01[:], mua[:])
        t2 = lane4("t2")
        nc.vector.tensor_mul(t2[:], mub2[:], g2za[:])
        nc.vector.tensor_tensor(d01[:], d01[:], t2[:], op=ALU.subtract)
        nc.vector.tensor_tensor(d01[:], col(3), d01[:], op=ALU.add)
        nc.vector.tensor_mul(d01[:], d01[:], rsta[:])
        nc.vector.tensor_mul(d01[:], d01[:], rstb[:])
        nc.vector.tensor_add(d01[:], d01[:], gbta[:])
        nc.vector.tensor_add(d01[:], d01[:], gbtb[:])
        nc.vector.tensor_scalar_add(d01[:], d01[:], s_bb_c[:])

        den = lane4("den")
        nc.scalar.activation(n2a[:], n2a[:], AF.Sqrt)
        nc.vector.tensor_scalar_max(n2a[:], n2a[:], EPS_COS)
        nc.scalar.activation(n2b[:], n2b[:], AF.Sqrt)
        nc.vector.tensor_scalar_max(n2b[:], n2b[:], EPS_COS)
        nc.vector.tensor_mul(den[:], n2a[:], n2b[:])
        nc.vector.reciprocal(den[:], den[:])
        atg_T = lane4("atg_T")
        nc.vector.tensor_mul(atg_T[:], d01[:], den[:])

        # transpose back [128,4] -> [4,128] and write out (deferred)
        def _finish(atg_T=atg_T, mt=mt):
            pback = ps_mm()
            nc.tensor.transpose(pback[0:4, 0:P], atg_T[:], ident_sb[:])
            atg_row = act.tile([4, P], F32, tag="atg_row", bufs=2,
                               name="atg_row")
            nc.vector.tensor_copy(atg_row[:], pback[0:4, 0:P])
            nc.gpsimd.dma_start(
                t["out"][1:2, ts(mt, NP)].rearrange("o (b q) -> (o b) q", q=P),
                atg_row[:])

        pending_fin[0] = _finish

    pending_fin[0]()


# ===================== host side =====================

def kernel(**inputs):
    f32 = np.float32
    bf16 = ml_dtypes.bfloat16
    fp8 = ml_dtypes.float8_e4m3
    txt = np.ascontiguousarray(
        np.asarray(inputs["text_embeddings"], f32).reshape(S, D))
    cand_full = np.asarray(inputs["candidate_embeddings"], f32).reshape(M * K, D)
    cand_bf = np.ascontiguousarray(cand_full.astype(bf16))
    starts = np.asarray(inputs["mention_starts"], np.int64)
    spans = np.asarray(inputs["span_lengths"], np.int64)
    ends = starts + spans
    c_start = np.maximum(0, starts - CTX)
    c_end = np.minimum(S - 1, ends + CTX)

    def w(name):
        return np.asarray(inputs[name], f32)

    def strips_oc(wmat, n_in, n_out):
        # [in, out] -> [n_out, P, n_in*P]  (strip oc: [p, i, q])
        a = wmat.reshape(n_in, P, n_out, P)
        return np.ascontiguousarray(a.transpose(2, 1, 0, 3).reshape(
            n_out, P, n_in * P))

    def resident(wmat):
        # [in, out] -> [P, FC(oc), FC(ic), P]
        a = wmat.reshape(FC, P, FC, P)
        return np.ascontiguousarray(a.transpose(1, 2, 0, 3))

    ffn_dt = fp8 if FP8_FFN else bf16
    fscale = W_SCALE if FP8_FFN else 1.0
    consts = {
        "ident": np.eye(P, dtype=f32),
        "identb": np.eye(P, dtype=f32).astype(bf16),
        "identb64": (np.eye(P, dtype=f32) * W_SCALE).astype(bf16),
        "hmat": np.repeat(np.eye(H, dtype=f32), DH, axis=0).astype(bf16),
        "i8neg": (-np.eye(H, dtype=f32)).astype(bf16),
        "wq_r": (resident(w("wq")) * W_SCALE).astype(fp8),
        "wk_r": (resident(w("wk")) * W_SCALE).astype(fp8),
        "wv_r": (resident(w("wv")) * W_SCALE).astype(fp8),
        "wo_r": (resident(w("wo")) * W_SCALE).astype(fp8),
        "w1b_r": resident(w("relik_w1")[D:]).astype(bf16),
        "w1a_s": strips_oc(w("relik_w1")[:D], FC, FC).astype(bf16),
        "u1a_s": strips_oc(w("uni_w1")[:D], FC, FC).astype(bf16),
        "u1b_s": strips_oc(w("uni_w1")[D:], FC, FC).astype(bf16),
        "fw1_s": (strips_oc(w("ffn_w1"), FC, HFC) * fscale).astype(ffn_dt),
        "fw2_s": (strips_oc(w("ffn_w2"), HFC, FC) * fscale).astype(ffn_dt),
    }
    vnames = ["relik_b1", "relik_w2", "bq", "bk", "bv", "bo",
              "ln1_g", "ln1_b", "ffn_b1", "ffn_b2",
              "ln2_g", "ln2_b", "uni_b1", "uni_w2"]
    weights = {n: np.ascontiguousarray(np.asarray(inputs[n], f32))
               for n in vnames}
    weights["relik_b2"] = np.asarray(inputs["relik_b2"], f32).reshape(1, 1)
    weights["uni_b2"] = np.ascontiguousarray(
        np.asarray(inputs["uni_b2"], f32).reshape(1, D))

    in_maps = []
    for core in range(NCORES):
        sl = slice(core * M_LOC, (core + 1) * M_LOC)
        # selector matrices with 1/len folded (pure index metadata)
        mark = np.zeros((S + 1, 2, M_LOC), f32)
        ar = np.arange(M_LOC)
        vm = 1.0 / (spans[sl] + 1).astype(f32)
        np.add.at(mark, (starts[sl], 0, ar), vm)
        np.add.at(mark, (ends[sl] + 1, 0, ar), -vm)
        vc = 1.0 / (c_end[sl] - c_start[sl]).astype(f32)
        np.add.at(mark, (c_start[sl], 1, ar), vc)
        np.add.at(mark, (c_end[sl], 1, ar), -vc)
        selm = np.cumsum(mark[:S], axis=0).reshape(NCH, P, 2 * M_LOC)
        im = {
            "txt": txt.astype(np.float16),
            "sel": np.ascontiguousarray(selm.astype(np.float16)),
            "cand": cand_bf[core * PAIRS:(core + 1) * PAIRS],
        }
        im.update(consts)
        im.update(weights)
        in_maps.append(im)

    if "nc" not in _NC_CACHE:
        _NC_CACHE["nc"] = _build_nc()
    nc = _NC_CACHE["nc"]

    results = bass_utils.run_bass_kernel_spmd(
        nc, in_maps, core_ids=list(range(NCORES))).results

    out = np.zeros((3, M, K), f32)
    for core in range(NCORES):
        sl = slice(core * M_LOC, (core + 1) * M_LOC)
        out[:, sl, :] = results[core]["out"].reshape(3, M_LOC, K)
    return out


if __name__ == "__main__":
    nc = _build_nc()
    print("built ok")


# revision 47
# speedup vs baseline: 1.8973x; 1.0116x over previous
"""Trainium2 Bass kernel for nn_EntityResolutionProcessor.

Data-parallel over mentions (M=1024 -> 128/core on 8 cores).
Host side: weights pre-converted to bf16/fp8 strip-major layouts,
candidates pre-converted to bf16, mention/context selector matrices
(index metadata with 1/len folded) built in numpy.
Device side per core:
  phase0: stream text chunks; mention/context means as feature-major
          selector matmuls accumulated in SBUF; per-mention projections.
  8 macro-tiles of 512 pairs: candidate DMA + PE transpose, k/v/q
  projections from SBUF-resident weights, relik/unirel heads, 2-token
  attention via sigmoid softmax, wo + LN1, FFN (fp8 DoubleRow), LN2 +
  cosine via sufficient statistics with the per-pair lane algebra
  transposed to pair-major so it runs 128 lanes wide.
"""

from contextlib import ExitStack

import ml_dtypes
import numpy as np

import concourse.bass as bass
import concourse.mybir as mybir
import concourse.tile as tile
from concourse import bacc, bass_isa, bass_utils
from concourse.bass import ds, ts

S, D, M, K, H = 4096, 768, 1024, 32, 8
DH = D // H
CTX = 10
NCORES = 8
P = 128
FC = D // P                     # 6 feature chunks
HFC = 4 * D // P                # 24 ffn hidden chunks
M_LOC = M // NCORES             # 128 mentions per core
PAIRS = M_LOC * K               # 4096 pairs per core
NP = 512                        # pairs per macro tile
G = NP // K                     # 16 mentions per macro tile
NMACRO = PAIRS // NP            # 8
NCH = S // P                    # 32 text chunks
ISQ = 1.0 / float(np.sqrt(np.float32(DH)))
EPS_LN = 1e-5
EPS_COS = 1e-8

FP8_FFN = True                  # fp8 DoubleRow FFN matmuls
W_SCALE = 64.0                  # fp8 weight scale (folded out at eviction)

F32 = mybir.dt.float32
BF16 = mybir.dt.bfloat16
FP16 = mybir.dt.float16
FP8 = mybir.dt.float8e4
I32 = mybir.dt.int32
AF = mybir.ActivationFunctionType
ALU = mybir.AluOpType
DR = mybir.MatmulPerfMode.DoubleRow

_NC_CACHE = {}

FFN_DT = FP8 if FP8_FFN else BF16


def _gk(ap):
    """view a [128, NP] AP as [128, G, K]"""
    return ap.rearrange("p (g k) -> p g k", g=G)


def _build_nc():
    nc = bacc.Bacc(
        "TRN2", target_bir_lowering=False, debug=False, num_devices=NCORES
    )

    def inp(name, shape, dtype=F32):
        return nc.dram_tensor(name, list(shape), dtype, kind="ExternalInput").ap()

    t = {}
    t["txt"] = inp("txt", [S, D], FP16)
    t["sel"] = inp("sel", [NCH, P, 2 * P], FP16)
    t["cand"] = inp("cand", [PAIRS, D], BF16)
    t["ident"] = inp("ident", [P, P])
    t["identb"] = inp("identb", [P, P], BF16)
    t["identb64"] = inp("identb64", [P, P], BF16)
    t["hmat"] = inp("hmat", [D, H], BF16)  # head indicator
    t["i8neg"] = inp("i8neg", [H, H], BF16)

    # resident weights [p, oc, ic, q]: attention fp8 (x W_SCALE), relik bf16
    for n in ["wq_r", "wk_r", "wv_r", "wo_r"]:
        t[n] = inp(n, [P, FC, FC, P], FP8)
    t["w1b_r"] = inp("w1b_r", [P, FC, FC, P], BF16)
    # streamed strips
    t["w1a_s"] = inp("w1a_s", [FC, P, FC * P], BF16)
    t["u1a_s"] = inp("u1a_s", [FC, P, FC * P], BF16)
    t["u1b_s"] = inp("u1b_s", [FC, P, FC * P], BF16)
    t["fw1_s"] = inp("fw1_s", [HFC, P, FC * P], FFN_DT)
    t["fw2_s"] = inp("fw2_s", [FC, P, HFC * P], FFN_DT)

    for n, shp in [("relik_b1", [D]), ("relik_w2", [D, 1]), ("relik_b2", [1, 1]),
                   ("bq", [D]), ("bk", [D]), ("bv", [D]), ("bo", [D]),
                   ("ln1_g", [D]), ("ln1_b", [D]),
                   ("ffn_b1", [4 * D]), ("ffn_b2", [D]),
                   ("ln2_g", [D]), ("ln2_b", [D]),
                   ("uni_b1", [D]), ("uni_w2", [D, D]), ("uni_b2", [1, D])]:
        t[n] = inp(n, shp)

    t["out"] = nc.dram_tensor("out", [3, PAIRS], F32, kind="ExternalOutput").ap()

    with tile.TileContext(nc) as tc:
        _body(nc, tc, t)
    nc.compile()
    return nc


def _vec6(v_ap, n=FC):
    """[D] dram AP -> [128, n] per-feature layout"""
    return v_ap.rearrange("(i p) -> p i", p=P)


def _body(nc, tc, t):
    with ExitStack() as _ctx:
        _body_inner(nc, tc, t, _ctx)


def _body_inner(nc, tc, t, _ctx):
    mm = lambda *a, **k: nc.tensor.matmul(*a, **k)

    # ---------------- pools ----------------
    psum = _ctx.enter_context(tc.tile_pool(name="psum", bufs=1, space="PSUM"))
    res = _ctx.enter_context(tc.tile_pool(name="res", bufs=1))

    def ps_mm(shape=(P, NP), dtype=F32):
        return psum.tile(list(shape), dtype, tag="mm", bufs=3,
                         padded_shape=[P, NP], name="ps_mm")

    def ps_score():
        return psum.tile([8, NP], F32, tag="score", bufs=1, name="ps_score")

    def ps_l1():
        return psum.tile([P, NP], F32, tag="l1", bufs=1, name="ps_l1")

    def ps_l2():
        return psum.tile([P, NP], F32, tag="l2", bufs=1, name="ps_l2")

    def ps_head():
        return psum.tile([1, NP], F32, tag="head", bufs=2, name="ps_head")

    # ---------------- resident constants ----------------
    def load_res(name, ap_src, shape, dtype=F32):
        tl = res.tile(list(shape), dtype, name=name)
        nc.gpsimd.dma_start(tl[:], ap_src)
        return tl

    # resident weights (issued first; load during phase0 on Pool queue)
    w_res = {}
    for n in ["wq_r", "wk_r", "wv_r", "wo_r"]:
        w_res[n] = load_res(n, t[n][:], [P, FC, FC, P], FP8)
    w_res["w1b_r"] = load_res("w1b_r", t["w1b_r"][:], [P, FC, FC, P], BF16)

    ident_sb = load_res("ident_sb", t["ident"][:], [P, P])
    identb_sb = load_res("identb_sb", t["identb"][:], [P, P], BF16)
    identb64_sb = load_res("identb64_sb", t["identb64"][:], [P, P], BF16)
    i8neg_sb = load_res("i8neg_sb", t["i8neg"][:], [H, H], BF16)
    h_sb = load_res("h_sb", t["hmat"].rearrange("(c p) h -> p c h", p=P),
                    [P, FC, H], BF16)
    ht_sb = load_res("ht_sb", t["hmat"].rearrange("(c p) h -> h c p", p=P),
                     [H, FC, P], BF16)
    negh_sb = res.tile([P, FC, H], BF16, name="negh_sb")
    nc.vector.tensor_scalar_mul(negh_sb[:], h_sb[:], -1.0)

    bq_sb = load_res("bq_sb", _vec6(t["bq"]), [P, FC])
    bk_sb = load_res("bk_sb", _vec6(t["bk"]), [P, FC])
    bv_sb = load_res("bv_sb", _vec6(t["bv"]), [P, FC])
    bo_sb = load_res("bo_sb", _vec6(t["bo"]), [P, FC])
    rb1_sb = load_res("rb1_sb", _vec6(t["relik_b1"]), [P, FC])
    ub1_sb = load_res("ub1_sb", _vec6(t["uni_b1"]), [P, FC])
    fb1_sb = load_res("fb1_sb", _vec6(t["ffn_b1"], HFC), [P, HFC])
    fb2_sb = load_res("fb2_sb", _vec6(t["ffn_b2"]), [P, FC])
    l1g_sb = load_res("l1g_sb", _vec6(t["ln1_g"]), [P, FC])
    l1b_sb = load_res("l1b_sb", _vec6(t["ln1_b"]), [P, FC])
    l2g_sb = load_res("l2g_sb", _vec6(t["ln2_g"]), [P, FC])
    l2b_sb = load_res("l2b_sb", _vec6(t["ln2_b"]), [P, FC])
    rw2_sb = load_res("rw2_sb",
                      t["relik_w2"].rearrange("(c p) o -> p c o", p=P),
                      [P, FC, 1], BF16)
    rb2_sb = load_res("rb2_sb", t["relik_b2"][:], [1, 1])

    ones_sb = res.tile([P, 1], BF16, name="ones_sb")
    nc.vector.memset(ones_sb[:], 1.0)
    onesf_sb = res.tile([P, 1], F32, name="onesf_sb")
    nc.vector.memset(onesf_sb[:], 1.0)
    ones_row = res.tile([1, P], BF16, name="ones_row")
    nc.vector.memset(ones_row[:], 1.0)
    eps_col = res.tile([P, 1], F32, name="eps_col")
    nc.vector.memset(eps_col[:], EPS_LN)
    onesf_sq = res.tile([P, P], F32, name="onesf_sq")
    nc.vector.memset(onesf_sq[:], 1.0)

    # stats lhsT [128, 6, 3]: cols = [1, g2^2, g2*b2] per feature chunk
    sl3_sb = res.tile([P, FC, 3], BF16, name="sl3_sb")
    g2sq_sb = res.tile([P, FC], F32, name="g2sq_sb")
    g2b2_sb = res.tile([P, FC], F32, name="g2b2_sb")
    nc.vector.tensor_mul(g2sq_sb[:], l2g_sb[:], l2g_sb[:])
    nc.vector.tensor_mul(g2b2_sb[:], l2g_sb[:], l2b_sb[:])
    for c in range(FC):
        nc.vector.tensor_copy(sl3_sb[:, c, 0:1], ones_sb[:])
        nc.vector.tensor_copy(sl3_sb[:, c, 1:2], g2sq_sb[:, c:c + 1])
        nc.vector.tensor_copy(sl3_sb[:, c, 2:3], g2b2_sb[:, c:c + 1])

    # scalar reductions of bias/gain vectors -> [128,1] columns (value
    # replicated on every partition; [0:1] slice gives the row-space form)
    def vec_sum(name, vecs):
        tmp = res.tile([P, FC], F32, name=name + "_t")
        if len(vecs) == 1:
            nc.vector.tensor_copy(tmp[:], vecs[0][:])
        else:
            nc.vector.tensor_mul(tmp[:], vecs[0][:], vecs[1][:])
            for v in vecs[2:]:
                nc.vector.tensor_mul(tmp[:], tmp[:], v[:])
        red = res.tile([P, 1], F32, name=name + "_r")
        nc.vector.tensor_reduce(red[:], tmp[:], axis=mybir.AxisListType.X,
                                op=ALU.add)
        pR = ps_mm((P, 1))
        mm(pR[:, 0:1], onesf_sq[:], red[:], start=True, stop=True)
        arr = res.tile([P, 1], F32, name=name)
        nc.vector.tensor_copy(arr[:], pR[:, 0:1])
        return arr

    s_bo_c = vec_sum("s_bo", [bo_sb])
    s_fb2_c = vec_sum("s_fb2", [fb2_sb])
    s_g2_c = vec_sum("s_g2", [l2g_sb, l2g_sb])
    s_gb_c = vec_sum("s_gb", [l2g_sb, l2b_sb])
    s_bb_c = vec_sum("s_bb", [l2b_sb, l2b_sb])
    s_g2f_c = vec_sum("s_g2f", [l2g_sb, l2g_sb, fb2_sb])
    s_gbf_c = vec_sum("s_gbf", [l2g_sb, l2b_sb, fb2_sb])
    s_bo = s_bo_c[0:1, 0:1]

    u2rs_sb = res.tile([P, FC], BF16, name="u2rs_sb")
    b2m_sb = res.tile([1, 1], F32, name="b2m_sb")

    # per-mention outputs (feature-major): mcT cols 0:128 mention, 128:256 ctx
    mc_T = res.tile([P, FC, 2 * P], F32, name="mc_T")
    m_T = mc_T[:, :, 0:P]
    m_Tb = res.tile([P, FC, P], BF16, name="m_Tb")
    c_Tb = res.tile([P, FC, P], BF16, name="c_Tb")
    m_q = res.tile([P, FC, P], BF16, name="m_q")
    m_k = res.tile([P, FC, P], BF16, name="m_k")
    m_v = res.tile([P, FC, P], BF16, name="m_v")
    m_relik = res.tile([P, FC, P], BF16, name="m_relik")
    c_uni = res.tile([P, FC, P], BF16, name="c_uni")
    s_aa_sb = res.tile([H, P], BF16, name="s_aa_sb")
    mprod_sb = res.tile([P, FC, P], BF16, name="mprod_sb")

    # ================= phase 0: uni_w2 reduce + selector means ==========
    with tc.tile_pool(name="p0", bufs=1) as p0:
        # uni_w2 row-sums (once)
        u2_sb = p0.tile([P, FC, D], F32, name="u2_sb")
        nc.gpsimd.dma_start(u2_sb[:], t["uni_w2"].rearrange("(i p) o -> p i o", p=P))
        u2r_f = p0.tile([P, FC], F32, name="u2r_f")
        nc.vector.tensor_reduce(u2r_f[:], u2_sb[:],
                                axis=mybir.AxisListType.X, op=ALU.add)
        nc.vector.tensor_copy(u2rs_sb[:], u2r_f[:])
        ub2_sb = p0.tile([1, D], F32, name="ub2_sb")
        nc.gpsimd.dma_start(ub2_sb[:], t["uni_b2"][:])
        b2r = p0.tile([1, 1], F32, name="b2r")
        nc.vector.tensor_reduce(b2r[:], ub2_sb[:], axis=mybir.AxisListType.X,
                                op=ALU.add)
        nc.scalar.activation(b2m_sb[:], b2r[:], AF.Copy, scale=1.0 / D)

        # ---- mention/context means: feature-major selector matmuls ----
        GRP = 4
        for g in range(NCH // GRP):
            txts = []
            sels = []
            for cc in range(GRP):
                c = g * GRP + cc
                txt_c = p0.tile([P, D], FP16, tag="txtc", bufs=2 * GRP + 2,
                                name="txt_c")
                nc.sync.dma_start(txt_c[:], t["txt"][c * P:(c + 1) * P, :])
                sel_c = p0.tile([P, 2 * P], FP16, tag="selc", bufs=2 * GRP + 2,
                                name="sel_c")
                nc.sync.dma_start(sel_c[:], t["sel"][c])
                txts.append(txt_c)
                sels.append(sel_c)
            for fc in range(FC):
                pA = ps_mm((P, 2 * P))
                for cc in range(GRP):
                    mm(pA[:], txts[cc][:, ts(fc, P)], sels[cc][:],
                       start=(cc == 0), stop=(cc == GRP - 1))
                if g == 0:
                    nc.vector.tensor_copy(mc_T[:, fc, :], pA[:])
                else:
                    nc.vector.tensor_tensor(mc_T[:, fc, :], mc_T[:, fc, :],
                                            pA[:], op=ALU.add)

        nc.vector.tensor_copy(m_Tb[:], mc_T[:, :, 0:P])
        nc.vector.tensor_copy(c_Tb[:], mc_T[:, :, P:2 * P])

    wts = _ctx.enter_context(tc.tile_pool(name="wts", bufs=1))
    act = _ctx.enter_context(tc.tile_pool(name="act", bufs=1))
    lane = _ctx.enter_context(tc.tile_pool(name="lane", bufs=1))

    # ---------- per-mention projections (bf16, N=128) ----------
    def load_strip(bf_dram, oc, tag="wstrip", bufs=6):
        st = wts.tile([P, FC, P], BF16, tag=tag, bufs=bufs, name="w_strip")
        nc.sync.dma_start(st[:],
                          bf_dram[oc].rearrange("p (i q) -> p i q", q=P))
        return st

    m_T8 = res.tile([P, FC, P], FP8, name="m_T8")
    nc.scalar.activation(m_T8[:], mc_T[:, :, 0:P], AF.Copy)
    for w_r, b_sb, out_t, src in (
        ("wq_r", bq_sb, m_q, m_T8),
        ("wk_r", bk_sb, m_k, m_T8),
        ("wv_r", bv_sb, m_v, m_T8),
        (None, rb1_sb, m_relik, m_Tb),
        (None, ub1_sb, c_uni, c_Tb),
    ):
        for oc in range(FC):
            pA = ps_mm((P, P))
            if w_r is None:
                strip_src = t["w1a_s"] if out_t is m_relik else t["u1a_s"]
                st_ = load_strip(strip_src, oc)
                for ic in range(FC):
                    mm(pA[:], st_[:, ic, :], src[:, ic, :],
                       start=(ic == 0), stop=(ic == FC - 1))
                sc = 1.0
            else:
                for i in range(FC // 2):
                    mm(pA[:], w_res[w_r][:, oc, 2 * i:2 * i + 2, :],
                       src[:, 2 * i:2 * i + 2, :],
                       start=(i == 0), stop=(i == FC // 2 - 1), perf_mode=DR)
                sc = 1.0 / W_SCALE
            nc.scalar.activation(out_t[:, oc, :], pA[:], AF.Identity,
                                 bias=b_sb[:, oc:oc + 1], scale=sc)

    # s_aa [8, 128]
    for c in range(FC):
        nc.vector.tensor_mul(mprod_sb[:, c, :], m_q[:, c, :], m_k[:, c, :])
    pS = ps_score()
    for c in range(FC):
        mm(pS[:, :P], h_sb[:, c, :], mprod_sb[:, c, :],
           start=(c == 0), stop=(c == FC - 1))
    nc.any.tensor_copy(s_aa_sb[:], pS[:, :P])

    def unit(tag, name, bufs=1):
        return act.tile([P, FC, NP], BF16, tag=tag, bufs=bufs, name=name)

    def chunk_t(name):
        return act.tile([P, NP], BF16, tag="tt", bufs=3, name=name)

    # ================= macro-tile loop =================
    # cosine finish of tile t is deferred into tile t+1 so the tiny
    # transpose-back matmul doesn't head-of-line block the PE queue while
    # the pair-major lane chain drains
    pending_fin = [None]
    for mt in range(NMACRO):
        g0 = mt * G
        gsl = ds(g0, G)

        lane_seq = [0]

        def lane_t(name, parts=1, width=NP):
            lane_seq[0] += 1
            return lane.tile([parts, width], F32, tag="lnrow", bufs=3,
                             name=f"{name}_{lane_seq[0]}")

        def mview(mt_tile, c):
            """mention-side bcast view [128, G, K]"""
            return mt_tile[:, c, gsl, None].to_broadcast([P, G, K])

        # ---- candidate load + PE transpose (bf16) ----
        cand_rm = act.tile([P, 4, D], BF16, tag="cand_rm", bufs=1,
                           name="cand_rm")
        nc.sync.dma_start(
            cand_rm[:],
            t["cand"].rearrange("(q p) d -> p q d", p=P)[:, ds(4 * mt, 4), :])
        candT = unit("candT", "candT", bufs=2)
        candT8 = act.tile([P, FC, NP], FP8, tag="candT8", bufs=2,
                          name="candT8")
        for fc in range(FC):
            pT = ps_mm(dtype=BF16)
            for pc in range(4):
                nc.tensor.transpose(pT[:, ts(pc, P)],
                                    cand_rm[:, pc, ts(fc, P)], identb_sb[:])
            nc.vector.tensor_copy(candT[:, fc, :], pT[:])
            nc.scalar.activation(candT8[:, fc, :], pT[:], AF.Copy)

        # ---- k/v projections (fp8 DoubleRow) ----
        k_b = unit("B", "k_b")
        v_b = unit("C", "v_b")
        for w_r, b_sb, out_t in (("wk_r", bk_sb, k_b), ("wv_r", bv_sb, v_b)):
            for oc in range(FC):
                pA = ps_mm()
                for i in range(FC // 2):
                    mm(pA[:], w_res[w_r][:, oc, 2 * i:2 * i + 2, :],
                       candT8[:, 2 * i:2 * i + 2, :],
                       start=(i == 0), stop=(i == FC // 2 - 1), perf_mode=DR)
                nc.scalar.activation(out_t[:, oc, :], pA[:], AF.Identity,
                                     bias=b_sb[:, oc:oc + 1],
                                     scale=1.0 / W_SCALE)

        if pending_fin[0] is not None:
            pending_fin[0]()
            pending_fin[0] = None

        # ---- attention scores ----
        pAB = ps_score()
        for c in range(FC):
            pr1 = chunk_t("pr1")
            nc.vector.tensor_tensor(_gk(pr1[:]), _gk(k_b[:, c, :]),
                                    mview(m_q, c), op=ALU.mult)
            mm(pAB[:], h_sb[:, c, :], pr1[:], start=(c == 0), stop=False)
        mm(pAB[:], i8neg_sb[:],
           s_aa_sb[:, gsl, None].to_broadcast([H, G, K]),
           start=False, stop=True)
        p_ab = act.tile([H, NP], BF16, tag="p_ab", bufs=2, name="p_ab")
        nc.scalar.activation(p_ab[:], pAB[:], AF.Sigmoid, scale=ISQ)

        pBA = ps_score()
        first = True
        for c in range(FC):
            pQ = ps_mm()
            for i in range(FC // 2):
                mm(pQ[:], w_res["wq_r"][:, c, 2 * i:2 * i + 2, :],
                   candT8[:, 2 * i:2 * i + 2, :],
                   start=(i == 0), stop=(i == FC // 2 - 1), perf_mode=DR)
            q_c = chunk_t("q_c")
            nc.scalar.activation(q_c[:], pQ[:], AF.Identity,
                                 bias=bq_sb[:, c:c + 1], scale=1.0 / W_SCALE)
            pr2 = chunk_t("pr2")
            nc.vector.tensor_tensor(_gk(pr2[:]), _gk(q_c[:]), mview(m_k, c),
                                    op=ALU.mult)
            mm(pBA[:], h_sb[:, c, :], pr2[:], start=first, stop=False)
            first = False
            pr3 = chunk_t("pr3")
            nc.vector.tensor_mul(pr3[:], q_c[:], k_b[:, c, :])
            mm(pBA[:], negh_sb[:, c, :], pr3[:],
               start=False, stop=(c == FC - 1))
        p_ba = act.tile([H, NP], BF16, tag="p_ba", bufs=2, name="p_ba")
        nc.scalar.activation(p_ba[:], pBA[:], AF.Sigmoid, scale=ISQ)

        # ---- attention outputs (fp8 for the wo matmul) ----
        o_a = act.tile([P, FC, NP], FP8, tag="o8a", bufs=1, name="o_a")
        o_b = act.tile([P, FC, NP], FP8, tag="o8b", bufs=1, name="o_b")
        for c in range(FC):
            dv = chunk_t("dv")
            nc.vector.tensor_tensor(_gk(dv[:]), _gk(v_b[:, c, :]),
                                    mview(m_v, c), op=ALU.subtract)
            pBC = ps_mm()
            mm(pBC[:], ht_sb[:, c, :], p_ab[:], start=True, stop=True)
            nc.vector.tensor_mul(o_a[:, c, :], pBC[:], dv[:])
            nc.vector.tensor_tensor(_gk(o_a[:, c, :]), _gk(o_a[:, c, :]),
                                    mview(m_v, c), op=ALU.add)
            pBC2 = ps_mm()
            mm(pBC2[:], ht_sb[:, c, :], p_ba[:], start=True, stop=True)
            nc.vector.tensor_mul(o_b[:, c, :], pBC2[:], dv[:])
            nc.vector.tensor_tensor(o_b[:, c, :], v_b[:, c, :], o_b[:, c, :],
                                    op=ALU.subtract)

        # ---- wo + residual (residual folded into psum via identity mm) ----
        r_a = unit("hh", "r_a", bufs=2)
        r_b = unit("hh", "r_b", bufs=2)
        for oc in range(FC):
            pA = ps_mm()
            for i in range(FC // 2):
                mm(pA[:], w_res["wo_r"][:, oc, 2 * i:2 * i + 2, :],
                   o_a[:, 2 * i:2 * i + 2, :],
                   start=(i == 0), stop=False, perf_mode=DR)
            mm(_gk(pA[:]), identb64_sb[:],
               m_Tb[:, oc, gsl, None].to_broadcast([P, G, K]),
               start=False, stop=True)
            nc.vector.tensor_scalar_mul(r_a[:, oc, :], pA[:], 1.0 / W_SCALE)
            pB = ps_mm()
            for i in range(FC // 2):
                mm(pB[:], w_res["wo_r"][:, oc, 2 * i:2 * i + 2, :],
                   o_b[:, 2 * i:2 * i + 2, :],
                   start=(i == 0), stop=False, perf_mode=DR)
            mm(pB[:], identb64_sb[:], candT[:, oc, :],
               start=False, stop=True)
            nc.vector.tensor_scalar_mul(r_b[:, oc, :], pB[:], 1.0 / W_SCALE)

        # ---- LN1: merged stat bank, rows a:(0,32) b:(64,96) ----
        pL1 = ps_l1()
        for r_t, base in ((r_a, 0), (r_b, 64)):
            for c in range(FC):
                sq = chunk_t("sq")
                nc.scalar.activation(sq[:], r_t[:, c, :], AF.Square,
                                     bias=bo_sb[:, c:c + 1])
                mm(pL1[base:base + 1, :], ones_sb[:], r_t[:, c, :],
                   start=(c == 0), stop=(c == FC - 1),
                   tile_position=(0, base))
                mm(pL1[base + 32:base + 33, :], ones_sb[:], sq[:],
                   start=(c == 0), stop=(c == FC - 1),
                   tile_position=(0, base + 32))

        # ---- relik / unirel heads (PE filler while LN1 lane math runs) ----
        for w_r, madd, htag, wv2, bias_ap, outrow, fn, scale in (
            ("w1b_r", m_relik, "C2", rw2_sb, rb2_sb[:], 0,
             AF.Identity, 1.0),
            (None, c_uni, "D", u2rs_sb, b2m_sb[:], 2,
             AF.Sigmoid, 1.0 / D),
        ):
            h_head = unit(htag, "hh_" + htag)
            for oc in range(FC):
                if w_r is None:
                    st_u = load_strip(t["u1b_s"], oc)
                    wsl = lambda ic: st_u[:, ic, :]
                else:
                    wsl = lambda ic: w_res[w_r][:, oc, ic, :]
                pA = ps_mm()
                for ic in range(FC):
                    mm(pA[:], wsl(ic), candT[:, ic, :],
                       start=(ic == 0), stop=False)
                mm(_gk(pA[:]), identb_sb[:], mview(madd, oc),
                   start=False, stop=True)
                nc.scalar.activation(h_head[:, oc, :], pA[:], AF.Relu)
            pH = ps_head()
            for c in range(FC):
                if wv2 is rw2_sb:
                    lhsT = wv2[:, c, :]
                else:
                    lhsT = wv2[:, c:c + 1]
                mm(pH[:], lhsT, h_head[:, c, :],
                   start=(c == 0), stop=(c == FC - 1))
            osl = lane_t("osl_" + htag)
            nc.scalar.activation(osl[:], pH[:], fn, bias=bias_ap, scale=scale)
            nc.gpsimd.dma_start(t["out"][outrow:outrow + 1, ts(mt, NP)], osl[:])

        def lnrow(name):
            lane_seq[0] += 1
            return lane.tile([1, NP], F32, tag="lnrow", bufs=3,
                             name=f"{name}_{lane_seq[0]}")

        def layernorm1(r_t, x1_t, base, tok):
            mu = lnrow("mu" + tok)
            nc.vector.tensor_scalar(mu[:], pL1[base:base + 1, :], s_bo,
                                    1.0 / D, op0=ALU.add, op1=ALU.mult)
            var = lnrow("var" + tok)
            nc.vector.tensor_mul(var[:], mu[:], mu[:])
            nc.vector.scalar_tensor_tensor(var[:], pL1[base + 32:base + 33, :],
                                           1.0 / D, var[:], op0=ALU.mult,
                                           op1=ALU.subtract)
            rstd = lnrow("rstd" + tok)
            nc.scalar.activation(rstd[:], var[:], AF.Sqrt,
                                 bias=eps_col[0:1, 0:1])
            nc.vector.reciprocal(rstd[:], rstd[:])
            mubf = act.tile([1, NP], BF16, tag="mubf", bufs=1, name="mubf")
            rstdbf = act.tile([1, NP], BF16, tag="rstdbf", bufs=1,
                              name="rstdbf")
            nc.vector.tensor_copy(mubf[:], mu[:])
            nc.vector.tensor_copy(rstdbf[:], rstd[:])
            mu_bc = ps_mm()
            rstd_bc = ps_mm()
            mm(mu_bc[:], ones_row[:], mubf[:], start=True, stop=True)
            mm(rstd_bc[:], ones_row[:], rstdbf[:], start=True, stop=True)
            for c in range(FC):
                nc.vector.tensor_tensor(x1_t[:, c, :], r_t[:, c, :],
                                        mu_bc[:], op=ALU.subtract)
            for c in range(FC):
                nc.vector.scalar_tensor_tensor(
                    x1_t[:, c, :], x1_t[:, c, :], bo_sb[:, c:c + 1],
                    rstd_bc[:], op0=ALU.add, op1=ALU.mult)
            for c in range(FC):
                nc.scalar.activation(
                    x1_t[:, c, :], x1_t[:, c, :], AF.Identity,
                    scale=l1g_sb[:, c:c + 1], bias=l1b_sb[:, c:c + 1])

        x1_a = unit("A", "x1_a")
        x1_b = unit("Bx", "x1_b")
        layernorm1(r_a, x1_a, 0, "a")
        layernorm1(r_b, x1_b, 64, "b")

        # ---- FFN (both tokens share each weight strip) ----
        if FP8_FFN:
            x1a_8 = act.tile([P, FC, NP], FP8, tag="x1a8", bufs=1, name="x1a8")
            x1b_8 = act.tile([P, FC, NP], FP8, tag="x1b8", bufs=1, name="x1b8")
            for c in range(FC):
                nc.scalar.activation(x1a_8[:, c, :], x1_a[:, c, :], AF.Copy)
                nc.scalar.activation(x1b_8[:, c, :], x1_b[:, c, :], AF.Copy)
            h_a = act.tile([P, HFC, NP], FP8, tag="h8a", bufs=1, name="h_a")
            h_b = act.tile([P, HFC, NP], FP8, tag="h8b", bufs=1, name="h_b")

            def ha_c(hc):
                return h_a[:, hc, :]

            def hb_c(hc):
                return h_b[:, hc, :]

            for hc in range(HFC):
                st = wts.tile([P, FC, P], FP8, tag="w1strip", bufs=4,
                              name="w1_strip")
                nc.sync.dma_start(
                    st[:], t["fw1_s"][hc].rearrange("p (i q) -> p i q", q=P))
                for x8_t, hcs in ((x1a_8, ha_c), (x1b_8, hb_c)):
                    pA = ps_mm()
                    for i in range(FC // 2):
                        mm(pA[:], st[:, 2 * i:2 * i + 2, :],
                           x8_t[:, 2 * i:2 * i + 2, :],
                           start=(i == 0), stop=(i == FC // 2 - 1),
                           perf_mode=DR)
                    nc.scalar.activation(hcs(hc), pA[:], AF.Relu,
                                         bias=fb1_sb[:, hc:hc + 1],
                                         scale=1.0 / W_SCALE)
            r2_a = unit("C2", "r2_a")
            r2_b = unit("D", "r2_b")
            for oc in range(FC):
                stw = wts.tile([P, HFC, P], FP8, tag="w2strip", bufs=2,
                               name="stw")
                nc.sync.dma_start(
                    stw[:],
                    t["fw2_s"][oc].rearrange("p (i q) -> p i q", q=P))
                for x1_t, h_t, r2_t in ((x1_a, h_a, r2_a), (x1_b, h_b, r2_b)):
                    pA = ps_mm()
                    for i in range(HFC // 2):
                        mm(pA[:], stw[:, 2 * i:2 * i + 2, :],
                           h_t[:, 2 * i:2 * i + 2, :],
                           start=(i == 0), stop=(i == HFC // 2 - 1),
                           perf_mode=DR)
                    nc.vector.scalar_tensor_tensor(
                        r2_t[:, oc, :], pA[:], 1.0 / W_SCALE, x1_t[:, oc, :],
                        op0=ALU.mult, op1=ALU.add)
        else:
            h_a = act.tile([P, HFC, NP], BF16, tag="h", bufs=1, name="h_a")
            hb = [unit("candT", "hb0"), unit("G", "hb1"),
                  unit("F", "hb2"), unit("hh", "hb3", bufs=2)]

            def ha_c(hc):
                return h_a[:, hc, :]

            def hb_c(hc):
                return hb[hc // FC][:, hc % FC, :]

            for hc in range(HFC):
                st = wts.tile([P, FC, P], BF16, tag="w1strip", bufs=4,
                              name="w1_strip")
                nc.sync.dma_start(
                    st[:], t["fw1_s"][hc].rearrange("p (i q) -> p i q", q=P))
                for x1_t, hcs in ((x1_a, ha_c), (x1_b, hb_c)):
                    pA = ps_mm()
                    for ic in range(FC):
                        mm(pA[:], st[:, ic, :], x1_t[:, ic, :],
                           start=(ic == 0), stop=(ic == FC - 1))
                    nc.scalar.activation(hcs(hc), pA[:],
                                         AF.Relu, bias=fb1_sb[:, hc:hc + 1])
            r2_a = unit("C2", "r2_a")
            r2_b = unit("D", "r2_b")
            for oc in range(FC):
                stw = wts.tile([P, HFC, P], BF16, tag="w2strip", bufs=2,
                               name="stw")
                nc.sync.dma_start(
                    stw[:],
                    t["fw2_s"][oc].rearrange("p (i q) -> p i q", q=P))
                for x1_t, hcs, r2_t in ((x1_a, ha_c, r2_a), (x1_b, hb_c, r2_b)):
                    pA = ps_mm()
                    for hc in range(HFC):
                        mm(pA[:], stw[:, hc, :], hcs(hc),
                           start=(hc == 0), stop=(hc == HFC - 1))
                    nc.vector.tensor_tensor(r2_t[:, oc, :], pA[:],
                                            x1_t[:, oc, :], op=ALU.add)

        # ---- LN2 + cosine via sufficient statistics ----
        # merged stat bank rows: a:(0..2, 32..33)  b:(64..66, 96..97)
        #   base+0: [sum, g2^2, g2*b2] . y      (y = r2 + fb2, via bias)
        #   base+32: [sum, g2^2] . y^2
        pL2 = ps_l2()
        pX = ps_head()
        for c in range(FC):
            sqa = chunk_t("sq")
            nc.scalar.activation(sqa[:], r2_a[:, c, :], AF.Square,
                                 bias=fb2_sb[:, c:c + 1])
            sqb = chunk_t("sq")
            nc.scalar.activation(sqb[:], r2_b[:, c, :], AF.Square,
                                 bias=fb2_sb[:, c:c + 1])
            rr = chunk_t("rr")
            nc.vector.tensor_scalar_add(rr[:], r2_b[:, c, :],
                                        fb2_sb[:, c:c + 1])
            nc.vector.scalar_tensor_tensor(rr[:], r2_a[:, c, :],
                                           fb2_sb[:, c:c + 1], rr[:],
                                           op0=ALU.add, op1=ALU.mult)
            mm(pL2[0:3, :], sl3_sb[:, c, 0:3], r2_a[:, c, :],
               start=(c == 0), stop=(c == FC - 1), tile_position=(0, 0))
            mm(pL2[32:34, :], sl3_sb[:, c, 0:2], sqa[:],
               start=(c == 0), stop=(c == FC - 1), tile_position=(0, 32))
            mm(pL2[64:67, :], sl3_sb[:, c, 0:3], r2_b[:, c, :],
               start=(c == 0), stop=(c == FC - 1), tile_position=(0, 64))
            mm(pL2[96:98, :], sl3_sb[:, c, 0:2], sqb[:],
               start=(c == 0), stop=(c == FC - 1), tile_position=(0, 96))
            mm(pX[:], sl3_sb[:, c, 1:2], rr[:],
               start=(c == 0), stop=(c == FC - 1))

        # evict stats + pX to SBUF, transpose to pair-major [128, 4, 128]
        # (pX lands in spare transposed column 3 via [1,128]^T matmuls)
        stat_sb = act.tile([P, NP], F32, tag="stat_sb", bufs=1, name="stat_sb")
        nc.vector.tensor_copy(stat_sb[:], pL2[:])
        pX_sb = act.tile([1, NP], F32, tag="pX_sb", bufs=1, name="pX_sb")
        nc.vector.tensor_copy(pX_sb[:], pX[:])
        pT = ps_mm()
        for b in range(4):
            nc.tensor.transpose(pT[:, ts(b, P)], stat_sb[:, ts(b, P)],
                                ident_sb[:])
        for b in range(4):
            mm(pT[:, b * P + 3:b * P + 4], pX_sb[0:1, ts(b, P)],
               onesf_sb[0:1, 0:1], start=True, stop=True)
        sT = act.tile([P, 4, P], F32, tag="sT", bufs=1, name="sT")
        nc.vector.tensor_copy(sT[:], pT[:])

        # pair-major lane algebra on [128, 4] slices
        def col(j):
            return sT[:, :, j]

        def lane4(name):
            lane_seq[0] += 1
            return lane.tile([P, 4], F32, tag=name + "4", bufs=1,
                             name=f"{name}4_{lane_seq[0]}")

        def ln2_lane(base, tok):
            muz = lane4("muz" + tok)
            nc.vector.tensor_scalar(muz[:], col(base + 0), s_fb2_c[:],
                                    1.0 / D, op0=ALU.add, op1=ALU.mult)
            g2z = lane4("g2z" + tok)
            nc.vector.tensor_scalar_add(g2z[:], col(base + 1), s_g2f_c[:])
            gbz = lane4("gbz" + tok)
            nc.vector.tensor_scalar_add(gbz[:], col(base + 2), s_gbf_c[:])
            var = lane4("var2" + tok)
            nc.vector.tensor_mul(var[:], muz[:], muz[:])
            nc.vector.scalar_tensor_tensor(var[:], col(base + 32), 1.0 / D,
                                           var[:], op0=ALU.mult,
                                           op1=ALU.subtract)
            rstd = lane4("rstd2" + tok)
            nc.scalar.activation(rstd[:], var[:], AF.Sqrt, bias=eps_col[:])
            nc.vector.reciprocal(rstd[:], rstd[:])
            g2q = col(base + 33)
            return muz, rstd, g2z, gbz, g2q

        mua, rsta, g2za, gbza, g2qa = ln2_lane(0, "a")
        mub2, rstb, g2zb, gbzb, g2qb = ln2_lane(64, "b")

        def gbt(mu, rstd, gbz, name):
            o_t = lane4(name)
            nc.vector.tensor_scalar_mul(o_t[:], mu[:], s_gb_c[:])
            nc.vector.tensor_tensor(o_t[:], gbz[:], o_t[:], op=ALU.subtract)
            nc.vector.tensor_mul(o_t[:], o_t[:], rstd[:])
            return o_t

        gbta = gbt(mua, rsta, gbza, "gbta")
        gbtb = gbt(mub2, rstb, gbzb, "gbtb")

        def normsq(mu, rstd, g2z, g2q, gbt_t, name):
            o_t = lane4(name)
            nc.vector.tensor_scalar_mul(o_t[:], mu[:], s_g2_c[:])
            nc.vector.scalar_tensor_tensor(o_t[:], g2z[:], -2.0, o_t[:],
                                           op0=ALU.mult, op1=ALU.add)
            nc.vector.tensor_mul(o_t[:], o_t[:], mu[:])
            nc.vector.tensor_tensor(o_t[:], o_t[:], g2q, op=ALU.add)
            nc.vector.tensor_mul(o_t[:], o_t[:], rstd[:])
            nc.vector.tensor_mul(o_t[:], o_t[:], rstd[:])
            nc.vector.scalar_tensor_tensor(o_t[:], gbt_t[:], 2.0, o_t[:],
                                           op0=ALU.mult, op1=ALU.add)
            nc.vector.tensor_scalar_add(o_t[:], o_t[:], s_bb_c[:])
            return o_t

        n2a = normsq(mua, rsta, g2za, g2qa, gbta, "n2a")
        n2b = normsq(mub2, rstb, g2zb, g2qb, gbtb, "n2b")

        d01 = lane4("d01")
        nc.vector.tensor_scalar_mul(d01[:], mub2[:], s_g2_c[:])
        nc.vector.tensor_tensor(d01[:], d01[:], g2zb[:], op=ALU.subtract)
        nc.vector.tensor_mul(d01[:], d01[:], mua[:])
        t2 = lane4("t2")
        nc.vector.tensor_mul(t2[:], mub2[:], g2za[:])
        nc.vector.tensor_tensor(d01[:], d01[:], t2[:], op=ALU.subtract)
        nc.vector.tensor_tensor(d01[:], col(3), d01[:], op=ALU.add)
        nc.vector.tensor_mul(d01[:], d01[:], rsta[:])
        nc.vector.tensor_mul(d01[:], d01[:], rstb[:])
        nc.vector.tensor_add(d01[:], d01[:], gbta[:])
        nc.vector.tensor_add(d01[:], d01[:], gbtb[:])
        nc.vector.tensor_scalar_add(d01[:], d01[:], s_bb_c[:])

        den = lane4("den")
        nc.scalar.activation(n2a[:], n2a[:], AF.Sqrt)
        nc.vector.tensor_scalar_max(n2a[:], n2a[:], EPS_COS)
        nc.scalar.activation(n2b[:], n2b[:], AF.Sqrt)
        nc.vector.tensor_scalar_max(n2b[:], n2b[:], EPS_COS)
        nc.vector.tensor_mul(den[:], n2a[:], n2b[:])
        nc.vector.reciprocal(den[:], den[:])
        atg_T = lane4("atg_T")
        nc.vector.tensor_mul(atg_T[:], d01[:], den[:])

        # transpose back [128,4] -> [4,128] and write out (deferred)
        def _finish(atg_T=atg_T, mt=mt):
            pback = ps_mm()
            nc.tensor.transpose(pback[0:4, 0:P], atg_T[:], ident_sb[:])
            atg_row = act.tile([4, P], F32, tag="atg_row", bufs=2,
                               name="atg_row")
            nc.vector.tensor_copy(atg_row[:], pback[0:4, 0:P])
            nc.gpsimd.dma_start(
                t["out"][1:2, ts(mt, NP)].rearrange("o (b q) -> (o b) q", q=P),
                atg_row[:])

        pending_fin[0] = _finish

    pending_fin[0]()


# ===================== host side =====================

def kernel(**inputs):
    f32 = np.float32
    bf16 = ml_dtypes.bfloat16
    fp8 = ml_dtypes.float8_e4m3
    txt = np.ascontiguousarray(
        np.asarray(inputs["text_embeddings"], f32).reshape(S, D))
    cand_full = np.asarray(inputs["candidate_embeddings"], f32).reshape(M * K, D)
    cand_bf = np.ascontiguousarray(cand_full.astype(bf16))
    starts = np.asarray(inputs["mention_starts"], np.int64)
    spans = np.asarray(inputs["span_lengths"], np.int64)
    ends = starts + spans
    c_start = np.maximum(0, starts - CTX)
    c_end = np.minimum(S - 1, ends + CTX)

    def w(name):
        return np.asarray(inputs[name], f32)

    def strips_oc(wmat, n_in, n_out):
        # [in, out] -> [n_out, P, n_in*P]  (strip oc: [p, i, q])
        a = wmat.reshape(n_in, P, n_out, P)
        return np.ascontiguousarray(a.transpose(2, 1, 0, 3).reshape(
            n_out, P, n_in * P))

    def resident(wmat):
        # [in, out] -> [P, FC(oc), FC(ic), P]
        a = wmat.reshape(FC, P, FC, P)
        return np.ascontiguousarray(a.transpose(1, 2, 0, 3))

    ffn_dt = fp8 if FP8_FFN else bf16
    fscale = W_SCALE if FP8_FFN else 1.0
    consts = {
        "ident": np.eye(P, dtype=f32),
        "identb": np.eye(P, dtype=f32).astype(bf16),
        "identb64": (np.eye(P, dtype=f32) * W_SCALE).astype(bf16),
        "hmat": np.repeat(np.eye(H, dtype=f32), DH, axis=0).astype(bf16),
        "i8neg": (-np.eye(H, dtype=f32)).astype(bf16),
        "wq_r": (resident(w("wq")) * W_SCALE).astype(fp8),
        "wk_r": (resident(w("wk")) * W_SCALE).astype(fp8),
        "wv_r": (resident(w("wv")) * W_SCALE).astype(fp8),
        "wo_r": (resident(w("wo")) * W_SCALE).astype(fp8),
        "w1b_r": resident(w("relik_w1")[D:]).astype(bf16),
        "w1a_s": strips_oc(w("relik_w1")[:D], FC, FC).astype(bf16),
        "u1a_s": strips_oc(w("uni_w1")[:D], FC, FC).astype(bf16),
        "u1b_s": strips_oc(w("uni_w1")[D:], FC, FC).astype(bf16),
        "fw1_s": (strips_oc(w("ffn_w1"), FC, HFC) * fscale).astype(ffn_dt),
        "fw2_s": (strips_oc(w("ffn_w2"), HFC, FC) * fscale).astype(ffn_dt),
    }
    vnames = ["relik_b1", "relik_w2", "bq", "bk", "bv", "bo",
              "ln1_g", "ln1_b", "ffn_b1", "ffn_b2",
              "ln2_g", "ln2_b", "uni_b1", "uni_w2"]
    weights = {n: np.ascontiguousarray(np.asarray(inputs[n], f32))
               for n in vnames}
    weights["relik_b2"] = np.asarray(inputs["relik_b2"], f32).reshape(1, 1)
    weights["uni_b2"] = np.ascontiguousarray(
        np.asarray(inputs["uni_b2"], f32).reshape(1, D))

    in_maps = []
    for core in range(NCORES):
        sl = slice(core * M_LOC, (core + 1) * M_LOC)
        # selector matrices with 1/len folded (pure index metadata)
        mark = np.zeros((S + 1, 2, M_LOC), f32)
        ar = np.arange(M_LOC)
        vm = 1.0 / (spans[sl] + 1).astype(f32)
        np.add.at(mark, (starts[sl], 0, ar), vm)
        np.add.at(mark, (ends[sl] + 1, 0, ar), -vm)
        vc = 1.0 / (c_end[sl] - c_start[sl]).astype(f32)
        np.add.at(mark, (c_start[sl], 1, ar), vc)
        np.add.at(mark, (c_end[sl], 1, ar), -vc)
        selm = np.cumsum(mark[:S], axis=0).reshape(NCH, P, 2 * M_LOC)
        im = {
            "txt": txt.astype(np.float16),
            "sel": np.ascontiguousarray(selm.astype(np.float16)),
            "cand": cand_bf[core * PAIRS:(core + 1) * PAIRS],
        }
        im.update(consts)
        im.update(weights)
        in_maps.append(im)

    if "nc" not in _NC_CACHE:
        _NC_CACHE["nc"] = _build_nc()
    nc = _NC_CACHE["nc"]

    results = bass_utils.run_bass_kernel_spmd(
        nc, in_maps, core_ids=list(range(NCORES))).results

    out = np.zeros((3, M, K), f32)
    for core in range(NCORES):
        sl = slice(core * M_LOC, (core + 1) * M_LOC)
        out[:, sl, :] = results[core]["out"].reshape(3, M_LOC, K)
    return out


if __name__ == "__main__":
    nc = _build_nc()
    print("built ok")


# revision 48
# speedup vs baseline: 1.9035x; 1.0033x over previous
"""Trainium2 Bass kernel for nn_EntityResolutionProcessor.

Data-parallel over mentions (M=1024 -> 128/core on 8 cores).
Host side: weights pre-converted to bf16/fp8 strip-major layouts,
candidates pre-converted to bf16, mention/context selector matrices
(index metadata with 1/len folded) built in numpy.
Device side per core:
  phase0: stream text chunks; mention/context means as feature-major
          selector matmuls accumulated in SBUF; per-mention projections.
  8 macro-tiles of 512 pairs: candidate DMA + PE transpose, k/v/q
  projections from SBUF-resident weights, relik/unirel heads, 2-token
  attention via sigmoid softmax, wo + LN1, FFN (fp8 DoubleRow), LN2 +
  cosine via sufficient statistics with the per-pair lane algebra
  transposed to pair-major so it runs 128 lanes wide.
"""

from contextlib import ExitStack

import ml_dtypes
import numpy as np

import concourse.bass as bass
import concourse.mybir as mybir
import concourse.tile as tile
from concourse import bacc, bass_isa, bass_utils
from concourse.bass import ds, ts

S, D, M, K, H = 4096, 768, 1024, 32, 8
DH = D // H
CTX = 10
NCORES = 8
P = 128
FC = D // P                     # 6 feature chunks
HFC = 4 * D // P                # 24 ffn hidden chunks
M_LOC = M // NCORES             # 128 mentions per core
PAIRS = M_LOC * K               # 4096 pairs per core
NP = 512                        # pairs per macro tile
G = NP // K                     # 16 mentions per macro tile
NMACRO = PAIRS // NP            # 8
NCH = S // P                    # 32 text chunks
ISQ = 1.0 / float(np.sqrt(np.float32(DH)))
EPS_LN = 1e-5
EPS_COS = 1e-8

FP8_FFN = True                  # fp8 DoubleRow FFN matmuls
W_SCALE = 64.0                  # fp8 weight scale (folded out at eviction)

F32 = mybir.dt.float32
BF16 = mybir.dt.bfloat16
FP16 = mybir.dt.float16
FP8 = mybir.dt.float8e4
I32 = mybir.dt.int32
AF = mybir.ActivationFunctionType
ALU = mybir.AluOpType
DR = mybir.MatmulPerfMode.DoubleRow

_NC_CACHE = {}

FFN_DT = FP8 if FP8_FFN else BF16


def _gk(ap):
    """view a [128, NP] AP as [128, G, K]"""
    return ap.rearrange("p (g k) -> p g k", g=G)


def _build_nc():
    nc = bacc.Bacc(
        "TRN2", target_bir_lowering=False, debug=False, num_devices=NCORES
    )

    def inp(name, shape, dtype=F32):
        return nc.dram_tensor(name, list(shape), dtype, kind="ExternalInput").ap()

    t = {}
    t["txt"] = inp("txt", [S, D], FP16)
    t["sel"] = inp("sel", [NCH, P, 2 * P], FP16)
    t["cand"] = inp("cand", [PAIRS, D], BF16)
    t["ident"] = inp("ident", [P, P])
    t["identb"] = inp("identb", [P, P], BF16)
    t["identb64"] = inp("identb64", [P, P], BF16)
    t["hmat"] = inp("hmat", [D, H], BF16)  # head indicator
    t["i8neg"] = inp("i8neg", [H, H], BF16)

    # resident weights [p, oc, ic, q]: attention fp8 (x W_SCALE), relik bf16
    for n in ["wq_r", "wk_r", "wv_r", "wo_r"]:
        t[n] = inp(n, [P, FC, FC, P], FP8)
    t["w1b_r"] = inp("w1b_r", [P, FC, FC, P], BF16)
    # streamed strips
    t["w1a_s"] = inp("w1a_s", [FC, P, FC * P], BF16)
    t["u1a_s"] = inp("u1a_s", [FC, P, FC * P], BF16)
    t["u1b_s"] = inp("u1b_s", [FC, P, FC * P], BF16)
    t["fw1_s"] = inp("fw1_s", [HFC, P, FC * P], FFN_DT)
    t["fw2_s"] = inp("fw2_s", [FC, P, HFC * P], FFN_DT)

    for n, shp in [("relik_b1", [D]), ("relik_w2", [D, 1]), ("relik_b2", [1, 1]),
                   ("bq", [D]), ("bk", [D]), ("bv", [D]), ("bo", [D]),
                   ("ln1_g", [D]), ("ln1_b", [D]),
                   ("ffn_b1", [4 * D]), ("ffn_b2", [D]),
                   ("ln2_g", [D]), ("ln2_b", [D]),
                   ("uni_b1", [D]), ("uni_w2", [D, D]), ("uni_b2", [1, D])]:
        t[n] = inp(n, shp)

    t["out"] = nc.dram_tensor("out", [3, PAIRS], F32, kind="ExternalOutput").ap()

    with tile.TileContext(nc) as tc:
        _body(nc, tc, t)
    nc.compile()
    return nc


def _vec6(v_ap, n=FC):
    """[D] dram AP -> [128, n] per-feature layout"""
    return v_ap.rearrange("(i p) -> p i", p=P)


def _body(nc, tc, t):
    with ExitStack() as _ctx:
        _body_inner(nc, tc, t, _ctx)


def _body_inner(nc, tc, t, _ctx):
    mm = lambda *a, **k: nc.tensor.matmul(*a, **k)

    # ---------------- pools ----------------
    psum = _ctx.enter_context(tc.tile_pool(name="psum", bufs=1, space="PSUM"))
    res = _ctx.enter_context(tc.tile_pool(name="res", bufs=1))

    def ps_mm(shape=(P, NP), dtype=F32):
        return psum.tile(list(shape), dtype, tag="mm", bufs=3,
                         padded_shape=[P, NP], name="ps_mm")

    def ps_score():
        return psum.tile([8, NP], F32, tag="score", bufs=1, name="ps_score")

    def ps_l1():
        return psum.tile([P, NP], F32, tag="l1", bufs=1, name="ps_l1")

    def ps_l2():
        return psum.tile([P, NP], F32, tag="l2", bufs=1, name="ps_l2")

    def ps_head():
        return psum.tile([1, NP], F32, tag="head", bufs=2, name="ps_head")

    # ---------------- resident constants ----------------
    def load_res(name, ap_src, shape, dtype=F32):
        tl = res.tile(list(shape), dtype, name=name)
        nc.gpsimd.dma_start(tl[:], ap_src)
        return tl

    # resident weights (issued first; load during phase0 on Pool queue)
    w_res = {}
    for n in ["wq_r", "wk_r", "wv_r", "wo_r"]:
        w_res[n] = load_res(n, t[n][:], [P, FC, FC, P], FP8)
    w_res["w1b_r"] = load_res("w1b_r", t["w1b_r"][:], [P, FC, FC, P], BF16)

    ident_sb = load_res("ident_sb", t["ident"][:], [P, P])
    identb_sb = load_res("identb_sb", t["identb"][:], [P, P], BF16)
    identb64_sb = load_res("identb64_sb", t["identb64"][:], [P, P], BF16)
    i8neg_sb = load_res("i8neg_sb", t["i8neg"][:], [H, H], BF16)
    h_sb = load_res("h_sb", t["hmat"].rearrange("(c p) h -> p c h", p=P),
                    [P, FC, H], BF16)
    ht_sb = load_res("ht_sb", t["hmat"].rearrange("(c p) h -> h c p", p=P),
                     [H, FC, P], BF16)
    negh_sb = res.tile([P, FC, H], BF16, name="negh_sb")
    nc.vector.tensor_scalar_mul(negh_sb[:], h_sb[:], -1.0)

    bq_sb = load_res("bq_sb", _vec6(t["bq"]), [P, FC])
    bk_sb = load_res("bk_sb", _vec6(t["bk"]), [P, FC])
    bv_sb = load_res("bv_sb", _vec6(t["bv"]), [P, FC])
    bo_sb = load_res("bo_sb", _vec6(t["bo"]), [P, FC])
    rb1_sb = load_res("rb1_sb", _vec6(t["relik_b1"]), [P, FC])
    ub1_sb = load_res("ub1_sb", _vec6(t["uni_b1"]), [P, FC])
    fb1_sb = load_res("fb1_sb", _vec6(t["ffn_b1"], HFC), [P, HFC])
    fb2_sb = load_res("fb2_sb", _vec6(t["ffn_b2"]), [P, FC])
    l1g_sb = load_res("l1g_sb", _vec6(t["ln1_g"]), [P, FC])
    l1b_sb = load_res("l1b_sb", _vec6(t["ln1_b"]), [P, FC])
    l2g_sb = load_res("l2g_sb", _vec6(t["ln2_g"]), [P, FC])
    l2b_sb = load_res("l2b_sb", _vec6(t["ln2_b"]), [P, FC])
    rw2_sb = load_res("rw2_sb",
                      t["relik_w2"].rearrange("(c p) o -> p c o", p=P),
                      [P, FC, 1], BF16)
    rb2_sb = load_res("rb2_sb", t["relik_b2"][:], [1, 1])

    ones_sb = res.tile([P, 1], BF16, name="ones_sb")
    nc.vector.memset(ones_sb[:], 1.0)
    onesf_sb = res.tile([P, 1], F32, name="onesf_sb")
    nc.vector.memset(onesf_sb[:], 1.0)
    ones_row = res.tile([1, P], BF16, name="ones_row")
    nc.vector.memset(ones_row[:], 1.0)
    eps_col = res.tile([P, 1], F32, name="eps_col")
    nc.vector.memset(eps_col[:], EPS_LN)
    onesf_sq = res.tile([P, P], F32, name="onesf_sq")
    nc.vector.memset(onesf_sq[:], 1.0)

    # stats lhsT [128, 6, 3]: cols = [1, g2^2, g2*b2] per feature chunk
    sl3_sb = res.tile([P, FC, 3], BF16, name="sl3_sb")
    g2sq_sb = res.tile([P, FC], F32, name="g2sq_sb")
    g2b2_sb = res.tile([P, FC], F32, name="g2b2_sb")
    nc.vector.tensor_mul(g2sq_sb[:], l2g_sb[:], l2g_sb[:])
    nc.vector.tensor_mul(g2b2_sb[:], l2g_sb[:], l2b_sb[:])
    for c in range(FC):
        nc.vector.tensor_copy(sl3_sb[:, c, 0:1], ones_sb[:])
        nc.vector.tensor_copy(sl3_sb[:, c, 1:2], g2sq_sb[:, c:c + 1])
        nc.vector.tensor_copy(sl3_sb[:, c, 2:3], g2b2_sb[:, c:c + 1])

    # scalar reductions of bias/gain vectors -> [128,1] columns (value
    # replicated on every partition; [0:1] slice gives the row-space form)
    def vec_sum(name, vecs):
        tmp = res.tile([P, FC], F32, name=name + "_t")
        if len(vecs) == 1:
            nc.vector.tensor_copy(tmp[:], vecs[0][:])
        else:
            nc.vector.tensor_mul(tmp[:], vecs[0][:], vecs[1][:])
            for v in vecs[2:]:
                nc.vector.tensor_mul(tmp[:], tmp[:], v[:])
        red = res.tile([P, 1], F32, name=name + "_r")
        nc.vector.tensor_reduce(red[:], tmp[:], axis=mybir.AxisListType.X,
                                op=ALU.add)
        pR = ps_mm((P, 1))
        mm(pR[:, 0:1], onesf_sq[:], red[:], start=True, stop=True)
        arr = res.tile([P, 1], F32, name=name)
        nc.vector.tensor_copy(arr[:], pR[:, 0:1])
        return arr

    s_bo_c = vec_sum("s_bo", [bo_sb])
    s_fb2_c = vec_sum("s_fb2", [fb2_sb])
    s_g2_c = vec_sum("s_g2", [l2g_sb, l2g_sb])
    s_gb_c = vec_sum("s_gb", [l2g_sb, l2b_sb])
    s_bb_c = vec_sum("s_bb", [l2b_sb, l2b_sb])
    s_g2f_c = vec_sum("s_g2f", [l2g_sb, l2g_sb, fb2_sb])
    s_gbf_c = vec_sum("s_gbf", [l2g_sb, l2b_sb, fb2_sb])
    s_bo = s_bo_c[0:1, 0:1]

    u2rs_sb = res.tile([P, FC], BF16, name="u2rs_sb")
    b2m_sb = res.tile([1, 1], F32, name="b2m_sb")

    # per-mention outputs (feature-major): mcT cols 0:128 mention, 128:256 ctx
    mc_T = res.tile([P, FC, 2 * P], F32, name="mc_T")
    m_T = mc_T[:, :, 0:P]
    m_Tb = res.tile([P, FC, P], BF16, name="m_Tb")
    c_Tb = res.tile([P, FC, P], BF16, name="c_Tb")
    m_q = res.tile([P, FC, P], BF16, name="m_q")
    m_k = res.tile([P, FC, P], BF16, name="m_k")
    m_v = res.tile([P, FC, P], BF16, name="m_v")
    m_relik = res.tile([P, FC, P], BF16, name="m_relik")
    c_uni = res.tile([P, FC, P], BF16, name="c_uni")
    s_aa_sb = res.tile([H, P], BF16, name="s_aa_sb")
    mprod_sb = res.tile([P, FC, P], BF16, name="mprod_sb")

    # ================= phase 0: uni_w2 reduce + selector means ==========
    with tc.tile_pool(name="p0", bufs=1) as p0:
        # uni_w2 row-sums (once)
        u2_sb = p0.tile([P, FC, D], F32, name="u2_sb")
        nc.gpsimd.dma_start(u2_sb[:], t["uni_w2"].rearrange("(i p) o -> p i o", p=P))
        u2r_f = p0.tile([P, FC], F32, name="u2r_f")
        nc.vector.tensor_reduce(u2r_f[:], u2_sb[:],
                                axis=mybir.AxisListType.X, op=ALU.add)
        nc.vector.tensor_copy(u2rs_sb[:], u2r_f[:])
        ub2_sb = p0.tile([1, D], F32, name="ub2_sb")
        nc.gpsimd.dma_start(ub2_sb[:], t["uni_b2"][:])
        b2r = p0.tile([1, 1], F32, name="b2r")
        nc.vector.tensor_reduce(b2r[:], ub2_sb[:], axis=mybir.AxisListType.X,
                                op=ALU.add)
        nc.scalar.activation(b2m_sb[:], b2r[:], AF.Copy, scale=1.0 / D)

        # ---- mention/context means: feature-major selector matmuls ----
        GRP = 4
        for g in range(NCH // GRP):
            txts = []
            sels = []
            for cc in range(GRP):
                c = g * GRP + cc
                txt_c = p0.tile([P, D], FP16, tag="txtc", bufs=2 * GRP + 2,
                                name="txt_c")
                nc.sync.dma_start(txt_c[:], t["txt"][c * P:(c + 1) * P, :])
                sel_c = p0.tile([P, 2 * P], FP16, tag="selc", bufs=2 * GRP + 2,
                                name="sel_c")
                nc.sync.dma_start(sel_c[:], t["sel"][c])
                txts.append(txt_c)
                sels.append(sel_c)
            for fc in range(FC):
                pA = ps_mm((P, 2 * P))
                for cc in range(GRP):
                    mm(pA[:], txts[cc][:, ts(fc, P)], sels[cc][:],
                       start=(cc == 0), stop=(cc == GRP - 1))
                if g == 0:
                    nc.vector.tensor_copy(mc_T[:, fc, :], pA[:])
                else:
                    nc.vector.tensor_tensor(mc_T[:, fc, :], mc_T[:, fc, :],
                                            pA[:], op=ALU.add)

        nc.vector.tensor_copy(m_Tb[:], mc_T[:, :, 0:P])
        nc.vector.tensor_copy(c_Tb[:], mc_T[:, :, P:2 * P])

    wts = _ctx.enter_context(tc.tile_pool(name="wts", bufs=1))
    act = _ctx.enter_context(tc.tile_pool(name="act", bufs=1))
    lane = _ctx.enter_context(tc.tile_pool(name="lane", bufs=1))

    # ---------- per-mention projections (bf16, N=128) ----------
    def load_strip(bf_dram, oc, tag="wstrip", bufs=6):
        st = wts.tile([P, FC, P], BF16, tag=tag, bufs=bufs, name="w_strip")
        nc.sync.dma_start(st[:],
                          bf_dram[oc].rearrange("p (i q) -> p i q", q=P))
        return st

    m_T8 = res.tile([P, FC, P], FP8, name="m_T8")
    nc.scalar.activation(m_T8[:], mc_T[:, :, 0:P], AF.Copy)
    for w_r, b_sb, out_t, src in (
        ("wq_r", bq_sb, m_q, m_T8),
        ("wk_r", bk_sb, m_k, m_T8),
        ("wv_r", bv_sb, m_v, m_T8),
        (None, rb1_sb, m_relik, m_Tb),
        (None, ub1_sb, c_uni, c_Tb),
    ):
        for oc in range(FC):
            pA = ps_mm((P, P))
            if w_r is None:
                strip_src = t["w1a_s"] if out_t is m_relik else t["u1a_s"]
                st_ = load_strip(strip_src, oc)
                for ic in range(FC):
                    mm(pA[:], st_[:, ic, :], src[:, ic, :],
                       start=(ic == 0), stop=(ic == FC - 1))
                sc = 1.0
            else:
                for i in range(FC // 2):
                    mm(pA[:], w_res[w_r][:, oc, 2 * i:2 * i + 2, :],
                       src[:, 2 * i:2 * i + 2, :],
                       start=(i == 0), stop=(i == FC // 2 - 1), perf_mode=DR)
                sc = 1.0 / W_SCALE
            nc.scalar.activation(out_t[:, oc, :], pA[:], AF.Identity,
                                 bias=b_sb[:, oc:oc + 1], scale=sc)

    # s_aa [8, 128]
    for c in range(FC):
        nc.vector.tensor_mul(mprod_sb[:, c, :], m_q[:, c, :], m_k[:, c, :])
    pS = ps_score()
    for c in range(FC):
        mm(pS[:, :P], h_sb[:, c, :], mprod_sb[:, c, :],
           start=(c == 0), stop=(c == FC - 1))
    nc.any.tensor_copy(s_aa_sb[:], pS[:, :P])

    def unit(tag, name, bufs=1):
        return act.tile([P, FC, NP], BF16, tag=tag, bufs=bufs, name=name)

    def chunk_t(name):
        return act.tile([P, NP], BF16, tag="tt", bufs=3, name=name)

    # ================= macro-tile loop =================
    # cosine finish of tile t is deferred into tile t+1 so the tiny
    # transpose-back matmul doesn't head-of-line block the PE queue while
    # the pair-major lane chain drains
    pending_fin = [None]
    for mt in range(NMACRO):
        g0 = mt * G
        gsl = ds(g0, G)

        lane_seq = [0]

        def lane_t(name, parts=1, width=NP):
            lane_seq[0] += 1
            return lane.tile([parts, width], F32, tag="lnrow", bufs=3,
                             name=f"{name}_{lane_seq[0]}")

        def mview(mt_tile, c):
            """mention-side bcast view [128, G, K]"""
            return mt_tile[:, c, gsl, None].to_broadcast([P, G, K])

        # ---- candidate load + PE transpose (bf16) ----
        cand_rm = act.tile([P, 4, D], BF16, tag="cand_rm", bufs=1,
                           name="cand_rm")
        nc.sync.dma_start(
            cand_rm[:],
            t["cand"].rearrange("(q p) d -> p q d", p=P)[:, ds(4 * mt, 4), :])
        candT = unit("candT", "candT", bufs=2)
        candT8 = act.tile([P, FC, NP], FP8, tag="candT8", bufs=2,
                          name="candT8")
        for fc in range(FC):
            pT = ps_mm(dtype=BF16)
            for pc in range(4):
                nc.tensor.transpose(pT[:, ts(pc, P)],
                                    cand_rm[:, pc, ts(fc, P)], identb_sb[:])
            nc.vector.tensor_copy(candT[:, fc, :], pT[:])
            nc.scalar.activation(candT8[:, fc, :], pT[:], AF.Copy)

        # ---- k/v projections (fp8 DoubleRow) ----
        k_b = unit("B", "k_b")
        v_b = unit("C", "v_b")
        for w_r, b_sb, out_t in (("wk_r", bk_sb, k_b), ("wv_r", bv_sb, v_b)):
            for oc in range(FC):
                pA = ps_mm()
                for i in range(FC // 2):
                    mm(pA[:], w_res[w_r][:, oc, 2 * i:2 * i + 2, :],
                       candT8[:, 2 * i:2 * i + 2, :],
                       start=(i == 0), stop=(i == FC // 2 - 1), perf_mode=DR)
                nc.scalar.activation(out_t[:, oc, :], pA[:], AF.Identity,
                                     bias=b_sb[:, oc:oc + 1],
                                     scale=1.0 / W_SCALE)

        if pending_fin[0] is not None:
            pending_fin[0]()
            pending_fin[0] = None

        # ---- attention scores ----
        pAB = ps_score()
        for c in range(FC):
            pr1 = chunk_t("pr1")
            nc.vector.tensor_tensor(_gk(pr1[:]), _gk(k_b[:, c, :]),
                                    mview(m_q, c), op=ALU.mult)
            mm(pAB[:], h_sb[:, c, :], pr1[:], start=(c == 0), stop=False)
        mm(pAB[:], i8neg_sb[:],
           s_aa_sb[:, gsl, None].to_broadcast([H, G, K]),
           start=False, stop=True)
        p_ab = act.tile([H, NP], BF16, tag="p_ab", bufs=2, name="p_ab")
        nc.scalar.activation(p_ab[:], pAB[:], AF.Sigmoid, scale=ISQ)

        pBA = ps_score()
        first = True
        for c in range(FC):
            pQ = ps_mm()
            for i in range(FC // 2):
                mm(pQ[:], w_res["wq_r"][:, c, 2 * i:2 * i + 2, :],
                   candT8[:, 2 * i:2 * i + 2, :],
                   start=(i == 0), stop=(i == FC // 2 - 1), perf_mode=DR)
            q_c = chunk_t("q_c")
            nc.scalar.activation(q_c[:], pQ[:], AF.Identity,
                                 bias=bq_sb[:, c:c + 1], scale=1.0 / W_SCALE)
            pr2 = chunk_t("pr2")
            nc.vector.tensor_tensor(_gk(pr2[:]), _gk(q_c[:]), mview(m_k, c),
                                    op=ALU.mult)
            mm(pBA[:], h_sb[:, c, :], pr2[:], start=first, stop=False)
            first = False
            pr3 = chunk_t("pr3")
            nc.vector.tensor_mul(pr3[:], q_c[:], k_b[:, c, :])
            mm(pBA[:], negh_sb[:, c, :], pr3[:],
               start=False, stop=(c == FC - 1))
        p_ba = act.tile([H, NP], BF16, tag="p_ba", bufs=2, name="p_ba")
        nc.scalar.activation(p_ba[:], pBA[:], AF.Sigmoid, scale=ISQ)

        # ---- attention outputs (fp8 for the wo matmul) ----
        o_a = act.tile([P, FC, NP], FP8, tag="o8a", bufs=1, name="o_a")
        o_b = act.tile([P, FC, NP], FP8, tag="o8b", bufs=1, name="o_b")
        for c in range(FC):
            dv = chunk_t("dv")
            nc.vector.tensor_tensor(_gk(dv[:]), _gk(v_b[:, c, :]),
                                    mview(m_v, c), op=ALU.subtract)
            pBC = ps_mm()
            mm(pBC[:], ht_sb[:, c, :], p_ab[:], start=True, stop=True)
            nc.vector.tensor_mul(o_a[:, c, :], pBC[:], dv[:])
            nc.vector.tensor_tensor(_gk(o_a[:, c, :]), _gk(o_a[:, c, :]),
                                    mview(m_v, c), op=ALU.add)
            pBC2 = ps_mm()
            mm(pBC2[:], ht_sb[:, c, :], p_ba[:], start=True, stop=True)
            nc.vector.tensor_mul(o_b[:, c, :], pBC2[:], dv[:])
            nc.vector.tensor_tensor(o_b[:, c, :], v_b[:, c, :], o_b[:, c, :],
                                    op=ALU.subtract)

        # ---- wo + residual (residual folded into psum via identity mm) ----
        r_a = unit("hh", "r_a", bufs=2)
        r_b = unit("hh", "r_b", bufs=2)
        for oc in range(FC):
            pA = ps_mm()
            for i in range(FC // 2):
                mm(pA[:], w_res["wo_r"][:, oc, 2 * i:2 * i + 2, :],
                   o_a[:, 2 * i:2 * i + 2, :],
                   start=(i == 0), stop=False, perf_mode=DR)
            mm(_gk(pA[:]), identb64_sb[:],
               m_Tb[:, oc, gsl, None].to_broadcast([P, G, K]),
               start=False, stop=True)
            nc.vector.tensor_scalar_mul(r_a[:, oc, :], pA[:], 1.0 / W_SCALE)
            pB = ps_mm()
            for i in range(FC // 2):
                mm(pB[:], w_res["wo_r"][:, oc, 2 * i:2 * i + 2, :],
                   o_b[:, 2 * i:2 * i + 2, :],
                   start=(i == 0), stop=False, perf_mode=DR)
            mm(pB[:], identb64_sb[:], candT[:, oc, :],
               start=False, stop=True)
            nc.vector.tensor_scalar_mul(r_b[:, oc, :], pB[:], 1.0 / W_SCALE)

        # ---- LN1: merged stat bank, rows a:(0,32) b:(64,96) ----
        pL1 = ps_l1()
        for r_t, base in ((r_a, 0), (r_b, 64)):
            for c in range(FC):
                sq = chunk_t("sq")
                nc.scalar.activation(sq[:], r_t[:, c, :], AF.Square,
                                     bias=bo_sb[:, c:c + 1])
                mm(pL1[base:base + 1, :], ones_sb[:], r_t[:, c, :],
                   start=(c == 0), stop=(c == FC - 1),
                   tile_position=(0, base))
                mm(pL1[base + 32:base + 33, :], ones_sb[:], sq[:],
                   start=(c == 0), stop=(c == FC - 1),
                   tile_position=(0, base + 32))

        # ---- relik / unirel heads (PE filler while LN1 lane math runs) ----
        for w_r, madd, htag, wv2, bias_ap, outrow, fn, scale in (
            ("w1b_r", m_relik, "C2", rw2_sb, rb2_sb[:], 0,
             AF.Identity, 1.0),
            (None, c_uni, "D", u2rs_sb, b2m_sb[:], 2,
             AF.Sigmoid, 1.0 / D),
        ):
            h_head = unit(htag, "hh_" + htag)
            for oc in range(FC):
                if w_r is None:
                    st_u = load_strip(t["u1b_s"], oc)
                    wsl = lambda ic: st_u[:, ic, :]
                else:
                    wsl = lambda ic: w_res[w_r][:, oc, ic, :]
                pA = ps_mm()
                for ic in range(FC):
                    mm(pA[:], wsl(ic), candT[:, ic, :],
                       start=(ic == 0), stop=False)
                mm(_gk(pA[:]), identb_sb[:], mview(madd, oc),
                   start=False, stop=True)
                nc.scalar.activation(h_head[:, oc, :], pA[:], AF.Relu)
            pH = ps_head()
            for c in range(FC):
                if wv2 is rw2_sb:
                    lhsT = wv2[:, c, :]
                else:
                    lhsT = wv2[:, c:c + 1]
                mm(pH[:], lhsT, h_head[:, c, :],
                   start=(c == 0), stop=(c == FC - 1))
            osl = lane_t("osl_" + htag)
            nc.scalar.activation(osl[:], pH[:], fn, bias=bias_ap, scale=scale)
            nc.gpsimd.dma_start(t["out"][outrow:outrow + 1, ts(mt, NP)], osl[:])

        def lnrow(name):
            lane_seq[0] += 1
            return lane.tile([1, NP], F32, tag="lnrow", bufs=3,
                             name=f"{name}_{lane_seq[0]}")

        def layernorm1(r_t, x1_t, base, tok):
            mu = lnrow("mu" + tok)
            nc.vector.tensor_scalar(mu[:], pL1[base:base + 1, :], s_bo,
                                    1.0 / D, op0=ALU.add, op1=ALU.mult)
            var = lnrow("var" + tok)
            nc.vector.tensor_mul(var[:], mu[:], mu[:])
            nc.vector.scalar_tensor_tensor(var[:], pL1[base + 32:base + 33, :],
                                           1.0 / D, var[:], op0=ALU.mult,
                                           op1=ALU.subtract)
            rstd = lnrow("rstd" + tok)
            nc.scalar.activation(rstd[:], var[:], AF.Sqrt,
                                 bias=eps_col[0:1, 0:1])
            nc.vector.reciprocal(rstd[:], rstd[:])
            mubf = act.tile([1, NP], BF16, tag="mubf", bufs=1, name="mubf")
            rstdbf = act.tile([1, NP], BF16, tag="rstdbf", bufs=1,
                              name="rstdbf")
            nc.vector.tensor_copy(mubf[:], mu[:])
            nc.vector.tensor_copy(rstdbf[:], rstd[:])
            # broadcast to SBUF so the psum mm rotation isn't pinned
            # across the x1 ops (rstd is a per-column scale; bf16 error
            # cancels in LN2)
            mu_bc = act.tile([P, NP], BF16, tag="bcast", bufs=2, name="mu_bc")
            rstd_bc = act.tile([P, NP], BF16, tag="bcast", bufs=2,
                               name="rstd_bc")
            pmu = ps_mm()
            mm(pmu[:], ones_row[:], mubf[:], start=True, stop=True)
            nc.scalar.activation(mu_bc[:], pmu[:], AF.Copy)
            prs = ps_mm()
            mm(prs[:], ones_row[:], rstdbf[:], start=True, stop=True)
            nc.scalar.activation(rstd_bc[:], prs[:], AF.Copy)
            for c in range(FC):
                nc.vector.tensor_tensor(x1_t[:, c, :], r_t[:, c, :],
                                        mu_bc[:], op=ALU.subtract)
            for c in range(FC):
                nc.vector.scalar_tensor_tensor(
                    x1_t[:, c, :], x1_t[:, c, :], bo_sb[:, c:c + 1],
                    rstd_bc[:], op0=ALU.add, op1=ALU.mult)
            for c in range(FC):
                nc.scalar.activation(
                    x1_t[:, c, :], x1_t[:, c, :], AF.Identity,
                    scale=l1g_sb[:, c:c + 1], bias=l1b_sb[:, c:c + 1])

        x1_a = unit("A", "x1_a")
        x1_b = unit("Bx", "x1_b")
        layernorm1(r_a, x1_a, 0, "a")
        layernorm1(r_b, x1_b, 64, "b")

        # ---- FFN (both tokens share each weight strip) ----
        if FP8_FFN:
            x1a_8 = act.tile([P, FC, NP], FP8, tag="x1a8", bufs=1, name="x1a8")
            x1b_8 = act.tile([P, FC, NP], FP8, tag="x1b8", bufs=1, name="x1b8")
            for c in range(FC):
                nc.scalar.activation(x1a_8[:, c, :], x1_a[:, c, :], AF.Copy)
                nc.scalar.activation(x1b_8[:, c, :], x1_b[:, c, :], AF.Copy)
            h_a = act.tile([P, HFC, NP], FP8, tag="h8a", bufs=1, name="h_a")
            h_b = act.tile([P, HFC, NP], FP8, tag="h8b", bufs=1, name="h_b")

            def ha_c(hc):
                return h_a[:, hc, :]

            def hb_c(hc):
                return h_b[:, hc, :]

            for hc in range(HFC):
                st = wts.tile([P, FC, P], FP8, tag="w1strip", bufs=4,
                              name="w1_strip")
                nc.sync.dma_start(
                    st[:], t["fw1_s"][hc].rearrange("p (i q) -> p i q", q=P))
                for x8_t, hcs in ((x1a_8, ha_c), (x1b_8, hb_c)):
                    pA = ps_mm()
                    for i in range(FC // 2):
                        mm(pA[:], st[:, 2 * i:2 * i + 2, :],
                           x8_t[:, 2 * i:2 * i + 2, :],
                           start=(i == 0), stop=(i == FC // 2 - 1),
                           perf_mode=DR)
                    nc.scalar.activation(hcs(hc), pA[:], AF.Relu,
                                         bias=fb1_sb[:, hc:hc + 1],
                                         scale=1.0 / W_SCALE)
            r2_a = unit("C2", "r2_a")
            r2_b = unit("D", "r2_b")
            for oc in range(FC):
                stw = wts.tile([P, HFC, P], FP8, tag="w2strip", bufs=2,
                               name="stw")
                nc.sync.dma_start(
                    stw[:],
                    t["fw2_s"][oc].rearrange("p (i q) -> p i q", q=P))
                for x1_t, h_t, r2_t in ((x1_a, h_a, r2_a), (x1_b, h_b, r2_b)):
                    pA = ps_mm()
                    for i in range(HFC // 2):
                        mm(pA[:], stw[:, 2 * i:2 * i + 2, :],
                           h_t[:, 2 * i:2 * i + 2, :],
                           start=(i == 0), stop=(i == HFC // 2 - 1),
                           perf_mode=DR)
                    nc.vector.scalar_tensor_tensor(
                        r2_t[:, oc, :], pA[:], 1.0 / W_SCALE, x1_t[:, oc, :],
                        op0=ALU.mult, op1=ALU.add)
        else:
            h_a = act.tile([P, HFC, NP], BF16, tag="h", bufs=1, name="h_a")
            hb = [unit("candT", "hb0"), unit("G", "hb1"),
                  unit("F", "hb2"), unit("hh", "hb3", bufs=2)]

            def ha_c(hc):
                return h_a[:, hc, :]

            def hb_c(hc):
                return hb[hc // FC][:, hc % FC, :]

            for hc in range(HFC):
                st = wts.tile([P, FC, P], BF16, tag="w1strip", bufs=4,
                              name="w1_strip")
                nc.sync.dma_start(
                    st[:], t["fw1_s"][hc].rearrange("p (i q) -> p i q", q=P))
                for x1_t, hcs in ((x1_a, ha_c), (x1_b, hb_c)):
                    pA = ps_mm()
                    for ic in range(FC):
                        mm(pA[:], st[:, ic, :], x1_t[:, ic, :],
                           start=(ic == 0), stop=(ic == FC - 1))
                    nc.scalar.activation(hcs(hc), pA[:],
                                         AF.Relu, bias=fb1_sb[:, hc:hc + 1])
            r2_a = unit("C2", "r2_a")
            r2_b = unit("D", "r2_b")
            for oc in range(FC):
                stw = wts.tile([P, HFC, P], BF16, tag="w2strip", bufs=2,
                               name="stw")
                nc.sync.dma_start(
                    stw[:],
                    t["fw2_s"][oc].rearrange("p (i q) -> p i q", q=P))
                for x1_t, hcs, r2_t in ((x1_a, ha_c, r2_a), (x1_b, hb_c, r2_b)):
                    pA = ps_mm()
                    for hc in range(HFC):
                        mm(pA[:], stw[:, hc, :], hcs(hc),
                           start=(hc == 0), stop=(hc == HFC - 1))
                    nc.vector.tensor_tensor(r2_t[:, oc, :], pA[:],
                                            x1_t[:, oc, :], op=ALU.add)

        # ---- LN2 + cosine via sufficient statistics ----
        # merged stat bank rows: a:(0..2, 32..33)  b:(64..66, 96..97)
        #   base+0: [sum, g2^2, g2*b2] . y      (y = r2 + fb2, via bias)
        #   base+32: [sum, g2^2] . y^2
        pL2 = ps_l2()
        pX = ps_head()
        for c in range(FC):
            sqa = chunk_t("sq")
            nc.scalar.activation(sqa[:], r2_a[:, c, :], AF.Square,
                                 bias=fb2_sb[:, c:c + 1])
            sqb = chunk_t("sq")
            nc.scalar.activation(sqb[:], r2_b[:, c, :], AF.Square,
                                 bias=fb2_sb[:, c:c + 1])
            rr = chunk_t("rr")
            nc.vector.tensor_scalar_add(rr[:], r2_b[:, c, :],
                                        fb2_sb[:, c:c + 1])
            nc.vector.scalar_tensor_tensor(rr[:], r2_a[:, c, :],
                                           fb2_sb[:, c:c + 1], rr[:],
                                           op0=ALU.add, op1=ALU.mult)
            mm(pL2[0:3, :], sl3_sb[:, c, 0:3], r2_a[:, c, :],
               start=(c == 0), stop=(c == FC - 1), tile_position=(0, 0))
            mm(pL2[32:34, :], sl3_sb[:, c, 0:2], sqa[:],
               start=(c == 0), stop=(c == FC - 1), tile_position=(0, 32))
            mm(pL2[64:67, :], sl3_sb[:, c, 0:3], r2_b[:, c, :],
               start=(c == 0), stop=(c == FC - 1), tile_position=(0, 64))
            mm(pL2[96:98, :], sl3_sb[:, c, 0:2], sqb[:],
               start=(c == 0), stop=(c == FC - 1), tile_position=(0, 96))
            mm(pX[:], sl3_sb[:, c, 1:2], rr[:],
               start=(c == 0), stop=(c == FC - 1))

        # evict stats + pX to SBUF, transpose to pair-major [128, 4, 128]
        # (pX lands in spare transposed column 3 via [1,128]^T matmuls)
        stat_sb = act.tile([P, NP], F32, tag="stat_sb", bufs=1, name="stat_sb")
        nc.vector.tensor_copy(stat_sb[:], pL2[:])
        pX_sb = act.tile([1, NP], F32, tag="pX_sb", bufs=1, name="pX_sb")
        nc.vector.tensor_copy(pX_sb[:], pX[:])
        pT = ps_mm()
        for b in range(4):
            nc.tensor.transpose(pT[:, ts(b, P)], stat_sb[:, ts(b, P)],
                                ident_sb[:])
        for b in range(4):
            mm(pT[:, b * P + 3:b * P + 4], pX_sb[0:1, ts(b, P)],
               onesf_sb[0:1, 0:1], start=True, stop=True)
        sT = act.tile([P, 4, P], F32, tag="sT", bufs=1, name="sT")
        nc.vector.tensor_copy(sT[:], pT[:])

        # pair-major lane algebra on [128, 4] slices
        def col(j):
            return sT[:, :, j]

        def lane4(name):
            lane_seq[0] += 1
            return lane.tile([P, 4], F32, tag=name + "4", bufs=1,
                             name=f"{name}4_{lane_seq[0]}")

        def ln2_lane(base, tok):
            muz = lane4("muz" + tok)
            nc.vector.tensor_scalar(muz[:], col(base + 0), s_fb2_c[:],
                                    1.0 / D, op0=ALU.add, op1=ALU.mult)
            g2z = lane4("g2z" + tok)
            nc.vector.tensor_scalar_add(g2z[:], col(base + 1), s_g2f_c[:])
            gbz = lane4("gbz" + tok)
            nc.vector.tensor_scalar_add(gbz[:], col(base + 2), s_gbf_c[:])
            var = lane4("var2" + tok)
            nc.vector.tensor_mul(var[:], muz[:], muz[:])
            nc.vector.scalar_tensor_tensor(var[:], col(base + 32), 1.0 / D,
                                           var[:], op0=ALU.mult,
                                           op1=ALU.subtract)
            rstd = lane4("rstd2" + tok)
            nc.scalar.activation(rstd[:], var[:], AF.Sqrt, bias=eps_col[:])
            nc.vector.reciprocal(rstd[:], rstd[:])
            g2q = col(base + 33)
            return muz, rstd, g2z, gbz, g2q

        mua, rsta, g2za, gbza, g2qa = ln2_lane(0, "a")
        mub2, rstb, g2zb, gbzb, g2qb = ln2_lane(64, "b")

        def gbt(mu, rstd, gbz, name):
            o_t = lane4(name)
            nc.vector.tensor_scalar_mul(o_t[:], mu[:], s_gb_c[:])
            nc.vector.tensor_tensor(o_t[:], gbz[:], o_t[:], op=ALU.subtract)
            nc.vector.tensor_mul(o_t[:], o_t[:], rstd[:])
            return o_t

        gbta = gbt(mua, rsta, gbza, "gbta")
        gbtb = gbt(mub2, rstb, gbzb, "gbtb")

        def normsq(mu, rstd, g2z, g2q, gbt_t, name):
            o_t = lane4(name)
            nc.vector.tensor_scalar_mul(o_t[:], mu[:], s_g2_c[:])
            nc.vector.scalar_tensor_tensor(o_t[:], g2z[:], -2.0, o_t[:],
                                           op0=ALU.mult, op1=ALU.add)
            nc.vector.tensor_mul(o_t[:], o_t[:], mu[:])
            nc.vector.tensor_tensor(o_t[:], o_t[:], g2q, op=ALU.add)
            nc.vector.tensor_mul(o_t[:], o_t[:], rstd[:])
            nc.vector.tensor_mul(o_t[:], o_t[:], rstd[:])
            nc.vector.scalar_tensor_tensor(o_t[:], gbt_t[:], 2.0, o_t[:],
                                           op0=ALU.mult, op1=ALU.add)
            nc.vector.tensor_scalar_add(o_t[:], o_t[:], s_bb_c[:])
            return o_t

        n2a = normsq(mua, rsta, g2za, g2qa, gbta, "n2a")
        n2b = normsq(mub2, rstb, g2zb, g2qb, gbtb, "n2b")

        d01 = lane4("d01")
        nc.vector.tensor_scalar_mul(d01[:], mub2[:], s_g2_c[:])
        nc.vector.tensor_tensor(d01[:], d01[:], g2zb[:], op=ALU.subtract)
        nc.vector.tensor_mul(d01[:], d01[:], mua[:])
        t2 = lane4("t2")
        nc.vector.tensor_mul(t2[:], mub2[:], g2za[:])
        nc.vector.tensor_tensor(d01[:], d01[:], t2[:], op=ALU.subtract)
        nc.vector.tensor_tensor(d01[:], col(3), d01[:], op=ALU.add)
        nc.vector.tensor_mul(d01[:], d01[:], rsta[:])
        nc.vector.tensor_mul(d01[:], d01[:], rstb[:])
        nc.vector.tensor_add(d01[:], d01[:], gbta[:])
        nc.vector.tensor_add(d01[:], d01[:], gbtb[:])
        nc.vector.tensor_scalar_add(d01[:], d01[:], s_bb_c[:])

        den = lane4("den")
        nc.scalar.activation(n2a[:], n2a[:], AF.Sqrt)
        nc.vector.tensor_scalar_max(n2a[:], n2a[:], EPS_COS)
        nc.scalar.activation(n2b[:], n2b[:], AF.Sqrt)
        nc.vector.tensor_scalar_max(n2b[:], n2b[:], EPS_COS)
        nc.vector.tensor_mul(den[:], n2a[:], n2b[:])
        nc.vector.reciprocal(den[:], den[:])
        atg_T = lane4("atg_T")
        nc.vector.tensor_mul(atg_T[:], d01[:], den[:])

        # transpose back [128,4] -> [4,128] and write out (deferred)
        def _finish(atg_T=atg_T, mt=mt):
            pback = ps_mm()
            nc.tensor.transpose(pback[0:4, 0:P], atg_T[:], ident_sb[:])
            atg_row = act.tile([4, P], F32, tag="atg_row", bufs=2,
                               name="atg_row")
            nc.vector.tensor_copy(atg_row[:], pback[0:4, 0:P])
            nc.gpsimd.dma_start(
                t["out"][1:2, ts(mt, NP)].rearrange("o (b q) -> (o b) q", q=P),
                atg_row[:])

        pending_fin[0] = _finish

    pending_fin[0]()


# ===================== host side =====================

def kernel(**inputs):
    f32 = np.float32
    bf16 = ml_dtypes.bfloat16
    fp8 = ml_dtypes.float8_e4m3
    txt = np.ascontiguousarray(
        np.asarray(inputs["text_embeddings"], f32).reshape(S, D))
    cand_full = np.asarray(inputs["candidate_embeddings"], f32).reshape(M * K, D)
    cand_bf = np.ascontiguousarray(cand_full.astype(bf16))
    starts = np.asarray(inputs["mention_starts"], np.int64)
    spans = np.asarray(inputs["span_lengths"], np.int64)
    ends = starts + spans
    c_start = np.maximum(0, starts - CTX)
    c_end = np.minimum(S - 1, ends + CTX)

    def w(name):
        return np.asarray(inputs[name], f32)

    def strips_oc(wmat, n_in, n_out):
        # [in, out] -> [n_out, P, n_in*P]  (strip oc: [p, i, q])
        a = wmat.reshape(n_in, P, n_out, P)
        return np.ascontiguousarray(a.transpose(2, 1, 0, 3).reshape(
            n_out, P, n_in * P))

    def resident(wmat):
        # [in, out] -> [P, FC(oc), FC(ic), P]
        a = wmat.reshape(FC, P, FC, P)
        return np.ascontiguousarray(a.transpose(1, 2, 0, 3))

    ffn_dt = fp8 if FP8_FFN else bf16
    fscale = W_SCALE if FP8_FFN else 1.0
    consts = {
        "ident": np.eye(P, dtype=f32),
        "identb": np.eye(P, dtype=f32).astype(bf16),
        "identb64": (np.eye(P, dtype=f32) * W_SCALE).astype(bf16),
        "hmat": np.repeat(np.eye(H, dtype=f32), DH, axis=0).astype(bf16),
        "i8neg": (-np.eye(H, dtype=f32)).astype(bf16),
        "wq_r": (resident(w("wq")) * W_SCALE).astype(fp8),
        "wk_r": (resident(w("wk")) * W_SCALE).astype(fp8),
        "wv_r": (resident(w("wv")) * W_SCALE).astype(fp8),
        "wo_r": (resident(w("wo")) * W_SCALE).astype(fp8),
        "w1b_r": resident(w("relik_w1")[D:]).astype(bf16),
        "w1a_s": strips_oc(w("relik_w1")[:D], FC, FC).astype(bf16),
        "u1a_s": strips_oc(w("uni_w1")[:D], FC, FC).astype(bf16),
        "u1b_s": strips_oc(w("uni_w1")[D:], FC, FC).astype(bf16),
        "fw1_s": (strips_oc(w("ffn_w1"), FC, HFC) * fscale).astype(ffn_dt),
        "fw2_s": (strips_oc(w("ffn_w2"), HFC, FC) * fscale).astype(ffn_dt),
    }
    vnames = ["relik_b1", "relik_w2", "bq", "bk", "bv", "bo",
              "ln1_g", "ln1_b", "ffn_b1", "ffn_b2",
              "ln2_g", "ln2_b", "uni_b1", "uni_w2"]
    weights = {n: np.ascontiguousarray(np.asarray(inputs[n], f32))
               for n in vnames}
    weights["relik_b2"] = np.asarray(inputs["relik_b2"], f32).reshape(1, 1)
    weights["uni_b2"] = np.ascontiguousarray(
        np.asarray(inputs["uni_b2"], f32).reshape(1, D))

    in_maps = []
    for core in range(NCORES):
        sl = slice(core * M_LOC, (core + 1) * M_LOC)
        # selector matrices with 1/len folded (pure index metadata)
        mark = np.zeros((S + 1, 2, M_LOC), f32)
        ar = np.arange(M_LOC)
        vm = 1.0 / (spans[sl] + 1).astype(f32)
        np.add.at(mark, (starts[sl], 0, ar), vm)
        np.add.at(mark, (ends[sl] + 1, 0, ar), -vm)
        vc = 1.0 / (c_end[sl] - c_start[sl]).astype(f32)
        np.add.at(mark, (c_start[sl], 1, ar), vc)
        np.add.at(mark, (c_end[sl], 1, ar), -vc)
        selm = np.cumsum(mark[:S], axis=0).reshape(NCH, P, 2 * M_LOC)
        im = {
            "txt": txt.astype(np.float16),
            "sel": np.ascontiguousarray(selm.astype(np.float16)),
            "cand": cand_bf[core * PAIRS:(core + 1) * PAIRS],
        }
        im.update(consts)
        im.update(weights)
        in_maps.append(im)

    if "nc" not in _NC_CACHE:
        _NC_CACHE["nc"] = _build_nc()
    nc = _NC_CACHE["nc"]

    results = bass_utils.run_bass_kernel_spmd(
        nc, in_maps, core_ids=list(range(NCORES))).results

    out = np.zeros((3, M, K), f32)
    for core in range(NCORES):
        sl = slice(core * M_LOC, (core + 1) * M_LOC)
        out[:, sl, :] = results[core]["out"].reshape(3, M_LOC, K)
    return out


if __name__ == "__main__":
    nc = _build_nc()
    print("built ok")
